# revision 1
# baseline (speedup 1.0000x reference)
"""Trainium2 Bass kernel for nn_Net_MP_68805376082308 (NNConv-style GNN).

Reference computation (see problem statement):
    h = x@fc1 + b
    e2 = relu(edge_attr@k1 + b1)                     # [E, 64]
    ew = (e2 @ k2 + b2).reshape(E, 64, 64)           # never materialized here!
    for 4 iters:
        msg  = einsum('ei,eio->eo', h[src], ew)
        agg  = segment_sum(msg, dst) / max(deg,1)
        h    = relu(agg + h@root)
    out = h@fc2 + b

Device algorithm (per core, node-sharded, dst-grouped edge slots):
    e2aug[e, c]: c in 0..63 = e2*invdeg[dst], c=64 = invdeg[dst], c=65 = 0
    z[e, c*64+i]   = e2aug[e,c] * h[src[e], i]       # DVE, stride-0 bcast APs
    zsumT[ci, v]   = sum_e z[e,ci] * SegMat[e,v]     # PE, z as stationary
                                                     #   (scatter commutes with
                                                     #    the k2 contraction)
    aggT[o, v]     = T_cm.T @ zsumT + root.T @ hT    # PE
    hT             = relu(aggT)                      # ACT
    h[src] gather via SWDGE dma_gather; h exchanged across 8 cores with an
    AllGather after each iteration.

kernel(**inputs) takes the FULL unsharded inputs and returns [10000, 1] fp32.
"""

import math
import os
import sys
from dataclasses import dataclass, field

import numpy as np

sys.path.insert(0, "/opt/trn_rl_repo")

import concourse.bacc as bacc
import concourse.bass as bass
import concourse.mybir as mybir
import concourse.tile as tile
from concourse import library_config

F32 = mybir.dt.float32
F16 = mybir.dt.float16
I16 = mybir.dt.int16

WIDTH = 64
DEPTH = 4


@dataclass
class Plan:
    """Host-side preprocessing result: all per-core device input arrays plus
    the compile-time structure constants."""

    n_cores: int
    n_windows: int          # total scatter windows
    wpc: int                # windows per core
    nt: int                 # edge tiles (128 slots) per window
    nodes_pad: int          # n_windows * win
    depth: int
    win: int = 128          # nodes per scatter window
    nchunk: int = 33        # ci chunks of 128 (66*64/128)
    devnode: np.ndarray = None     # [N] original node -> device row
    in_maps: list = field(default_factory=list)
    fc2_b: float = 0.0

    @property
    def ntiles(self):       # edge tiles per core
        return self.wpc * self.nt

    @property
    def epc(self):          # edge slots per core
        return self.ntiles * 128


def make_plan(x, edge_index, edge_attr, fc1_W, fc1_b, k1_W, k1_b, k2_W, k2_b,
              root, conv_b, fc2_W, fc2_b, n_cores=8, depth=DEPTH):
    W = WIDTH
    N = x.shape[0]
    E = edge_index.shape[1]
    src = np.asarray(edge_index[0], dtype=np.int64)
    dst = np.asarray(edge_index[1], dtype=np.int64)
    assert np.all(np.asarray(conv_b) == 0.0), "kernel assumes conv_b == 0"

    WIN = 128
    n_windows = n_cores * max(1, int(math.ceil(N / WIN / n_cores)))
    nodes_pad = n_windows * WIN
    wpc = n_windows // n_cores

    counts = np.bincount(dst, minlength=N).astype(np.float64)
    denom = np.where(counts > 0, counts, 1.0)
    invdeg_node = (1.0 / denom).astype(np.float32)

    # Greedy balance: nodes into windows (64 slots each), minimizing the max
    # edge count per window.
    order = np.argsort(-counts, kind="stable")
    win_edges = np.zeros(n_windows, dtype=np.int64)
    win_fill = np.zeros(n_windows, dtype=np.int64)
    node_window = np.zeros(N, dtype=np.int64)
    node_slot = np.zeros(N, dtype=np.int64)
    # vectorized-ish greedy: iterate nodes, pick least-loaded window with room
    INF = 1 << 60
    load = win_edges.copy()
    for n in order:
        w = int(np.argmin(load))
        node_window[n] = w
        node_slot[n] = win_fill[w]
        win_fill[w] += 1
        win_edges[w] += counts[n]
        load[w] = win_edges[w] if win_fill[w] < WIN else INF
    nt = int(math.ceil(win_edges.max() / 128))
    eslot_w = nt * 128

    devnode = node_window * WIN + node_slot

    # edge -> slot within its dst window
    edge_win = node_window[dst]
    ord_e = np.argsort(edge_win, kind="stable")
    fill = np.zeros(n_windows, dtype=np.int64)
    eslot = np.zeros(E, dtype=np.int64)
    for e in ord_e:
        w = edge_win[e]
        eslot[e] = w * eslot_w + fill[w]
        fill[w] += 1
    assert fill.max() <= eslot_w

    tot_slots = n_windows * eslot_w
    slot_src = np.zeros(tot_slots, dtype=np.int64)
    slot_used = np.zeros(tot_slots, dtype=bool)
    slot_vloc = np.zeros(tot_slots, dtype=np.int64)
    slot_invdeg = np.zeros(tot_slots, dtype=np.float32)
    slot_ea = np.zeros((tot_slots, 3), dtype=np.float32)
    slot_src[eslot] = devnode[src]
    slot_used[eslot] = True
    slot_vloc[eslot] = node_slot[dst]
    slot_invdeg[eslot] = invdeg_node[dst]
    slot_ea[eslot] = np.asarray(edge_attr, dtype=np.float32)

    # weight repacks
    T_cm = np.zeros((66 * 64, W), dtype=np.float32)
    T_cm[: 64 * 64] = np.ascontiguousarray(
        np.asarray(k2_W, dtype=np.float32).reshape(64, 64, 64)
    ).reshape(64 * 64, 64)
    T_cm[64 * 64 : 65 * 64] = np.asarray(k2_b, dtype=np.float32).reshape(64, 64)
    nchunk = 66 * 64 // 128  # 33
    # chunk layout for SBUF: T_sb[p, k*64+o] = T_cm[k*128+p, o]
    T_sb = np.ascontiguousarray(
        T_cm.reshape(nchunk, 128, W).transpose(1, 0, 2)
    ).reshape(128, nchunk * W).astype(np.float16)

    k1_Wb = np.concatenate(
        [np.asarray(k1_W, dtype=np.float32),
         np.asarray(k1_b, dtype=np.float32)[None, :]], axis=0)  # [4, 64]

    h0 = np.zeros((nodes_pad, W), dtype=np.float32)
    h0[devnode] = np.asarray(x, np.float32) @ np.asarray(fc1_W, np.float32) \
        + np.asarray(fc1_b, np.float32)

    ident = np.eye(64, dtype=np.float32)
    root_np = np.asarray(root, dtype=np.float32)
    fc2_np = np.asarray(fc2_W, dtype=np.float32).reshape(W, 1)

    plan = Plan(n_cores=n_cores, n_windows=n_windows, wpc=wpc, nt=nt,
                nodes_pad=nodes_pad, depth=depth, nchunk=nchunk, win=WIN,
                devnode=devnode, fc2_b=float(np.asarray(fc2_b).reshape(())))

    epc = plan.epc
    ntiles = plan.ntiles
    for r in range(n_cores):
        sl = slice(r * epc, (r + 1) * epc)
        c_ea = slot_ea[sl]
        c_used = slot_used[sl]
        c_invd = slot_invdeg[sl]
        c_vloc = slot_vloc[sl]
        c_src = slot_src[sl]

        eaT = np.zeros((4, epc), dtype=np.float32)
        eaT[:3] = c_ea.T
        eaT[3] = 1.0
        # invdeg in [partition, tile] layout (slot s -> (s//128, s%128))
        invd = np.ascontiguousarray(
            c_invd.reshape(ntiles, 128).T)                       # [128, ntiles]
        segT = np.zeros((ntiles, 128, WIN), dtype=np.float16)
        tt = np.arange(epc) // 128
        pp = np.arange(epc) % 128
        segT[tt[c_used], pp[c_used], c_vloc[c_used]] = 1.0
        segT = np.ascontiguousarray(segT.transpose(1, 0, 2)).reshape(128, ntiles * WIN)

        idx = np.zeros((128, epc // 16), dtype=np.int16)
        base = c_src.astype(np.int16).reshape(epc // 16, 16).T   # [16, epc/16]
        for g in range(8):
            idx[16 * g : 16 * (g + 1)] = base

        h0T = np.ascontiguousarray(
            h0[r * wpc * WIN : (r + 1) * wpc * WIN].T)           # [64, wpc*WIN]

        plan.in_maps.append({
            "eaT": eaT,
            "invdeg": invd,
            "segmatT": segT,
            "idx": idx,
            "h0": h0,
            "h0T": h0T,
            "T_sb": T_sb,
            "k1_Wb": k1_Wb,
            "root": root_np,
            "fc2_W": fc2_np,
            "fc2_b": np.full((WIN, 1), plan.fc2_b, dtype=np.float32),
            "ident": ident,
        })
    return plan


def build_program(plan: Plan, debug=False, single_core=False):
    """Build the SPMD Bass program (one program, run on all cores).

    single_core=True replaces the AllGather with a local DRAM copy (and drops
    addr_space="Shared") so the program can run under TimelineSim for cost
    modeling. Results are numerically wrong in that mode; timing is
    representative minus ~10us per skipped collective."""
    W = WIDTH
    NT = plan.nt
    WPC = plan.wpc
    WIN = plan.win
    NTILES = plan.ntiles
    EPC = plan.epc
    NCH = plan.nchunk
    KH = 8                  # chunks per PSUM pass (8*WIN f32 = 2 banks, so the
                            # pass tile can double-buffer within 8 banks)
    NPAD = plan.nodes_pad
    DEP = plan.depth
    NC_ = plan.n_cores
    Relu = mybir.ActivationFunctionType.Relu

    nc = bacc.Bacc("TRN2", target_bir_lowering=False, debug=debug,
                   num_devices=NC_)

    # ---- I/O ----
    eaT_d = nc.dram_tensor("eaT", [4, EPC], F32, kind="ExternalInput")
    invd_d = nc.dram_tensor("invdeg", [128, NTILES], F32, kind="ExternalInput")
    segT_d = nc.dram_tensor("segmatT", [128, NTILES * WIN], F16, kind="ExternalInput")
    idx_d = nc.dram_tensor("idx", [128, EPC // 16], I16, kind="ExternalInput")
    h0_d = nc.dram_tensor("h0", [NPAD, W], F32, kind="ExternalInput")
    h0T_d = nc.dram_tensor("h0T", [W, WPC * WIN], F32, kind="ExternalInput")
    Tsb_d = nc.dram_tensor("T_sb", [128, NCH * W], F16, kind="ExternalInput")
    k1_d = nc.dram_tensor("k1_Wb", [4, W], F32, kind="ExternalInput")
    root_d = nc.dram_tensor("root", [W, W], F32, kind="ExternalInput")
    fc2_d = nc.dram_tensor("fc2_W", [W, 1], F32, kind="ExternalInput")
    fc2b_d = nc.dram_tensor("fc2_b", [WIN, 1], F32, kind="ExternalInput")
    id_d = nc.dram_tensor("ident", [64, 64], F32, kind="ExternalInput")
    y_d = nc.dram_tensor("y", [WPC * WIN, 1], F32, kind="ExternalOutput")

    # internal DRAM for the h exchange
    h_slice = [nc.dram_tensor(f"h_slice{i}", [WPC * WIN, W], F32)
               for i in range(DEP - 1)]
    if single_core:
        h_full = [nc.dram_tensor(f"h_full{i}", [NPAD, W], F32)
                  for i in range(DEP - 1)]
    else:
        h_full = [nc.dram_tensor(f"h_full{i}", [NPAD, W], F32,
                                 addr_space="Shared")
                  for i in range(DEP - 1)]

    with tile.TileContext(nc) as tc:
        with (
            tc.tile_pool(name="const", bufs=1) as cpool,
            tc.tile_pool(name="hsrc", bufs=2) as hsrc_pool,
            tc.tile_pool(name="z", bufs=2 * plan.nt + 1) as zpool,
            tc.tile_pool(name="zsum_sb", bufs=2) as zsum_sb_pool,
            tc.tile_pool(name="hT", bufs=2) as hT_pool,
            tc.tile_pool(name="small", bufs=4) as spool,
            tc.tile_pool(name="zsum_ps", bufs=2, space="PSUM") as zsum_ps_pool,
            tc.tile_pool(name="agg_ps", bufs=2, space="PSUM") as agg_ps_pool,
            tc.tile_pool(name="tr_ps", bufs=1, space="PSUM") as tr_ps_pool,
            # agg_ps holds every [128,64]-or-smaller PSUM tile under ONE tag
            # ("a") so the pool stays at 2 banks; tr_ps holds the transpose
            # output (1 bank). Total: 5 + 2 + 1 = 8 banks.
        ):
            nc.gpsimd.load_library(library_config.mlp)

            # ---- load constants ----
            eaT = cpool.tile([4, EPC], F32)
            nc.sync.dma_start(eaT[:], eaT_d[:])
            invd = cpool.tile([128, NTILES], F32)
            nc.sync.dma_start(invd[:], invd_d[:])
            segT = cpool.tile([128, NTILES * WIN], F16)
            nc.sync.dma_start(segT[:], segT_d[:])
            idx = cpool.tile([128, EPC // 16], I16)
            nc.sync.dma_start(idx[:], idx_d[:])
            Tsb = cpool.tile([128, NCH * W], F16)
            nc.sync.dma_start(Tsb[:], Tsb_d[:])
            k1 = cpool.tile([4, W], F32)
            nc.sync.dma_start(k1[:], k1_d[:])
            rootW = cpool.tile([W, W], F32)
            nc.sync.dma_start(rootW[:], root_d[:])
            fc2 = cpool.tile([W, 1], F32)
            nc.sync.dma_start(fc2[:], fc2_d[:])
            fc2b = cpool.tile([WIN, 1], F32)
            nc.sync.dma_start(fc2b[:], fc2b_d[:])
            ident = cpool.tile([64, 64], F32)
            nc.sync.dma_start(ident[:], id_d[:])

            # ---- e2aug: [128, NTILES*66] fp32 ----
            e2aug = cpool.tile([128, NTILES * 66], F32)
            nc.vector.memset(e2aug[:], 0.0)
            for t in range(NTILES):
                e2_ps = agg_ps_pool.tile([128, W], F32, tag="a")
                nc.tensor.matmul(e2_ps[:], eaT[:, t * 128:(t + 1) * 128],
                                 k1[:], start=True, stop=True)
                # relu into e2aug cols [t*66, t*66+64)
                nc.scalar.activation(e2aug[:, t * 66: t * 66 + 64], e2_ps[:], Relu)
                # scale by invdeg (per-partition scalar)
                nc.vector.tensor_scalar_mul(
                    e2aug[:, t * 66: t * 66 + 64],
                    e2aug[:, t * 66: t * 66 + 64],
                    invd[:, t: t + 1])
                # col 64 = invdeg
                nc.vector.tensor_copy(
                    e2aug[:, t * 66 + 64: t * 66 + 65], invd[:, t: t + 1])

            hT_cur = cpool.tile([W, WPC * WIN], F32)
            nc.sync.dma_start(hT_cur[:], h0T_d[:])

            for it in range(DEP):
                gather_src = h0_d if it == 0 else h_full[it - 1]
                h_src = hsrc_pool.tile([128, NTILES, W], F32)
                # <=512 idx per call: a single huge gather overflows the
                # SWDGE descriptor ring and faults NRT.
                GCH = 512
                for o in range(0, EPC, GCH):
                    n = min(GCH, EPC - o)
                    nc.gpsimd.dma_gather(
                        h_src[:, o // 128:(o + n) // 128, :], gather_src[:],
                        idx[:, o // 16:(o + n) // 16], n, n, W)

                hT_next = hT_pool.tile([W, WPC * WIN], F32)
                for w in range(WPC):
                    zs = []
                    for et in range(NT):
                        t = w * NT + et
                        z = zpool.tile([128, 66 * 64], F16)
                        zv = z[:].rearrange("p (c i) -> p c i", c=66)
                        hs = h_src[:, t, :].unsqueeze(1).broadcast_to((128, 66, 64))
                        e2 = e2aug[:, t * 66:(t + 1) * 66].unsqueeze(2) \
                            .broadcast_to((128, 66, 64))
                        nc.vector.tensor_mul(zv, hs, e2)
                        zs.append(z)
                    # chunk-major: each PSUM accumulation group runs to
                    # completion before the next opens — start=True clears
                    # has_written for the WHOLE bank, so groups sharing a
                    # bank must never interleave. zsumT [128, NCH*WIN] f32
                    # exceeds PSUM, so run the chunks in two passes over the
                    # SBUF-resident z tiles.
                    zsum_sb = zsum_sb_pool.tile([128, NCH * WIN], F16)
                    for p0 in range(0, NCH, KH):
                        p1 = min(p0 + KH, NCH)
                        zsum_ps = zsum_ps_pool.tile([128, KH * WIN], F32)
                        for k in range(p0, p1):
                            for et in range(NT):
                                nc.tensor.matmul(
                                    zsum_ps[:, (k - p0) * WIN:(k - p0 + 1) * WIN],
                                    zs[et][:, k * 128:(k + 1) * 128],
                                    segT[:, (w * NT + et) * WIN:(w * NT + et + 1) * WIN],
                                    start=(et == 0), stop=(et == NT - 1))
                        # keep the DVE free for z-builds (critical engine) —
                        # drain PSUM on ACT
                        nc.scalar.copy(zsum_sb[:, p0 * WIN:p1 * WIN],
                                       zsum_ps[:, :(p1 - p0) * WIN])

                    agg_ps = agg_ps_pool.tile([64, WIN], F32, tag="a")
                    for k in range(NCH):
                        nc.tensor.matmul(agg_ps[:],
                                         Tsb[:, k * W:(k + 1) * W],
                                         zsum_sb[:, k * WIN:(k + 1) * WIN],
                                         start=(k == 0), stop=False)
                    nc.tensor.matmul(agg_ps[:], rootW[:],
                                     hT_cur[:, w * WIN:(w + 1) * WIN],
                                     start=False, stop=True)
                    nc.scalar.activation(hT_next[:, w * WIN:(w + 1) * WIN],
                                         agg_ps[:], Relu)
                    if it < DEP - 1:
                        h_ps = tr_ps_pool.tile([WIN, 64], F32)
                        nc.tensor.transpose(h_ps[:],
                                            hT_next[:, w * WIN:(w + 1) * WIN],
                                            ident[:])
                        h_sb = spool.tile([WIN, 64], F32, tag="hnew")
                        nc.scalar.copy(h_sb[:], h_ps[:])
                        nc.sync.dma_start(h_slice[it][w * WIN:(w + 1) * WIN, :],
                                          h_sb[:])
                hT_cur = hT_next
                if it < DEP - 1:
                    if single_core:
                        nc.sync.dma_start(h_full[it][: WPC * WIN, :],
                                          h_slice[it][:])
                    else:
                        nc.gpsimd.collective_compute(
                            "AllGather",
                            mybir.AluOpType.bypass,
                            ins=[h_slice[it][:].opt()],
                            outs=[h_full[it][:].opt()],
                            replica_groups=[list(range(NC_))],
                        )

            # ---- epilogue: y = h @ fc2 + b ----
            y_sb = spool.tile([WIN, WPC], F32, tag="y")
            for w in range(WPC):
                y_ps = agg_ps_pool.tile([WIN, 1], F32, tag="a")
                nc.tensor.matmul(y_ps[:], hT_cur[:, w * WIN:(w + 1) * WIN],
                                 fc2[:], start=True, stop=True)
                nc.vector.tensor_add(y_sb[:, w: w + 1], y_ps[:], fc2b[:])
            y_view = y_d[:].rearrange("(w v) o -> v (w o)", w=WPC)
            nc.sync.dma_start(y_view, y_sb[:])

    nc.compile()
    return nc


def bench(inputs, iters=20):
    """Jit the SPMD program once, then time repeated executions with
    device-resident inputs. Returns (output, per-exec seconds list)."""
    import time

    import jax
    from jax.sharding import Mesh, PartitionSpec
    from jax.experimental.shard_map import shard_map
    from concourse import bass2jax
    from concourse.bass2jax import _bass_exec_p, partition_id_tensor

    bass2jax.install_neuronx_cc_hook()

    plan = make_plan(**{k: np.asarray(v) for k, v in inputs.items()})
    nc = build_program(plan)
    n_cores = plan.n_cores
    in_maps = plan.in_maps

    partition_name = nc.partition_id_tensor.name if nc.partition_id_tensor else None
    in_names, out_names, out_avals, zero_outs = [], [], [], []
    for alloc in nc.m.functions[0].allocations:
        if not isinstance(alloc, mybir.MemoryLocationSet):
            continue
        name = alloc.memorylocations[0].name
        if alloc.kind == "ExternalInput":
            if name != partition_name:
                in_names.append(name)
        elif alloc.kind == "ExternalOutput":
            shape = tuple(alloc.tensor_shape)
            dtype = mybir.dt.np(alloc.dtype)
            out_names.append(name)
            out_avals.append(jax.core.ShapedArray(shape, dtype))
            zero_outs.append(np.zeros(shape, dtype))
    n_params = len(in_names)
    all_in_names = list(in_names) + list(out_names)
    if partition_name is not None:
        all_in_names.append(partition_name)

    def _body(*args):
        operands = list(args)
        if partition_name is not None:
            operands.append(partition_id_tensor())
        return tuple(_bass_exec_p.bind(
            *operands,
            out_avals=tuple(out_avals),
            in_names=tuple(all_in_names),
            out_names=tuple(out_names),
            lowering_input_output_aliases=(),
            sim_require_finite=True,
            sim_require_nnan=True,
            nc=nc,
        ))

    devices = jax.devices()[:n_cores]
    mesh = Mesh(np.asarray(devices), ("core",))
    in_specs = (PartitionSpec("core"),) * (n_params + len(out_names))
    out_specs = (PartitionSpec("core"),) * len(out_names)
    sharded = jax.jit(shard_map(_body, mesh=mesh, in_specs=in_specs,
                                out_specs=out_specs, check_rep=False),
                      keep_unused=True)

    concat_in = [np.concatenate([np.asarray(in_maps[c][n]) for c in range(n_cores)],
                                axis=0) for n in in_names]
    concat_zeros = [np.zeros((n_cores * z.shape[0], *z.shape[1:]), z.dtype)
                    for z in zero_outs]
    dev_in = [jax.device_put(a) for a in concat_in]
    dev_zero = [jax.device_put(a) for a in concat_zeros]

    out = sharded(*dev_in, *dev_zero)  # compile + first exec
    jax.block_until_ready(out)

    times = []
    for _ in range(iters):
        t0 = time.perf_counter()
        out = sharded(*dev_in, *dev_zero)
        jax.block_until_ready(out)
        times.append(time.perf_counter() - t0)

    y_all = np.asarray(out[out_names.index("y")]).reshape(n_cores, -1, 1)
    y = np.concatenate([y_all[c] for c in range(n_cores)], axis=0)
    return y[plan.devnode], times


def kernel(**inputs) -> np.ndarray:
    from concourse.bass_utils import run_bass_kernel_spmd

    plan = make_plan(**{k: np.asarray(v) for k, v in inputs.items()})
    nc = build_program(plan)
    core_ids = list(range(plan.n_cores))
    res = run_bass_kernel_spmd(nc, plan.in_maps, core_ids,
                               trace=bool(int(os.environ.get("KERNEL_TRACE", "0"))))
    y = np.concatenate([res.results[r]["y"] for r in range(plan.n_cores)], axis=0)
    out = y[plan.devnode]
    kernel.last_results = res
    kernel.last_plan = plan
    return out



# revision 3
# speedup vs baseline: 1.7879x; 1.7879x over previous
"""Trainium2 Bass kernel for nn_Net_MP_68805376082308 (NNConv-style GNN).

Reference computation (see problem statement):
    h = x@fc1 + b
    e2 = relu(edge_attr@k1 + b1)                     # [E, 64]
    ew = (e2 @ k2 + b2).reshape(E, 64, 64)           # never materialized here!
    for 4 iters:
        msg  = einsum('ei,eio->eo', h[src], ew)
        agg  = segment_sum(msg, dst) / max(deg,1)
        h    = relu(agg + h@root)
    out = h@fc2 + b

Device algorithm (per core, node-sharded, dst-grouped edge slots):
    e2aug[e, c]: c in 0..63 = e2*invdeg[dst], c=64 = invdeg[dst], c=65 = 0
    z[e, c*64+i]   = e2aug[e,c] * h[src[e], i]       # DVE, stride-0 bcast APs
    zsumT[ci, v]   = sum_e z[e,ci] * SegMat[e,v]     # PE, z as stationary
                                                     #   (scatter commutes with
                                                     #    the k2 contraction)
    aggT[o, v]     = T_cm.T @ zsumT + root.T @ hT    # PE
    hT             = relu(aggT)                      # ACT
    h[src] gather via SWDGE dma_gather; h exchanged across 8 cores with an
    AllGather after each iteration.

kernel(**inputs) takes the FULL unsharded inputs and returns [10000, 1] fp32.
"""

import math
import os
import sys
from dataclasses import dataclass, field

import numpy as np

sys.path.insert(0, "/opt/trn_rl_repo")

import concourse.bacc as bacc
import concourse.bass as bass
import concourse.mybir as mybir
import concourse.tile as tile
from concourse import library_config

F32 = mybir.dt.float32
F16 = mybir.dt.float16
I16 = mybir.dt.int16

WIDTH = 64
DEPTH = 4


@dataclass
class Plan:
    """Host-side preprocessing result: all per-core device input arrays plus
    the compile-time structure constants."""

    n_cores: int
    n_windows: int          # total scatter windows
    wpc: int                # windows per core
    nt: int                 # edge tiles (128 slots) per window
    nodes_pad: int          # n_windows * win
    depth: int
    win: int = 128          # nodes per scatter window
    nchunk: int = 33        # ci chunks of 128 (66*64/128)
    devnode: np.ndarray = None     # [N] original node -> device row
    in_maps: list = field(default_factory=list)
    fc2_b: float = 0.0

    @property
    def ntiles(self):       # edge tiles per core
        return self.wpc * self.nt

    @property
    def epc(self):          # edge slots per core
        return self.ntiles * 128


def make_plan(x, edge_index, edge_attr, fc1_W, fc1_b, k1_W, k1_b, k2_W, k2_b,
              root, conv_b, fc2_W, fc2_b, n_cores=8, depth=DEPTH):
    W = WIDTH
    N = x.shape[0]
    E = edge_index.shape[1]
    src = np.asarray(edge_index[0], dtype=np.int64)
    dst = np.asarray(edge_index[1], dtype=np.int64)
    assert np.all(np.asarray(conv_b) == 0.0), "kernel assumes conv_b == 0"

    WIN = 128
    n_windows = n_cores * max(1, int(math.ceil(N / WIN / n_cores)))
    nodes_pad = n_windows * WIN
    wpc = n_windows // n_cores

    counts = np.bincount(dst, minlength=N).astype(np.float64)
    denom = np.where(counts > 0, counts, 1.0)
    invdeg_node = (1.0 / denom).astype(np.float32)

    # Greedy balance: nodes into windows (64 slots each), minimizing the max
    # edge count per window.
    order = np.argsort(-counts, kind="stable")
    win_edges = np.zeros(n_windows, dtype=np.int64)
    win_fill = np.zeros(n_windows, dtype=np.int64)
    node_window = np.zeros(N, dtype=np.int64)
    node_slot = np.zeros(N, dtype=np.int64)
    # vectorized-ish greedy: iterate nodes, pick least-loaded window with room
    INF = 1 << 60
    load = win_edges.copy()
    for n in order:
        w = int(np.argmin(load))
        node_window[n] = w
        node_slot[n] = win_fill[w]
        win_fill[w] += 1
        win_edges[w] += counts[n]
        load[w] = win_edges[w] if win_fill[w] < WIN else INF
    nt = int(math.ceil(win_edges.max() / 128))
    eslot_w = nt * 128

    devnode = node_window * WIN + node_slot

    # edge -> slot within its dst window
    edge_win = node_window[dst]
    ord_e = np.argsort(edge_win, kind="stable")
    fill = np.zeros(n_windows, dtype=np.int64)
    eslot = np.zeros(E, dtype=np.int64)
    for e in ord_e:
        w = edge_win[e]
        eslot[e] = w * eslot_w + fill[w]
        fill[w] += 1
    assert fill.max() <= eslot_w

    tot_slots = n_windows * eslot_w
    slot_src = np.zeros(tot_slots, dtype=np.int64)
    slot_used = np.zeros(tot_slots, dtype=bool)
    slot_vloc = np.zeros(tot_slots, dtype=np.int64)
    slot_invdeg = np.zeros(tot_slots, dtype=np.float32)
    slot_ea = np.zeros((tot_slots, 3), dtype=np.float32)
    slot_src[eslot] = devnode[src]
    slot_used[eslot] = True
    slot_vloc[eslot] = node_slot[dst]
    slot_invdeg[eslot] = invdeg_node[dst]
    slot_ea[eslot] = np.asarray(edge_attr, dtype=np.float32)

    # weight repacks
    T_cm = np.zeros((66 * 64, W), dtype=np.float32)
    T_cm[: 64 * 64] = np.ascontiguousarray(
        np.asarray(k2_W, dtype=np.float32).reshape(64, 64, 64)
    ).reshape(64 * 64, 64)
    T_cm[64 * 64 : 65 * 64] = np.asarray(k2_b, dtype=np.float32).reshape(64, 64)
    nchunk = 66 * 64 // 128  # 33
    # chunk layout for SBUF: T_sb[p, k*64+o] = T_cm[k*128+p, o]
    T_sb = np.ascontiguousarray(
        T_cm.reshape(nchunk, 128, W).transpose(1, 0, 2)
    ).reshape(128, nchunk * W).astype(np.float16)

    k1_Wb = np.concatenate(
        [np.asarray(k1_W, dtype=np.float32),
         np.asarray(k1_b, dtype=np.float32)[None, :]], axis=0).astype(np.float16)

    # h rows padded to 128 f16 (=256B) so SWDGE dma_gather's 256B-multiple
    # row-stride restriction is met; cols 64.. are never read by compute.
    h0 = np.zeros((nodes_pad, 2 * W), dtype=np.float16)
    h0[devnode, :W] = (np.asarray(x, np.float32) @ np.asarray(fc1_W, np.float32)
                       + np.asarray(fc1_b, np.float32)).astype(np.float16)

    ident = np.eye(64, dtype=np.float16)
    root_np = np.asarray(root, dtype=np.float16)
    fc2_np = np.asarray(fc2_W, dtype=np.float16).reshape(W, 1)

    plan = Plan(n_cores=n_cores, n_windows=n_windows, wpc=wpc, nt=nt,
                nodes_pad=nodes_pad, depth=depth, nchunk=nchunk, win=WIN,
                devnode=devnode, fc2_b=float(np.asarray(fc2_b).reshape(())))

    epc = plan.epc
    ntiles = plan.ntiles
    for r in range(n_cores):
        sl = slice(r * epc, (r + 1) * epc)
        c_ea = slot_ea[sl]
        c_used = slot_used[sl]
        c_invd = slot_invdeg[sl]
        c_vloc = slot_vloc[sl]
        c_src = slot_src[sl]

        eaT = np.zeros((4, epc), dtype=np.float16)
        eaT[:3] = c_ea.T.astype(np.float16)
        eaT[3] = 1.0
        # invdeg in [partition, tile] layout (slot s -> (s//128, s%128))
        invd = np.ascontiguousarray(
            c_invd.reshape(ntiles, 128).T)                       # [128, ntiles]
        segT = np.zeros((ntiles, 128, WIN), dtype=np.float16)
        tt = np.arange(epc) // 128
        pp = np.arange(epc) % 128
        segT[tt[c_used], pp[c_used], c_vloc[c_used]] = 1.0
        segT = np.ascontiguousarray(segT.transpose(1, 0, 2)).reshape(128, ntiles * WIN)

        idx = np.zeros((128, epc // 16), dtype=np.int16)
        base = c_src.astype(np.int16).reshape(epc // 16, 16).T   # [16, epc/16]
        for g in range(8):
            idx[16 * g : 16 * (g + 1)] = base

        h0T = np.ascontiguousarray(
            h0[r * wpc * WIN : (r + 1) * wpc * WIN, :W].T)       # [64, wpc*WIN]

        plan.in_maps.append({
            "eaT": eaT,
            "invdeg": invd,
            "segmatT": segT,
            "idx": idx,
            "h0": h0,
            "h0T": h0T,
            "T_sb": T_sb,
            "k1_Wb": k1_Wb,
            "root": root_np,
            "fc2_W": fc2_np,
            "fc2_b": np.full((WIN, 1), plan.fc2_b, dtype=np.float32),
            "ident": ident,
        })
    return plan


def build_program(plan: Plan, debug=False, single_core=False):
    """Build the SPMD Bass program (one program, run on all cores).

    single_core=True replaces the AllGather with a local DRAM copy (and drops
    addr_space="Shared") so the program can run under TimelineSim for cost
    modeling. Results are numerically wrong in that mode; timing is
    representative minus ~10us per skipped collective."""
    W = WIDTH
    NT = plan.nt
    WPC = plan.wpc
    WIN = plan.win
    NTILES = plan.ntiles
    EPC = plan.epc
    NCH = plan.nchunk
    KH = 8                  # chunks per PSUM pass (8*WIN f32 = 2 banks, so the
                            # pass tile can double-buffer within 8 banks)
    NPAD = plan.nodes_pad
    DEP = plan.depth
    NC_ = plan.n_cores
    Relu = mybir.ActivationFunctionType.Relu

    nc = bacc.Bacc("TRN2", target_bir_lowering=False, debug=debug,
                   num_devices=NC_)

    # ---- I/O ----
    eaT_d = nc.dram_tensor("eaT", [4, EPC], F16, kind="ExternalInput")
    invd_d = nc.dram_tensor("invdeg", [128, NTILES], F32, kind="ExternalInput")
    segT_d = nc.dram_tensor("segmatT", [128, NTILES * WIN], F16, kind="ExternalInput")
    idx_d = nc.dram_tensor("idx", [128, EPC // 16], I16, kind="ExternalInput")
    h0_d = nc.dram_tensor("h0", [NPAD, 2 * W], F16, kind="ExternalInput")
    h0T_d = nc.dram_tensor("h0T", [W, WPC * WIN], F16, kind="ExternalInput")
    Tsb_d = nc.dram_tensor("T_sb", [128, NCH * W], F16, kind="ExternalInput")
    k1_d = nc.dram_tensor("k1_Wb", [4, W], F16, kind="ExternalInput")
    root_d = nc.dram_tensor("root", [W, W], F16, kind="ExternalInput")
    fc2_d = nc.dram_tensor("fc2_W", [W, 1], F16, kind="ExternalInput")
    fc2b_d = nc.dram_tensor("fc2_b", [WIN, 1], F32, kind="ExternalInput")
    id_d = nc.dram_tensor("ident", [64, 64], F16, kind="ExternalInput")
    y_d = nc.dram_tensor("y", [WPC * WIN, 1], F32, kind="ExternalOutput")

    # internal DRAM for the h exchange
    h_slice = [nc.dram_tensor(f"h_slice{i}", [WPC * WIN, 2 * W], F16)
               for i in range(DEP - 1)]
    if single_core:
        h_full = [nc.dram_tensor(f"h_full{i}", [NPAD, 2 * W], F16)
                  for i in range(DEP - 1)]
    else:
        h_full = [nc.dram_tensor(f"h_full{i}", [NPAD, 2 * W], F16,
                                 addr_space="Shared")
                  for i in range(DEP - 1)]

    with tile.TileContext(nc) as tc:
        with (
            tc.tile_pool(name="const", bufs=1) as cpool,
            tc.tile_pool(name="hsrc", bufs=2) as hsrc_pool,
            tc.tile_pool(name="z", bufs=2 * plan.nt + 1) as zpool,
            tc.tile_pool(name="zsum_sb", bufs=2) as zsum_sb_pool,
            tc.tile_pool(name="hT", bufs=2) as hT_pool,
            tc.tile_pool(name="small", bufs=4) as spool,
            tc.tile_pool(name="zsum_ps", bufs=2, space="PSUM") as zsum_ps_pool,
            tc.tile_pool(name="agg_ps", bufs=2, space="PSUM") as agg_ps_pool,
            tc.tile_pool(name="tr_ps", bufs=1, space="PSUM") as tr_ps_pool,
            # agg_ps holds every [128,64]-or-smaller PSUM tile under ONE tag
            # ("a") so the pool stays at 2 banks; tr_ps holds the transpose
            # output (1 bank). Total: 5 + 2 + 1 = 8 banks.
        ):
            nc.gpsimd.load_library(library_config.mlp)

            # ---- load constants ----
            eaT = cpool.tile([4, EPC], F16)
            nc.sync.dma_start(eaT[:], eaT_d[:])
            invd = cpool.tile([128, NTILES], F32)
            nc.sync.dma_start(invd[:], invd_d[:])
            segT = cpool.tile([128, NTILES * WIN], F16)
            nc.sync.dma_start(segT[:], segT_d[:])
            idx = cpool.tile([128, EPC // 16], I16)
            nc.sync.dma_start(idx[:], idx_d[:])
            Tsb = cpool.tile([128, NCH * W], F16)
            nc.sync.dma_start(Tsb[:], Tsb_d[:])
            k1 = cpool.tile([4, W], F16)
            nc.sync.dma_start(k1[:], k1_d[:])
            rootW = cpool.tile([W, W], F16)
            nc.sync.dma_start(rootW[:], root_d[:])
            fc2 = cpool.tile([W, 1], F16)
            nc.sync.dma_start(fc2[:], fc2_d[:])
            fc2b = cpool.tile([WIN, 1], F32)
            nc.sync.dma_start(fc2b[:], fc2b_d[:])
            ident = cpool.tile([64, 64], F16)
            nc.sync.dma_start(ident[:], id_d[:])

            # ---- e2aug: [128, NTILES*66] fp32 ----
            e2aug = cpool.tile([128, NTILES * 66], F32)
            nc.vector.memset(e2aug[:], 0.0)
            for t in range(NTILES):
                e2_ps = agg_ps_pool.tile([128, W], F32, tag="a")
                nc.tensor.matmul(e2_ps[:], eaT[:, t * 128:(t + 1) * 128],
                                 k1[:], start=True, stop=True)
                # relu into e2aug cols [t*66, t*66+64)
                nc.scalar.activation(e2aug[:, t * 66: t * 66 + 64], e2_ps[:], Relu)
                # scale by invdeg (per-partition scalar)
                nc.vector.tensor_scalar_mul(
                    e2aug[:, t * 66: t * 66 + 64],
                    e2aug[:, t * 66: t * 66 + 64],
                    invd[:, t: t + 1])
                # col 64 = invdeg
                nc.vector.tensor_copy(
                    e2aug[:, t * 66 + 64: t * 66 + 65], invd[:, t: t + 1])

            # e2dup[p, t, c, b] = e2aug[p, t*66+c] for b in {0,1}: fp16 with
            # every value duplicated so the z-build APs end in a packed
            # (stride 1, count 2) dim on ALL operands -> DVE 2x mode.
            e2dup = cpool.tile([128, NTILES * 66 * 2], F16)
            nc.vector.tensor_copy(
                e2dup[:].rearrange("p (t b) -> p t b", b=2),
                e2aug[:].unsqueeze(2).broadcast_to((128, NTILES * 66, 2)))

            hT_cur = cpool.tile([W, WPC * WIN], F16)
            nc.sync.dma_start(hT_cur[:], h0T_d[:])

            for it in range(DEP):
                gather_src = h0_d if it == 0 else h_full[it - 1]
                h_src = hsrc_pool.tile([128, NTILES, 2 * W], F16)
                # <=512 idx per call: a single huge gather overflows the
                # SWDGE descriptor ring and faults NRT.
                GCH = 512
                for o in range(0, EPC, GCH):
                    n = min(GCH, EPC - o)
                    nc.gpsimd.dma_gather(
                        h_src[:, o // 128:(o + n) // 128, :], gather_src[:],
                        idx[:, o // 16:(o + n) // 16], n, n, 2 * W)

                hT_next = hT_pool.tile([W, WPC * WIN], F16)
                for w in range(WPC):
                    zs = []
                    for et in range(NT):
                        t = w * NT + et
                        z = zpool.tile([128, 66 * 64], F16)
                        # all-fp16 operands with packed (1,2) last dims hit
                        # the DVE 2x perf mode (stride-0 last dims do not)
                        zv = z[:].rearrange("p (c a b) -> p c a b", c=66, b=2)
                        hs = h_src[:, t, :W].rearrange("p (a b) -> p a b", b=2) \
                            .unsqueeze(1).broadcast_to((128, 66, 32, 2))
                        e2 = e2dup[:, t * 132:(t + 1) * 132] \
                            .rearrange("p (c b) -> p c b", b=2) \
                            .unsqueeze(2).broadcast_to((128, 66, 32, 2))
                        nc.vector.tensor_mul(zv, hs, e2)
                        zs.append(z)
                    # chunk-major: each PSUM accumulation group runs to
                    # completion before the next opens — start=True clears
                    # has_written for the WHOLE bank, so groups sharing a
                    # bank must never interleave. zsumT [128, NCH*WIN] f32
                    # exceeds PSUM, so run the chunks in two passes over the
                    # SBUF-resident z tiles.
                    zsum_sb = zsum_sb_pool.tile([128, NCH * WIN], F16)
                    for p0 in range(0, NCH, KH):
                        p1 = min(p0 + KH, NCH)
                        zsum_ps = zsum_ps_pool.tile([128, KH * WIN], F32)
                        for k in range(p0, p1):
                            for et in range(NT):
                                nc.tensor.matmul(
                                    zsum_ps[:, (k - p0) * WIN:(k - p0 + 1) * WIN],
                                    zs[et][:, k * 128:(k + 1) * 128],
                                    segT[:, (w * NT + et) * WIN:(w * NT + et + 1) * WIN],
                                    start=(et == 0), stop=(et == NT - 1))
                        # keep the DVE free for z-builds (critical engine) —
                        # drain PSUM on ACT
                        nc.scalar.copy(zsum_sb[:, p0 * WIN:p1 * WIN],
                                       zsum_ps[:, :(p1 - p0) * WIN])

                    agg_ps = agg_ps_pool.tile([64, WIN], F32, tag="a")
                    for k in range(NCH):
                        nc.tensor.matmul(agg_ps[:],
                                         Tsb[:, k * W:(k + 1) * W],
                                         zsum_sb[:, k * WIN:(k + 1) * WIN],
                                         start=(k == 0), stop=False)
                    nc.tensor.matmul(agg_ps[:], rootW[:],
                                     hT_cur[:, w * WIN:(w + 1) * WIN],
                                     start=False, stop=True)
                    nc.scalar.activation(hT_next[:, w * WIN:(w + 1) * WIN],
                                         agg_ps[:], Relu)
                    if it < DEP - 1:
                        h_ps = tr_ps_pool.tile([WIN, 64], F16)
                        nc.tensor.transpose(h_ps[:],
                                            hT_next[:, w * WIN:(w + 1) * WIN],
                                            ident[:])
                        h_sb = spool.tile([WIN, 64], F16, tag="hnew")
                        nc.scalar.copy(h_sb[:], h_ps[:])
                        nc.sync.dma_start(h_slice[it][w * WIN:(w + 1) * WIN, :W],
                                          h_sb[:])
                hT_cur = hT_next
                if it < DEP - 1:
                    if single_core:
                        nc.sync.dma_start(h_full[it][: WPC * WIN, :],
                                          h_slice[it][:])
                    else:
                        nc.gpsimd.collective_compute(
                            "AllGather",
                            mybir.AluOpType.bypass,
                            ins=[h_slice[it][:].opt()],
                            outs=[h_full[it][:].opt()],
                            replica_groups=[list(range(NC_))],
                        )

            # ---- epilogue: y = h @ fc2 + b ----
            y_sb = spool.tile([WIN, WPC], F32, tag="y")
            for w in range(WPC):
                y_ps = agg_ps_pool.tile([WIN, 1], F32, tag="a")
                nc.tensor.matmul(y_ps[:], hT_cur[:, w * WIN:(w + 1) * WIN],
                                 fc2[:], start=True, stop=True)
                nc.vector.tensor_add(y_sb[:, w: w + 1], y_ps[:], fc2b[:])
            y_view = y_d[:].rearrange("(w v) o -> v (w o)", w=WPC)
            nc.sync.dma_start(y_view, y_sb[:])

    nc.compile()
    return nc


def bench(inputs, iters=20):
    """Jit the SPMD program once, then time repeated executions with
    device-resident inputs. Returns (output, per-exec seconds list)."""
    import time

    import jax
    from jax.sharding import Mesh, PartitionSpec
    from jax.experimental.shard_map import shard_map
    from concourse import bass2jax
    from concourse.bass2jax import _bass_exec_p, partition_id_tensor

    bass2jax.install_neuronx_cc_hook()

    plan = make_plan(**{k: np.asarray(v) for k, v in inputs.items()})
    nc = build_program(plan)
    n_cores = plan.n_cores
    in_maps = plan.in_maps

    partition_name = nc.partition_id_tensor.name if nc.partition_id_tensor else None
    in_names, out_names, out_avals, zero_outs = [], [], [], []
    for alloc in nc.m.functions[0].allocations:
        if not isinstance(alloc, mybir.MemoryLocationSet):
            continue
        name = alloc.memorylocations[0].name
        if alloc.kind == "ExternalInput":
            if name != partition_name:
                in_names.append(name)
        elif alloc.kind == "ExternalOutput":
            shape = tuple(alloc.tensor_shape)
            dtype = mybir.dt.np(alloc.dtype)
            out_names.append(name)
            out_avals.append(jax.core.ShapedArray(shape, dtype))
            zero_outs.append(np.zeros(shape, dtype))
    n_params = len(in_names)
    all_in_names = list(in_names) + list(out_names)
    if partition_name is not None:
        all_in_names.append(partition_name)

    def _body(*args):
        operands = list(args)
        if partition_name is not None:
            operands.append(partition_id_tensor())
        return tuple(_bass_exec_p.bind(
            *operands,
            out_avals=tuple(out_avals),
            in_names=tuple(all_in_names),
            out_names=tuple(out_names),
            lowering_input_output_aliases=(),
            sim_require_finite=True,
            sim_require_nnan=True,
            nc=nc,
        ))

    devices = jax.devices()[:n_cores]
    mesh = Mesh(np.asarray(devices), ("core",))
    in_specs = (PartitionSpec("core"),) * (n_params + len(out_names))
    out_specs = (PartitionSpec("core"),) * len(out_names)
    sharded = jax.jit(shard_map(_body, mesh=mesh, in_specs=in_specs,
                                out_specs=out_specs, check_rep=False),
                      keep_unused=True)

    concat_in = [np.concatenate([np.asarray(in_maps[c][n]) for c in range(n_cores)],
                                axis=0) for n in in_names]
    concat_zeros = [np.zeros((n_cores * z.shape[0], *z.shape[1:]), z.dtype)
                    for z in zero_outs]
    dev_in = [jax.device_put(a) for a in concat_in]
    dev_zero = [jax.device_put(a) for a in concat_zeros]

    out = sharded(*dev_in, *dev_zero)  # compile + first exec
    jax.block_until_ready(out)

    times = []
    for _ in range(iters):
        t0 = time.perf_counter()
        out = sharded(*dev_in, *dev_zero)
        jax.block_until_ready(out)
        times.append(time.perf_counter() - t0)

    y_all = np.asarray(out[out_names.index("y")]).reshape(n_cores, -1, 1)
    y = np.concatenate([y_all[c] for c in range(n_cores)], axis=0)
    return y[plan.devnode], times


def kernel(**inputs) -> np.ndarray:
    from concourse.bass_utils import run_bass_kernel_spmd

    plan = make_plan(**{k: np.asarray(v) for k, v in inputs.items()})
    nc = build_program(plan)
    core_ids = list(range(plan.n_cores))
    res = run_bass_kernel_spmd(nc, plan.in_maps, core_ids,
                               trace=bool(int(os.environ.get("KERNEL_TRACE", "0"))))
    y = np.concatenate([res.results[r]["y"] for r in range(plan.n_cores)], axis=0)
    out = y[plan.devnode]
    kernel.last_results = res
    kernel.last_plan = plan
    return out



# revision 13
# speedup vs baseline: 1.9475x; 1.0893x over previous
"""Trainium2 Bass kernel for nn_Net_MP_68805376082308 (NNConv-style GNN).

Reference computation (see problem statement):
    h = x@fc1 + b
    e2 = relu(edge_attr@k1 + b1)                     # [E, 64]
    ew = (e2 @ k2 + b2).reshape(E, 64, 64)           # never materialized here!
    for 4 iters:
        msg  = einsum('ei,eio->eo', h[src], ew)
        agg  = segment_sum(msg, dst) / max(deg,1)
        h    = relu(agg + h@root)
    out = h @ fc2 + b

Device algorithm (per core, node-sharded, dst-grouped edge slots):
    e2s[e, c]    = relu((edge_attr@k1+b1)[e,c]) * invdeg[dst[e]]  (c in 0..63)
    z[e, c*64+i] = e2s[e,c] * h[src[e], i]        # DVE, fp16 pair-trick APs
    zsumT[ci, v] = sum_e z[e,ci] * SegMat[e,v]    # PE, z stationary (scatter
                                                  #  commutes with k2 contract)
    hb[i, v]     = sum_e h[e,i] * segB[e,v]       # segB = invdeg-weighted seg
    aggT[o, v]   = T_cm.T@zsumT + B.T@hb + root.T@hT   # B = k2_b.reshape(64,64)
    hT           = relu(aggT)                     # ACT
    h[src] gather via SWDGE dma_gather; h exchanged across 8 cores with an
    AllGather after each iteration.

Windows hold 128 dst-node slots each; per core the window edge-tile counts are
asymmetric ([6,5,6,5,6,5,6,5,5,1]) so the LAST window's zsum tail (which sits
on the critical path into the next iteration's gather) is tiny.

kernel(**inputs) takes the FULL unsharded inputs and returns [10000, 1] fp32.
"""

import math
import os
import sys
from dataclasses import dataclass, field

import numpy as np

sys.path.insert(0, "/opt/trn_rl_repo")

import concourse.bacc as bacc
import concourse.bass as bass
import concourse.mybir as mybir
import concourse.tile as tile
from concourse import library_config

F32 = mybir.dt.float32
F16 = mybir.dt.float16
I16 = mybir.dt.int16

WIDTH = 64
DEPTH = 4
NCH = 33                # ci chunks of 128 (66*64/128)
KH = 8                  # chunks per PSUM pass


@dataclass
class Plan:
    """Host-side preprocessing result: all per-core device input arrays plus
    the compile-time structure constants."""

    n_cores: int
    wpc: int                 # windows per core
    nt_w: list = None        # tiles per window (same layout for every core)
    nodes_pad: int = 0
    depth: int = DEPTH
    win: int = 128           # nodes per scatter window
    devnode: np.ndarray = None     # [N] original node -> device row
    in_maps: list = field(default_factory=list)
    fc2_b: float = 0.0

    @property
    def ntiles(self):        # edge tiles per core
        return sum(self.nt_w)

    @property
    def epc(self):           # edge slots per core
        return self.ntiles * 128

    @property
    def tile_off(self):      # first tile index of each window
        off, out = 0, []
        for n in self.nt_w:
            out.append(off)
            off += n
        return out


def make_plan(x, edge_index, edge_attr, fc1_W, fc1_b, k1_W, k1_b, k2_W, k2_b,
              root, conv_b, fc2_W, fc2_b, n_cores=8, depth=DEPTH):
    W = WIDTH
    N = x.shape[0]
    E = edge_index.shape[1]
    src = np.asarray(edge_index[0], dtype=np.int64)
    dst = np.asarray(edge_index[1], dtype=np.int64)
    assert np.all(np.asarray(conv_b) == 0.0), "kernel assumes conv_b == 0"

    WIN = 128
    wpc = max(1, int(math.ceil(N / WIN / n_cores)))
    n_windows = n_cores * wpc
    nodes_pad = n_windows * WIN

    counts = np.bincount(dst, minlength=N).astype(np.float64)
    denom = np.where(counts > 0, counts, 1.0)
    invdeg_node = (1.0 / denom).astype(np.float32)

    # Per-window edge-tile capacities (uniform; asymmetric layouts lose:
    # low-degree tail nodes still bring >1 tile of edges and the greedy
    # overflow then costs more steady-state tiles than the tail saves).
    base_nt = [5] * 10
    if wpc != 10:  # generic fallback: balanced with one tiny last window
        per = int(math.ceil(E / n_cores / max(1, wpc - 1) / 128)) + 1
        base_nt = [per] * (wpc - 1) + [1]
    cap = np.array([nt * 128 for _ in range(n_cores) for nt in base_nt],
                   dtype=np.int64)

    order = np.argsort(-counts, kind="stable")
    win_edges = np.zeros(n_windows, dtype=np.int64)
    win_fill = np.zeros(n_windows, dtype=np.int64)
    node_window = np.zeros(N, dtype=np.int64)
    node_slot = np.zeros(N, dtype=np.int64)
    NEG = -(1 << 60)
    # greedy: place desc-degree nodes into the window with the most remaining
    # edge capacity that still has node slots; grow a window's capacity by a
    # tile if nothing fits.
    rem = cap.copy()
    for n in order:
        d = int(counts[n])
        w = int(np.argmax(rem))
        if rem[w] < d:
            cap[w] += 128 * int(math.ceil((d - rem[w]) / 128))
            rem[w] = cap[w] - win_edges[w]
        node_window[n] = w
        node_slot[n] = win_fill[w]
        win_fill[w] += 1
        win_edges[w] += d
        rem[w] = cap[w] - win_edges[w] if win_fill[w] < WIN else NEG
    nt_all = (cap // 128).reshape(n_cores, wpc)
    # every core runs one compiled program -> shared nt layout: per-position max
    nt_w = [int(nt_all[:, i].max()) for i in range(wpc)]

    plan = Plan(n_cores=n_cores, wpc=wpc, nt_w=nt_w, nodes_pad=nodes_pad,
                depth=depth, win=WIN,
                fc2_b=float(np.asarray(fc2_b).reshape(())))
    ntiles = plan.ntiles
    epc = plan.epc
    woff = [128 * t for t in plan.tile_off]   # slot offset of window in core

    plan.devnode = node_window * WIN + node_slot

    # edge -> slot within its dst window. Edges whose SOURCE lies in any
    # core's last window ("fix" edges) go to the tail of the window's slot
    # range (the last tile): the other tiles' gather then depends only on
    # h windows 0..wpc-2 and overlaps the last window's compute.
    devnode = node_window * WIN + node_slot
    edge_win = node_window[dst]
    is_fix = (devnode[src] % (wpc * WIN)) >= (wpc - 1) * WIN
    ord_e = np.argsort(edge_win, kind="stable")
    fill = np.zeros(n_windows, dtype=np.int64)
    fillb = np.zeros(n_windows, dtype=np.int64)
    eslot = np.zeros(E, dtype=np.int64)
    for e in ord_e:
        w = edge_win[e]
        core, wl = divmod(w, wpc)
        capw = nt_w[wl] * 128
        if is_fix[e]:
            fillb[w] += 1
            eslot[e] = core * epc + woff[wl] + capw - fillb[w]
        else:
            eslot[e] = core * epc + woff[wl] + fill[w]
            fill[w] += 1
    assert all(fill[w] + fillb[w] <= nt_w[w % wpc] * 128
               for w in range(n_windows))
    assert fillb.max() <= 128, "fix edges must fit the last tile"

    tot_slots = n_cores * epc
    slot_src = np.zeros(tot_slots, dtype=np.int64)
    slot_used = np.zeros(tot_slots, dtype=bool)
    slot_vloc = np.zeros(tot_slots, dtype=np.int64)
    slot_invdeg = np.zeros(tot_slots, dtype=np.float32)
    slot_ea = np.zeros((tot_slots, 3), dtype=np.float32)
    slot_src[eslot] = devnode[src]
    del devnode
    slot_used[eslot] = True
    slot_vloc[eslot] = node_slot[dst]
    slot_invdeg[eslot] = invdeg_node[dst]
    slot_ea[eslot] = np.asarray(edge_attr, dtype=np.float32)

    # weight repacks: T_cm [66*64, 64]: rows 0..4095 = k2_W, rows
    # 4096..4159 = k2_b (paired with z's invdeg column), rest zero.
    # chunk layout: T_sb[p, k*64+o] = T_cm[k*128+p, o]
    T_cm = np.zeros((66 * 64, W), dtype=np.float32)
    T_cm[: 64 * 64] = np.ascontiguousarray(
        np.asarray(k2_W, dtype=np.float32).reshape(64, 64, 64)
    ).reshape(64 * 64, W)
    T_cm[64 * 64 : 65 * 64] = np.asarray(k2_b, dtype=np.float32).reshape(64, 64)
    T_sb = np.ascontiguousarray(
        T_cm.reshape(NCH, 128, W).transpose(1, 0, 2)
    ).reshape(128, NCH * W).astype(np.float16)

    # k1 extended to 66 cols: 0-63 = [k1_W; k1_b], 64 = bias-row one (the
    # invdeg ACT-scale turns it into the invdeg column), 65 = zero pad.
    k1_Wb = np.zeros((4, 66), dtype=np.float16)
    k1_Wb[:3, :64] = np.asarray(k1_W, dtype=np.float16)
    k1_Wb[3, :64] = np.asarray(k1_b, dtype=np.float16)
    k1_Wb[3, 64] = 1.0

    # h rows padded to 128 f16 (=256B) so SWDGE dma_gather's 256B-multiple
    # row-stride restriction is met; cols 64.. are never read by compute.
    h0 = np.zeros((nodes_pad, 2 * W), dtype=np.float16)
    h0[plan.devnode, :W] = (np.asarray(x, np.float32) @ np.asarray(fc1_W, np.float32)
                       + np.asarray(fc1_b, np.float32)).astype(np.float16)

    ident = np.eye(64, dtype=np.float16)
    root_np = np.asarray(root, dtype=np.float16)
    fc2_np = np.asarray(fc2_W, dtype=np.float16).reshape(W, 1)

    for r in range(n_cores):
        sl = slice(r * epc, (r + 1) * epc)
        c_ea = slot_ea[sl]
        c_used = slot_used[sl]
        c_invd = slot_invdeg[sl]
        c_vloc = slot_vloc[sl]
        c_src = slot_src[sl]

        eaT = np.zeros((4, epc), dtype=np.float16)
        eaT[:3] = c_ea.T.astype(np.float16)
        eaT[3] = 1.0
        # invdeg in [partition, tile] layout (slot s -> (s//128, s%128))
        invd = np.ascontiguousarray(
            c_invd.reshape(ntiles, 128).T)                       # [128, ntiles]
        tt = np.arange(epc) // 128
        pp = np.arange(epc) % 128
        segT = np.zeros((ntiles, 128, WIN), dtype=np.float16)
        segT[tt[c_used], pp[c_used], c_vloc[c_used]] = 1.0
        segT = np.ascontiguousarray(segT.transpose(1, 0, 2)).reshape(128, ntiles * WIN)

        idx = np.zeros((128, epc // 16), dtype=np.int16)
        base = c_src.astype(np.int16).reshape(epc // 16, 16).T   # [16, epc/16]
        for g in range(8):
            idx[16 * g : 16 * (g + 1)] = base

        h0T = np.ascontiguousarray(
            h0[r * wpc * WIN : (r + 1) * wpc * WIN, :W].T)       # [64, wpc*WIN]


        plan.in_maps.append({
            "eaT": eaT,
            "invdeg": invd,
            "segmatT": segT,
            "idx": idx,
            "h0": h0,
            "h0T": h0T,
            "T_sb": T_sb,
            "k1_Wb": k1_Wb,
            "root": root_np,
            "fc2_W": fc2_np,
            "fc2_b": np.full((WIN, 1), plan.fc2_b, dtype=np.float32),
            "ident": ident,
        })
    return plan


def build_program(plan: Plan, debug=False, single_core=False):
    """Build the SPMD Bass program (one program, run on all cores).

    single_core=True replaces the AllGather with direct local h_full writes
    (and drops addr_space="Shared") so the program can run under TimelineSim
    for cost modeling."""
    W = WIDTH
    WPC = plan.wpc
    WIN = plan.win
    NT_W = plan.nt_w
    TOFF = plan.tile_off
    NTILES = plan.ntiles
    EPC = plan.epc
    NPAD = plan.nodes_pad
    DEP = plan.depth
    NC_ = plan.n_cores
    Relu = mybir.ActivationFunctionType.Relu

    nc = bacc.Bacc("TRN2", target_bir_lowering=False, debug=debug,
                   num_devices=NC_)

    # ---- I/O ----
    eaT_d = nc.dram_tensor("eaT", [4, EPC], F16, kind="ExternalInput")
    invd_d = nc.dram_tensor("invdeg", [128, NTILES], F32, kind="ExternalInput")
    segT_d = nc.dram_tensor("segmatT", [128, NTILES * WIN], F16, kind="ExternalInput")
    idx_d = nc.dram_tensor("idx", [128, EPC // 16], I16, kind="ExternalInput")
    h0_d = nc.dram_tensor("h0", [NPAD, 2 * W], F16, kind="ExternalInput")
    h0T_d = nc.dram_tensor("h0T", [W, WPC * WIN], F16, kind="ExternalInput")
    Tsb_d = nc.dram_tensor("T_sb", [128, NCH * W], F16, kind="ExternalInput")
    k1_d = nc.dram_tensor("k1_Wb", [4, 66], F16, kind="ExternalInput")
    root_d = nc.dram_tensor("root", [W, W], F16, kind="ExternalInput")
    fc2_d = nc.dram_tensor("fc2_W", [W, 1], F16, kind="ExternalInput")
    fc2b_d = nc.dram_tensor("fc2_b", [WIN, 1], F32, kind="ExternalInput")
    id_d = nc.dram_tensor("ident", [64, 64], F16, kind="ExternalInput")
    y_d = nc.dram_tensor("y", [WPC * WIN, 1], F32, kind="ExternalOutput")

    # internal DRAM for the h exchange
    h_slice = [nc.dram_tensor(f"h_slice{i}", [WPC * WIN, 2 * W], F16)
               for i in range(DEP - 1)]
    if single_core:
        h_fullA = [nc.dram_tensor(f"h_fullA{i}", [NPAD, 2 * W], F16)
                   for i in range(DEP - 1)]
        h_full = [nc.dram_tensor(f"h_fullB{i}", [NPAD, 2 * W], F16)
                  for i in range(DEP - 1)]
    else:
        h_full = [nc.dram_tensor(f"h_full{i}", [NPAD, 2 * W], F16,
                                 addr_space="Shared")
                  for i in range(DEP - 1)]
        h_fullA = h_full

    MAXNT = max(NT_W)
    with tile.TileContext(nc) as tc:
        with (
            tc.tile_pool(name="const", bufs=1) as cpool,
            tc.tile_pool(name="hsrc", bufs=2) as hsrc_pool,
            tc.tile_pool(name="z", bufs=2 * MAXNT + 1) as zpool,
            tc.tile_pool(name="zsum_sb", bufs=2) as zsum_sb_pool,
            tc.tile_pool(name="hT", bufs=2) as hT_pool,
            tc.tile_pool(name="small", bufs=4) as spool,
            tc.tile_pool(name="zsum_ps", bufs=2, space="PSUM") as zsum_ps_pool,
            tc.tile_pool(name="agg_ps", bufs=2, space="PSUM") as agg_ps_pool,
            tc.tile_pool(name="tr_ps", bufs=1, space="PSUM") as tr_ps_pool,
        ):
            nc.gpsimd.load_library(library_config.mlp)

            # preload the ACT function table (1.3us) under the const DMAs
            warm = cpool.tile([1, 1], F32)
            nc.vector.memset(warm[:], 0.0)
            nc.scalar.activation(warm[:], warm[:],
                                 mybir.ActivationFunctionType.Relu)

            # ---- constants; gather-critical tensors first so window 0's
            # gather + e2 chain + first zsum start ASAP ----
            idx = cpool.tile([128, EPC // 16], I16)
            nc.sync.dma_start(idx[:], idx_d[:])
            eaT = cpool.tile([4, EPC], F16)
            nc.sync.dma_start(eaT[:], eaT_d[:])
            invd = cpool.tile([128, NTILES], F32)
            nc.sync.dma_start(invd[:], invd_d[:])
            k1 = cpool.tile([4, 66], F16)
            nc.sync.dma_start(k1[:], k1_d[:])
            segT = cpool.tile([128, NTILES * WIN], F16)
            nc.sync.dma_start(segT[:], segT_d[:])
            Tsb = cpool.tile([128, NCH * W], F16)
            nc.sync.dma_start(Tsb[:], Tsb_d[:])
            h0T = cpool.tile([W, WPC * WIN], F16)
            nc.sync.dma_start(h0T[:], h0T_d[:])
            rootW = cpool.tile([W, W], F16)
            nc.sync.dma_start(rootW[:], root_d[:])
            fc2 = cpool.tile([W, 1], F16)
            nc.sync.dma_start(fc2[:], fc2_d[:])
            fc2b = cpool.tile([WIN, 1], F32)
            nc.sync.dma_start(fc2b[:], fc2b_d[:])
            ident = cpool.tile([64, 64], F16)
            nc.sync.dma_start(ident[:], id_d[:])

            # ---- e2dup: [128, NTILES*64*2] fp16, every value twice so the
            # z-build APs end in a packed (stride 1, count 2) dim on ALL
            # operands -> DVE 2x mode. relu(x*invdeg) = invdeg*relu(x) folds
            # the scatter-mean denominator into the ACT scale. ----
            e2dup = cpool.tile([128, NTILES * 66 * 2], F16)

            def build_e2dup(t):
                e2_ps = agg_ps_pool.tile([128, 66], F32, tag="a")
                nc.tensor.matmul(e2_ps[:], eaT[:, t * 128:(t + 1) * 128],
                                 k1[:], start=True, stop=True)
                dup = e2dup[:, t * 132:(t + 1) * 132] \
                    .rearrange("p (c b) -> p c b", b=2)
                for b in range(2):
                    nc.scalar.activation(dup[:, :, b], e2_ps[:], Relu,
                                         scale=invd[:, t: t + 1])

            def build_e2dup_win(w):
                for et in range(NT_W[w]):
                    build_e2dup(TOFF[w] + et)

            # only the first two windows' e2dup up front: emitting all of it
            # here would queue 24us of ACT work ahead of iteration 0's PSUM
            # drains (ACT executes in order) and stall the whole pipeline.
            build_e2dup_win(0)
            build_e2dup_win(1)

            hT_cur = h0T
            y_sb = spool.tile([WIN, WPC], F32, tag="y")

            for it in range(DEP):
                gsrcA = h0_d if it == 0 else h_fullA[it - 1]
                gsrcB = h0_d if it == 0 else h_full[it - 1]
                # Two gathers per window into its own tiles: the MAIN gather
                # (tiles 0..nt-2, whose edges by construction have sources in
                # windows 0..wpc-2) runs as soon as those h windows land and
                # overlaps the last window's compute; only the small FIX
                # gather (last tile) waits for the final h window.
                h_srcs = [None] * WPC
                h_fix = [None] * WPC

                def issue_gather(w):
                    nt = NT_W[w]
                    o = TOFF[w] * 128
                    nm = (nt - 1) * 128
                    hs_w = hsrc_pool.tile([128, nt - 1, 2 * W], F16,
                                          tag=f"h{w}")
                    nc.gpsimd.dma_gather(
                        hs_w[:], gsrcA[:],
                        idx[:, o // 16:(o + nm) // 16], nm, nm, 2 * W)
                    h_srcs[w] = hs_w
                    hf_w = hsrc_pool.tile([128, 1, 2 * W], F16, tag=f"hf{w}")
                    nc.gpsimd.dma_gather(
                        hf_w[:], gsrcB[:],
                        idx[:, (o + nm) // 16:(o + nm + 128) // 16], 128, 128,
                        2 * W)
                    h_fix[w] = hf_w

                issue_gather(0)
                issue_gather(1)
                hT_next = hT_pool.tile([W, WPC * WIN], F16)
                for w in range(WPC):
                    nt = NT_W[w]
                    t0 = TOFF[w]
                    zs = []
                    for et in range(nt):
                        t = t0 + et
                        z = zpool.tile([128, NCH * 128], F16)
                        # all-fp16 operands with packed (1,2) last dims hit
                        # the DVE 2x perf mode (stride-0 last dims do not)
                        zv = z[:].rearrange("p (c a b) -> p c a b", c=66, b=2)
                        h_t = h_srcs[w] if et < nt - 1 else h_fix[w]
                        e_t = et if et < nt - 1 else 0
                        hs = h_t[:, e_t, :W] \
                            .rearrange("p (a b) -> p a b", b=2) \
                            .unsqueeze(1).broadcast_to((128, 66, 32, 2))
                        e2 = e2dup[:, t * 132:(t + 1) * 132] \
                            .rearrange("p (c b) -> p c b", b=2) \
                            .unsqueeze(2).broadcast_to((128, 66, 32, 2))
                        # offload part of the first tile of each window to
                        # the (idle) GPSIMD engine; DVE builds the rest.
                        # Not in iteration 0's first windows: Pool is still
                        # busy with the initial gather burst there.
                        if et == 0 and not (it == 0 and w < 3):
                            nc.gpsimd.tensor_mul(
                                zv[:, :30, :, :], hs[:, :30, :, :],
                                e2[:, :30, :, :])
                            nc.vector.tensor_mul(
                                zv[:, 30:, :, :], hs[:, 30:, :, :],
                                e2[:, 30:, :, :])
                        else:
                            nc.vector.tensor_mul(zv, hs, e2)
                        zs.append(z)
                    if w + 2 < WPC:
                        issue_gather(w + 2)
                    if it == 0 and w + 2 < WPC:
                        build_e2dup_win(w + 2)
                    # zsum in KH-chunk PSUM passes (back-to-back on PE; the
                    # drains pipeline on ACT), then the T-contract block.
                    zsum_sb = zsum_sb_pool.tile([128, NCH * WIN], F16)
                    for p0 in range(0, NCH, KH):
                        p1 = min(p0 + KH, NCH)
                        zsum_ps = zsum_ps_pool.tile([128, KH * WIN], F32)
                        for k in range(p0, p1):
                            for et in range(nt):
                                nc.tensor.matmul(
                                    zsum_ps[:, (k - p0) * WIN:(k - p0 + 1) * WIN],
                                    zs[et][:, k * 128:(k + 1) * 128],
                                    segT[:, (t0 + et) * WIN:(t0 + et + 1) * WIN],
                                    start=(et == 0), stop=(et == nt - 1))
                        # keep the DVE free for z-builds (critical engine) —
                        # drain PSUM on ACT
                        nc.scalar.copy(zsum_sb[:, p0 * WIN:p1 * WIN],
                                       zsum_ps[:, :(p1 - p0) * WIN])
                    agg_ps = agg_ps_pool.tile([64, WIN], F32, tag="a")
                    for k in range(NCH):
                        nc.tensor.matmul(agg_ps[:],
                                         Tsb[:, k * W:(k + 1) * W],
                                         zsum_sb[:, k * WIN:(k + 1) * WIN],
                                         start=(k == 0), stop=False)
                    nc.tensor.matmul(agg_ps[:], rootW[:],
                                     hT_cur[:, w * WIN:(w + 1) * WIN],
                                     start=False, stop=True)
                    nc.scalar.activation(hT_next[:, w * WIN:(w + 1) * WIN],
                                         agg_ps[:], Relu)
                    if it == DEP - 1:
                        y_ps = agg_ps_pool.tile([WIN, 1], F32, tag="a")
                        nc.tensor.matmul(y_ps[:],
                                         hT_next[:, w * WIN:(w + 1) * WIN],
                                         fc2[:], start=True, stop=True)
                        nc.vector.tensor_add(y_sb[:, w: w + 1], y_ps[:], fc2b[:])
                    else:
                        h_ps = tr_ps_pool.tile([WIN, 64], F16, tag="tr")
                        nc.tensor.transpose(h_ps[:],
                                            hT_next[:, w * WIN:(w + 1) * WIN],
                                            ident[:])
                        h_sb = spool.tile([WIN, 64], F16, tag="hnew")
                        nc.scalar.copy(h_sb[:], h_ps[:])
                        if single_core:
                            if w < WPC - 1:
                                nc.sync.dma_start(
                                    h_fullA[it][w * WIN:(w + 1) * WIN, :W],
                                    h_sb[:])
                                if w == WPC - 2:
                                    # B gets windows 0..wpc-2 via one bulk
                                    # copy (fix gathers wait for the last
                                    # window anyway, so this is off the
                                    # critical path)
                                    nc.sync.dma_start(
                                        h_full[it][: (WPC - 1) * WIN, :],
                                        h_fullA[it][: (WPC - 1) * WIN, :])
                            else:
                                nc.sync.dma_start(
                                    h_full[it][w * WIN:(w + 1) * WIN, :W],
                                    h_sb[:])
                        else:
                            nc.sync.dma_start(
                                h_slice[it][w * WIN:(w + 1) * WIN, :W], h_sb[:])
                hT_cur = hT_next
                if it < DEP - 1 and not single_core:
                    nc.gpsimd.collective_compute(
                        "AllGather",
                        mybir.AluOpType.bypass,
                        ins=[h_slice[it][:].opt()],
                        outs=[h_full[it][:].opt()],
                        replica_groups=[list(range(NC_))],
                    )

            # ---- output ----
            y_view = y_d[:].rearrange("(w v) o -> v (w o)", w=WPC)
            nc.sync.dma_start(y_view, y_sb[:])

    nc.compile()
    return nc


def kernel(**inputs) -> np.ndarray:
    from concourse.bass_utils import run_bass_kernel_spmd

    plan = make_plan(**{k: np.asarray(v) for k, v in inputs.items()})
    nc = build_program(plan)
    core_ids = list(range(plan.n_cores))
    res = run_bass_kernel_spmd(nc, plan.in_maps, core_ids,
                               trace=bool(int(os.environ.get("KERNEL_TRACE", "0"))))
    y = np.concatenate([res.results[r]["y"] for r in range(plan.n_cores)], axis=0)
    out = y[plan.devnode]
    kernel.last_results = res
    kernel.last_plan = plan
    return out


# revision 15
# speedup vs baseline: 1.9562x; 1.0045x over previous
"""Trainium2 Bass kernel for nn_Net_MP_68805376082308 (NNConv-style GNN).

Reference computation (see problem statement):
    h = x@fc1 + b
    e2 = relu(edge_attr@k1 + b1)                     # [E, 64]
    ew = (e2 @ k2 + b2).reshape(E, 64, 64)           # never materialized here!
    for 4 iters:
        msg  = einsum('ei,eio->eo', h[src], ew)
        agg  = segment_sum(msg, dst) / max(deg,1)
        h    = relu(agg + h@root)
    out = h @ fc2 + b

Device algorithm (per core, node-sharded, dst-grouped edge slots):
    e2s[e, c]    = relu((edge_attr@k1+b1)[e,c]) * invdeg[dst[e]]  (c in 0..63)
    z[e, c*64+i] = e2s[e,c] * h[src[e], i]        # DVE, fp16 pair-trick APs
    zsumT[ci, v] = sum_e z[e,ci] * SegMat[e,v]    # PE, z stationary (scatter
                                                  #  commutes with k2 contract)
    hb[i, v]     = sum_e h[e,i] * segB[e,v]       # segB = invdeg-weighted seg
    aggT[o, v]   = T_cm.T@zsumT + B.T@hb + root.T@hT   # B = k2_b.reshape(64,64)
    hT           = relu(aggT)                     # ACT
    h[src] gather via SWDGE dma_gather; h exchanged across 8 cores with an
    AllGather after each iteration.

Windows hold 128 dst-node slots each; per core the window edge-tile counts are
asymmetric ([6,5,6,5,6,5,6,5,5,1]) so the LAST window's zsum tail (which sits
on the critical path into the next iteration's gather) is tiny.

kernel(**inputs) takes the FULL unsharded inputs and returns [10000, 1] fp32.
"""

import math
import os
import sys
from dataclasses import dataclass, field

import numpy as np

sys.path.insert(0, "/opt/trn_rl_repo")

import concourse.bacc as bacc
import concourse.bass as bass
import concourse.mybir as mybir
import concourse.tile as tile
from concourse import library_config

F32 = mybir.dt.float32
F16 = mybir.dt.float16
I16 = mybir.dt.int16

WIDTH = 64
DEPTH = 4
NCH = 33                # ci chunks of 128 (66*64/128)
KH = 8                  # chunks per PSUM pass


@dataclass
class Plan:
    """Host-side preprocessing result: all per-core device input arrays plus
    the compile-time structure constants."""

    n_cores: int
    wpc: int                 # windows per core
    nt_w: list = None        # tiles per window (same layout for every core)
    nodes_pad: int = 0
    depth: int = DEPTH
    win: int = 128           # nodes per scatter window
    devnode: np.ndarray = None     # [N] original node -> device row
    in_maps: list = field(default_factory=list)
    fc2_b: float = 0.0

    @property
    def ntiles(self):        # edge tiles per core
        return sum(self.nt_w)

    @property
    def epc(self):           # edge slots per core
        return self.ntiles * 128

    @property
    def tile_off(self):      # first tile index of each window
        off, out = 0, []
        for n in self.nt_w:
            out.append(off)
            off += n
        return out


def make_plan(x, edge_index, edge_attr, fc1_W, fc1_b, k1_W, k1_b, k2_W, k2_b,
              root, conv_b, fc2_W, fc2_b, n_cores=8, depth=DEPTH):
    W = WIDTH
    N = x.shape[0]
    E = edge_index.shape[1]
    src = np.asarray(edge_index[0], dtype=np.int64)
    dst = np.asarray(edge_index[1], dtype=np.int64)
    assert np.all(np.asarray(conv_b) == 0.0), "kernel assumes conv_b == 0"

    WIN = 128
    wpc = max(1, int(math.ceil(N / WIN / n_cores)))
    n_windows = n_cores * wpc
    nodes_pad = n_windows * WIN

    counts = np.bincount(dst, minlength=N).astype(np.float64)
    denom = np.where(counts > 0, counts, 1.0)
    invdeg_node = (1.0 / denom).astype(np.float32)

    # Per-window edge-tile capacities (uniform; asymmetric layouts lose:
    # low-degree tail nodes still bring >1 tile of edges and the greedy
    # overflow then costs more steady-state tiles than the tail saves).
    base_nt = [5] * 10
    if wpc != 10:  # generic fallback: balanced with one tiny last window
        per = int(math.ceil(E / n_cores / max(1, wpc - 1) / 128)) + 1
        base_nt = [per] * (wpc - 1) + [1]
    cap = np.array([nt * 128 for _ in range(n_cores) for nt in base_nt],
                   dtype=np.int64)

    order = np.argsort(-counts, kind="stable")
    win_edges = np.zeros(n_windows, dtype=np.int64)
    win_fill = np.zeros(n_windows, dtype=np.int64)
    node_window = np.zeros(N, dtype=np.int64)
    node_slot = np.zeros(N, dtype=np.int64)
    NEG = -(1 << 60)
    # greedy: place desc-degree nodes into the window with the most remaining
    # edge capacity that still has node slots; grow a window's capacity by a
    # tile if nothing fits.
    rem = cap.copy()
    for n in order:
        d = int(counts[n])
        w = int(np.argmax(rem))
        if rem[w] < d:
            cap[w] += 128 * int(math.ceil((d - rem[w]) / 128))
            rem[w] = cap[w] - win_edges[w]
        node_window[n] = w
        node_slot[n] = win_fill[w]
        win_fill[w] += 1
        win_edges[w] += d
        rem[w] = cap[w] - win_edges[w] if win_fill[w] < WIN else NEG
    nt_all = (cap // 128).reshape(n_cores, wpc)
    # every core runs one compiled program -> shared nt layout: per-position max
    nt_w = [int(nt_all[:, i].max()) for i in range(wpc)]

    plan = Plan(n_cores=n_cores, wpc=wpc, nt_w=nt_w, nodes_pad=nodes_pad,
                depth=depth, win=WIN,
                fc2_b=float(np.asarray(fc2_b).reshape(())))
    ntiles = plan.ntiles
    epc = plan.epc
    woff = [128 * t for t in plan.tile_off]   # slot offset of window in core

    plan.devnode = node_window * WIN + node_slot

    # edge -> slot within its dst window. Edges whose SOURCE lies in any
    # core's last window ("fix" edges) go to the tail of the window's slot
    # range (the last tile): the other tiles' gather then depends only on
    # h windows 0..wpc-2 and overlaps the last window's compute.
    devnode = node_window * WIN + node_slot
    edge_win = node_window[dst]
    is_fix = (devnode[src] % (wpc * WIN)) >= (wpc - 1) * WIN
    ord_e = np.argsort(edge_win, kind="stable")
    fill = np.zeros(n_windows, dtype=np.int64)
    fillb = np.zeros(n_windows, dtype=np.int64)
    eslot = np.zeros(E, dtype=np.int64)
    for e in ord_e:
        w = edge_win[e]
        core, wl = divmod(w, wpc)
        capw = nt_w[wl] * 128
        if is_fix[e]:
            fillb[w] += 1
            eslot[e] = core * epc + woff[wl] + capw - fillb[w]
        else:
            eslot[e] = core * epc + woff[wl] + fill[w]
            fill[w] += 1
    assert all(fill[w] + fillb[w] <= nt_w[w % wpc] * 128
               for w in range(n_windows))
    assert fillb.max() <= 128, "fix edges must fit the last tile"

    tot_slots = n_cores * epc
    slot_src = np.zeros(tot_slots, dtype=np.int64)
    slot_used = np.zeros(tot_slots, dtype=bool)
    slot_vloc = np.zeros(tot_slots, dtype=np.int64)
    slot_invdeg = np.zeros(tot_slots, dtype=np.float32)
    slot_ea = np.zeros((tot_slots, 3), dtype=np.float32)
    slot_src[eslot] = devnode[src]
    del devnode
    slot_used[eslot] = True
    slot_vloc[eslot] = node_slot[dst]
    slot_invdeg[eslot] = invdeg_node[dst]
    slot_ea[eslot] = np.asarray(edge_attr, dtype=np.float32)

    # weight repacks: T_cm [66*64, 64]: rows 0..4095 = k2_W, rows
    # 4096..4159 = k2_b (paired with z's invdeg column), rest zero.
    # chunk layout: T_sb[p, k*64+o] = T_cm[k*128+p, o]
    T_cm = np.zeros((66 * 64, W), dtype=np.float32)
    T_cm[: 64 * 64] = np.ascontiguousarray(
        np.asarray(k2_W, dtype=np.float32).reshape(64, 64, 64)
    ).reshape(64 * 64, W)
    T_cm[64 * 64 : 65 * 64] = np.asarray(k2_b, dtype=np.float32).reshape(64, 64)
    T_sb = np.ascontiguousarray(
        T_cm.reshape(NCH, 128, W).transpose(1, 0, 2)
    ).reshape(128, NCH * W).astype(np.float16)

    # k1 extended to 66 cols: 0-63 = [k1_W; k1_b], 64 = bias-row one (the
    # invdeg ACT-scale turns it into the invdeg column), 65 = zero pad.
    k1_Wb = np.zeros((4, 66), dtype=np.float16)
    k1_Wb[:3, :64] = np.asarray(k1_W, dtype=np.float16)
    k1_Wb[3, :64] = np.asarray(k1_b, dtype=np.float16)
    k1_Wb[3, 64] = 1.0

    # h rows padded to 128 f16 (=256B) so SWDGE dma_gather's 256B-multiple
    # row-stride restriction is met; cols 64.. are never read by compute.
    h0 = np.zeros((nodes_pad, 2 * W), dtype=np.float16)
    h0[plan.devnode, :W] = (np.asarray(x, np.float32) @ np.asarray(fc1_W, np.float32)
                       + np.asarray(fc1_b, np.float32)).astype(np.float16)

    ident = np.eye(64, dtype=np.float16)
    root_np = np.asarray(root, dtype=np.float16)
    fc2_np = np.asarray(fc2_W, dtype=np.float16).reshape(W, 1)

    for r in range(n_cores):
        sl = slice(r * epc, (r + 1) * epc)
        c_ea = slot_ea[sl]
        c_used = slot_used[sl]
        c_invd = slot_invdeg[sl]
        c_vloc = slot_vloc[sl]
        c_src = slot_src[sl]

        eaT = np.zeros((4, epc), dtype=np.float16)
        eaT[:3] = c_ea.T.astype(np.float16)
        eaT[3] = 1.0
        # invdeg in [partition, tile] layout (slot s -> (s//128, s%128))
        invd = np.ascontiguousarray(
            c_invd.reshape(ntiles, 128).T)                       # [128, ntiles]
        tt = np.arange(epc) // 128
        pp = np.arange(epc) % 128
        segT = np.zeros((ntiles, 128, WIN), dtype=np.float16)
        segT[tt[c_used], pp[c_used], c_vloc[c_used]] = 1.0
        segT = np.ascontiguousarray(segT.transpose(1, 0, 2)).reshape(128, ntiles * WIN)

        idx = np.zeros((128, epc // 16), dtype=np.int16)
        base = c_src.astype(np.int16).reshape(epc // 16, 16).T   # [16, epc/16]
        for g in range(8):
            idx[16 * g : 16 * (g + 1)] = base

        h0T = np.ascontiguousarray(
            h0[r * wpc * WIN : (r + 1) * wpc * WIN, :W].T)       # [64, wpc*WIN]


        plan.in_maps.append({
            "eaT": eaT,
            "invdeg": invd,
            "segmatT": segT,
            "idx": idx,
            "h0": h0,
            "h0T": h0T,
            "T_sb": T_sb,
            "k1_Wb": k1_Wb,
            "root": root_np,
            "fc2_W": fc2_np,
            "fc2_b": np.full((WIN, 1), plan.fc2_b, dtype=np.float32),
            "ident": ident,
        })
    return plan


def build_program(plan: Plan, debug=False, single_core=False):
    """Build the SPMD Bass program (one program, run on all cores).

    single_core=True replaces the AllGather with direct local h_full writes
    (and drops addr_space="Shared") so the program can run under TimelineSim
    for cost modeling."""
    W = WIDTH
    WPC = plan.wpc
    WIN = plan.win
    NT_W = plan.nt_w
    TOFF = plan.tile_off
    NTILES = plan.ntiles
    EPC = plan.epc
    NPAD = plan.nodes_pad
    DEP = plan.depth
    NC_ = plan.n_cores
    Relu = mybir.ActivationFunctionType.Relu

    nc = bacc.Bacc("TRN2", target_bir_lowering=False, debug=debug,
                   num_devices=NC_)

    # ---- I/O ----
    eaT_d = nc.dram_tensor("eaT", [4, EPC], F16, kind="ExternalInput")
    invd_d = nc.dram_tensor("invdeg", [128, NTILES], F32, kind="ExternalInput")
    segT_d = nc.dram_tensor("segmatT", [128, NTILES * WIN], F16, kind="ExternalInput")
    idx_d = nc.dram_tensor("idx", [128, EPC // 16], I16, kind="ExternalInput")
    h0_d = nc.dram_tensor("h0", [NPAD, 2 * W], F16, kind="ExternalInput")
    h0T_d = nc.dram_tensor("h0T", [W, WPC * WIN], F16, kind="ExternalInput")
    Tsb_d = nc.dram_tensor("T_sb", [128, NCH * W], F16, kind="ExternalInput")
    k1_d = nc.dram_tensor("k1_Wb", [4, 66], F16, kind="ExternalInput")
    root_d = nc.dram_tensor("root", [W, W], F16, kind="ExternalInput")
    fc2_d = nc.dram_tensor("fc2_W", [W, 1], F16, kind="ExternalInput")
    fc2b_d = nc.dram_tensor("fc2_b", [WIN, 1], F32, kind="ExternalInput")
    id_d = nc.dram_tensor("ident", [64, 64], F16, kind="ExternalInput")
    y_d = nc.dram_tensor("y", [WPC * WIN, 1], F32, kind="ExternalOutput")

    # internal DRAM for the h exchange
    h_slice = [nc.dram_tensor(f"h_slice{i}", [WPC * WIN, 2 * W], F16)
               for i in range(DEP - 1)]
    if single_core:
        h_fullA = [nc.dram_tensor(f"h_fullA{i}", [NPAD, 2 * W], F16)
                   for i in range(DEP - 1)]
        h_full = [nc.dram_tensor(f"h_fullB{i}", [NPAD, 2 * W], F16)
                  for i in range(DEP - 1)]
    else:
        h_full = [nc.dram_tensor(f"h_full{i}", [NPAD, 2 * W], F16,
                                 addr_space="Shared")
                  for i in range(DEP - 1)]
        h_fullA = h_full

    MAXNT = max(NT_W)
    with tile.TileContext(nc) as tc:
        with (
            tc.tile_pool(name="const", bufs=1) as cpool,
            tc.tile_pool(name="hsrc", bufs=2) as hsrc_pool,
            tc.tile_pool(name="z", bufs=2 * MAXNT + 1) as zpool,
            tc.tile_pool(name="zsum_sb", bufs=2) as zsum_sb_pool,
            tc.tile_pool(name="hT", bufs=2) as hT_pool,
            tc.tile_pool(name="small", bufs=4) as spool,
            tc.tile_pool(name="zsum_ps", bufs=2, space="PSUM") as zsum_ps_pool,
            tc.tile_pool(name="agg_ps", bufs=2, space="PSUM") as agg_ps_pool,
            tc.tile_pool(name="tr_ps", bufs=1, space="PSUM") as tr_ps_pool,
        ):
            nc.gpsimd.load_library(library_config.mlp)

            # preload the ACT function table (1.3us) under the const DMAs;
            # Copy needs no bias const-AP (whose DMA would land late)
            warm = cpool.tile([1, 1], F32)
            nc.vector.memset(warm[:], 0.0)
            nc.scalar.activation(warm[:], warm[:],
                                 mybir.ActivationFunctionType.Copy)

            # ---- constants; gather-critical tensors first so window 0's
            # gather + e2 chain + first zsum start ASAP ----
            n0 = NT_W[0] * 128 // 16
            idx0 = cpool.tile([128, n0], I16)
            nc.sync.dma_start(idx0[:], idx_d[:, :n0])
            idx = cpool.tile([128, EPC // 16], I16)
            nc.sync.dma_start(idx[:], idx_d[:])
            eaT = cpool.tile([4, EPC], F16)
            nc.sync.dma_start(eaT[:], eaT_d[:])
            invd = cpool.tile([128, NTILES], F32)
            nc.sync.dma_start(invd[:], invd_d[:])
            k1 = cpool.tile([4, 66], F16)
            nc.sync.dma_start(k1[:], k1_d[:])
            segT = cpool.tile([128, NTILES * WIN], F16)
            nc.sync.dma_start(segT[:], segT_d[:])
            Tsb = cpool.tile([128, NCH * W], F16)
            nc.sync.dma_start(Tsb[:], Tsb_d[:])
            h0T = cpool.tile([W, WPC * WIN], F16)
            nc.sync.dma_start(h0T[:], h0T_d[:])
            rootW = cpool.tile([W, W], F16)
            nc.sync.dma_start(rootW[:], root_d[:])
            fc2 = cpool.tile([W, 1], F16)
            nc.sync.dma_start(fc2[:], fc2_d[:])
            fc2b = cpool.tile([WIN, 1], F32)
            nc.sync.dma_start(fc2b[:], fc2b_d[:])
            ident = cpool.tile([64, 64], F16)
            nc.sync.dma_start(ident[:], id_d[:])

            # ---- e2dup: [128, NTILES*64*2] fp16, every value twice so the
            # z-build APs end in a packed (stride 1, count 2) dim on ALL
            # operands -> DVE 2x mode. relu(x*invdeg) = invdeg*relu(x) folds
            # the scatter-mean denominator into the ACT scale. ----
            e2dup = cpool.tile([128, NTILES * 66 * 2], F16)

            def build_e2dup(t):
                e2_ps = agg_ps_pool.tile([128, 66], F32, tag="a")
                nc.tensor.matmul(e2_ps[:], eaT[:, t * 128:(t + 1) * 128],
                                 k1[:], start=True, stop=True)
                dup = e2dup[:, t * 132:(t + 1) * 132] \
                    .rearrange("p (c b) -> p c b", b=2)
                for b in range(2):
                    nc.scalar.activation(dup[:, :, b], e2_ps[:], Relu,
                                         scale=invd[:, t: t + 1])

            def build_e2dup_win(w):
                for et in range(NT_W[w]):
                    build_e2dup(TOFF[w] + et)

            # only window 0's e2dup up front: emitting all of it here would
            # queue 24us of ACT work ahead of iteration 0's PSUM drains (ACT
            # executes in order) and stall the whole pipeline; z(w) also
            # waits on every e2dup write emitted before it (tile-granular
            # dependency tracking), so later windows' builds are staggered
            # through iteration 0.
            build_e2dup_win(0)

            hT_cur = h0T
            y_sb = spool.tile([WIN, WPC], F32, tag="y")

            for it in range(DEP):
                gsrcA = h0_d if it == 0 else h_fullA[it - 1]
                gsrcB = h0_d if it == 0 else h_full[it - 1]
                # Two gathers per window into its own tiles: the MAIN gather
                # (tiles 0..nt-2, whose edges by construction have sources in
                # windows 0..wpc-2) runs as soon as those h windows land and
                # overlaps the last window's compute; only the small FIX
                # gather (last tile) waits for the final h window.
                h_srcs = [None] * WPC
                h_fix = [None] * WPC

                def issue_gather(w):
                    nt = NT_W[w]
                    o = TOFF[w] * 128
                    nm = (nt - 1) * 128
                    ix = idx0 if (w == 0 and it == 0) else idx
                    hs_w = hsrc_pool.tile([128, nt - 1, 2 * W], F16,
                                          tag=f"h{w}")
                    nc.gpsimd.dma_gather(
                        hs_w[:], gsrcA[:],
                        ix[:, o // 16:(o + nm) // 16], nm, nm, 2 * W)
                    h_srcs[w] = hs_w
                    hf_w = hsrc_pool.tile([128, 1, 2 * W], F16, tag=f"hf{w}")
                    nc.gpsimd.dma_gather(
                        hf_w[:], gsrcB[:],
                        ix[:, (o + nm) // 16:(o + nm + 128) // 16], 128, 128,
                        2 * W)
                    h_fix[w] = hf_w

                issue_gather(0)
                issue_gather(1)
                hT_next = hT_pool.tile([W, WPC * WIN], F16)
                for w in range(WPC):
                    nt = NT_W[w]
                    t0 = TOFF[w]
                    zs = []
                    for et in range(nt):
                        t = t0 + et
                        z = zpool.tile([128, NCH * 128], F16)
                        # all-fp16 operands with packed (1,2) last dims hit
                        # the DVE 2x perf mode (stride-0 last dims do not)
                        zv = z[:].rearrange("p (c a b) -> p c a b", c=66, b=2)
                        h_t = h_srcs[w] if et < nt - 1 else h_fix[w]
                        e_t = et if et < nt - 1 else 0
                        hs = h_t[:, e_t, :W] \
                            .rearrange("p (a b) -> p a b", b=2) \
                            .unsqueeze(1).broadcast_to((128, 66, 32, 2))
                        e2 = e2dup[:, t * 132:(t + 1) * 132] \
                            .rearrange("p (c b) -> p c b", b=2) \
                            .unsqueeze(2).broadcast_to((128, 66, 32, 2))
                        if et == nt - 1 and w == 0:
                            # window 0's fix tile sits on the iteration
                            # boundary critical path: build it in KH-chunk
                            # pieces so pass 0 can start after the first one
                            for c0 in range(0, 66, 16):
                                c1 = min(c0 + 16, 66)
                                nc.vector.tensor_mul(
                                    zv[:, c0:c1, :, :], hs[:, c0:c1, :, :],
                                    e2[:, c0:c1, :, :])
                            zs.append(z)
                            continue
                        # offload part of the first tile of each window to
                        # the (idle) GPSIMD engine; DVE builds the rest.
                        # Not in iteration 0's first windows: Pool is still
                        # busy with the initial gather burst there.
                        if et == 0 and not (it == 0 and w < 3):
                            nc.gpsimd.tensor_mul(
                                zv[:, :30, :, :], hs[:, :30, :, :],
                                e2[:, :30, :, :])
                            nc.vector.tensor_mul(
                                zv[:, 30:, :, :], hs[:, 30:, :, :],
                                e2[:, 30:, :, :])
                        else:
                            nc.vector.tensor_mul(zv, hs, e2)
                        zs.append(z)
                    if w + 2 < WPC:
                        issue_gather(w + 2)
                    if it == 0 and w + 1 < WPC:
                        build_e2dup_win(w + 1)
                    # zsum in KH-chunk PSUM passes (back-to-back on PE; the
                    # drains pipeline on ACT), then the T-contract block.
                    zsum_sb = zsum_sb_pool.tile([128, NCH * WIN], F16)
                    for p0 in range(0, NCH, KH):
                        p1 = min(p0 + KH, NCH)
                        zsum_ps = zsum_ps_pool.tile([128, KH * WIN], F32)
                        for k in range(p0, p1):
                            for et in range(nt):
                                nc.tensor.matmul(
                                    zsum_ps[:, (k - p0) * WIN:(k - p0 + 1) * WIN],
                                    zs[et][:, k * 128:(k + 1) * 128],
                                    segT[:, (t0 + et) * WIN:(t0 + et + 1) * WIN],
                                    start=(et == 0), stop=(et == nt - 1))
                        # keep the DVE free for z-builds (critical engine) —
                        # drain PSUM on ACT
                        nc.scalar.copy(zsum_sb[:, p0 * WIN:p1 * WIN],
                                       zsum_ps[:, :(p1 - p0) * WIN])
                    agg_ps = agg_ps_pool.tile([64, WIN], F32, tag="a")
                    for k in range(NCH):
                        nc.tensor.matmul(agg_ps[:],
                                         Tsb[:, k * W:(k + 1) * W],
                                         zsum_sb[:, k * WIN:(k + 1) * WIN],
                                         start=(k == 0), stop=False)
                    nc.tensor.matmul(agg_ps[:], rootW[:],
                                     hT_cur[:, w * WIN:(w + 1) * WIN],
                                     start=False, stop=True)
                    nc.scalar.activation(hT_next[:, w * WIN:(w + 1) * WIN],
                                         agg_ps[:], Relu)
                    if it == DEP - 1:
                        y_ps = agg_ps_pool.tile([WIN, 1], F32, tag="a")
                        nc.tensor.matmul(y_ps[:],
                                         hT_next[:, w * WIN:(w + 1) * WIN],
                                         fc2[:], start=True, stop=True)
                        nc.vector.tensor_add(y_sb[:, w: w + 1], y_ps[:], fc2b[:])
                    else:
                        h_ps = tr_ps_pool.tile([WIN, 64], F16, tag="tr")
                        nc.tensor.transpose(h_ps[:],
                                            hT_next[:, w * WIN:(w + 1) * WIN],
                                            ident[:])
                        h_sb = spool.tile([WIN, 64], F16, tag="hnew")
                        nc.scalar.copy(h_sb[:], h_ps[:])
                        if single_core:
                            if w < WPC - 1:
                                nc.sync.dma_start(
                                    h_fullA[it][w * WIN:(w + 1) * WIN, :W],
                                    h_sb[:])
                                if w == WPC - 2:
                                    # B gets windows 0..wpc-2 via one bulk
                                    # copy (fix gathers wait for the last
                                    # window anyway, so this is off the
                                    # critical path)
                                    nc.sync.dma_start(
                                        h_full[it][: (WPC - 1) * WIN, :],
                                        h_fullA[it][: (WPC - 1) * WIN, :])
                            else:
                                nc.sync.dma_start(
                                    h_full[it][w * WIN:(w + 1) * WIN, :W],
                                    h_sb[:])
                        else:
                            nc.sync.dma_start(
                                h_slice[it][w * WIN:(w + 1) * WIN, :W], h_sb[:])
                hT_cur = hT_next
                if it < DEP - 1 and not single_core:
                    nc.gpsimd.collective_compute(
                        "AllGather",
                        mybir.AluOpType.bypass,
                        ins=[h_slice[it][:].opt()],
                        outs=[h_full[it][:].opt()],
                        replica_groups=[list(range(NC_))],
                    )

            # ---- output ----
            y_view = y_d[:].rearrange("(w v) o -> v (w o)", w=WPC)
            nc.sync.dma_start(y_view, y_sb[:])

    nc.compile()
    return nc


def kernel(**inputs) -> np.ndarray:
    from concourse.bass_utils import run_bass_kernel_spmd

    plan = make_plan(**{k: np.asarray(v) for k, v in inputs.items()})
    nc = build_program(plan)
    core_ids = list(range(plan.n_cores))
    res = run_bass_kernel_spmd(nc, plan.in_maps, core_ids,
                               trace=bool(int(os.environ.get("KERNEL_TRACE", "0"))))
    y = np.concatenate([res.results[r]["y"] for r in range(plan.n_cores)], axis=0)
    out = y[plan.devnode]
    kernel.last_results = res
    kernel.last_plan = plan
    return out


# revision 22
# speedup vs baseline: 2.0031x; 1.0240x over previous
"""Trainium2 Bass kernel for nn_Net_MP_68805376082308 (NNConv-style GNN).

Reference computation (see problem statement):
    h = x@fc1 + b
    e2 = relu(edge_attr@k1 + b1)                     # [E, 64]
    ew = (e2 @ k2 + b2).reshape(E, 64, 64)           # never materialized here!
    for 4 iters:
        msg  = einsum('ei,eio->eo', h[src], ew)
        agg  = segment_sum(msg, dst) / max(deg,1)
        h    = relu(agg + h@root)
    out = h @ fc2 + b

Device algorithm (per core, node-sharded, dst-grouped edge slots):
    e2s[e, c]    = relu((edge_attr@k1+b1)[e,c]) * invdeg[dst[e]]  (c in 0..63)
    z[e, c*64+i] = e2s[e,c] * h[src[e], i]        # DVE, fp16 pair-trick APs
    zsumT[ci, v] = sum_e z[e,ci] * SegMat[e,v]    # PE, z stationary (scatter
                                                  #  commutes with k2 contract)
    aggT[o, v]   = T_cm.T @ zsumT + root.T @ hT   # PE (T_cm rows 4096..4159
                                                  #  carry k2_b, paired with
                                                  #  z's invdeg column)
    hT           = relu(aggT)                     # ACT
    h[src] gather via SWDGE dma_gather; h exchanged across 8 cores with an
    AllGather after each iteration.

Windows hold 128 dst-node slots and 5 edge tiles each. Edges whose SOURCE
node lies in any core's last window are segregated into each window's last
("fix") tile: the other tiles' gathers then depend only on h windows 0..8 and
overlap the last window's compute at the iteration boundary (split h_fullA/B
tensors express this to the dependency tracker in the single-core cost model;
the real 8-core path keeps one h_full fed by an AllGather).

kernel(**inputs) takes the FULL unsharded inputs and returns [10000, 1] fp32.
"""

import math
import os
import sys
from dataclasses import dataclass, field

import numpy as np

sys.path.insert(0, "/opt/trn_rl_repo")

import concourse.bacc as bacc
import concourse.bass as bass
import concourse.mybir as mybir
import concourse.tile as tile
from concourse import library_config

F32 = mybir.dt.float32
F16 = mybir.dt.float16
I16 = mybir.dt.int16

WIDTH = 64
DEPTH = 4
NCH = 33                # ci chunks of 128 (66*64/128)
KH = 8                  # chunks per PSUM pass


@dataclass
class Plan:
    """Host-side preprocessing result: all per-core device input arrays plus
    the compile-time structure constants."""

    n_cores: int
    wpc: int                 # windows per core
    nt_w: list = None        # tiles per window (same layout for every core)
    nodes_pad: int = 0
    depth: int = DEPTH
    win: int = 128           # nodes per scatter window
    devnode: np.ndarray = None     # [N] original node -> device row
    in_maps: list = field(default_factory=list)
    fc2_b: float = 0.0

    @property
    def ntiles(self):        # edge tiles per core
        return sum(self.nt_w)

    @property
    def epc(self):           # edge slots per core
        return self.ntiles * 128

    @property
    def tile_off(self):      # first tile index of each window
        off, out = 0, []
        for n in self.nt_w:
            out.append(off)
            off += n
        return out


def make_plan(x, edge_index, edge_attr, fc1_W, fc1_b, k1_W, k1_b, k2_W, k2_b,
              root, conv_b, fc2_W, fc2_b, n_cores=8, depth=DEPTH):
    W = WIDTH
    N = x.shape[0]
    E = edge_index.shape[1]
    src = np.asarray(edge_index[0], dtype=np.int64)
    dst = np.asarray(edge_index[1], dtype=np.int64)
    assert np.all(np.asarray(conv_b) == 0.0), "kernel assumes conv_b == 0"

    WIN = 128
    wpc = max(1, int(math.ceil(N / WIN / n_cores)))
    n_windows = n_cores * wpc
    nodes_pad = n_windows * WIN

    counts = np.bincount(dst, minlength=N).astype(np.float64)
    denom = np.where(counts > 0, counts, 1.0)
    invdeg_node = (1.0 / denom).astype(np.float32)

    # Per-window edge-tile capacities (uniform; asymmetric layouts lose:
    # low-degree tail nodes still bring >1 tile of edges and the greedy
    # overflow then costs more steady-state tiles than the tail saves).
    base_nt = [5] * 10
    if wpc != 10:  # generic fallback: balanced with one tiny last window
        per = int(math.ceil(E / n_cores / max(1, wpc - 1) / 128)) + 1
        base_nt = [per] * (wpc - 1) + [1]
    cap = np.array([nt * 128 for _ in range(n_cores) for nt in base_nt],
                   dtype=np.int64)

    order = np.argsort(-counts, kind="stable")
    win_edges = np.zeros(n_windows, dtype=np.int64)
    win_fill = np.zeros(n_windows, dtype=np.int64)
    node_window = np.zeros(N, dtype=np.int64)
    node_slot = np.zeros(N, dtype=np.int64)
    NEG = -(1 << 60)
    # greedy: place desc-degree nodes into the window with the most remaining
    # edge capacity that still has node slots; grow a window's capacity by a
    # tile if nothing fits.
    rem = cap.copy()
    for n in order:
        d = int(counts[n])
        w = int(np.argmax(rem))
        if rem[w] < d:
            cap[w] += 128 * int(math.ceil((d - rem[w]) / 128))
            rem[w] = cap[w] - win_edges[w]
        node_window[n] = w
        node_slot[n] = win_fill[w]
        win_fill[w] += 1
        win_edges[w] += d
        rem[w] = cap[w] - win_edges[w] if win_fill[w] < WIN else NEG
    nt_all = (cap // 128).reshape(n_cores, wpc)
    # every core runs one compiled program -> shared nt layout: per-position max
    nt_w = [int(nt_all[:, i].max()) for i in range(wpc)]

    plan = Plan(n_cores=n_cores, wpc=wpc, nt_w=nt_w, nodes_pad=nodes_pad,
                depth=depth, win=WIN,
                fc2_b=float(np.asarray(fc2_b).reshape(())))
    ntiles = plan.ntiles
    epc = plan.epc
    woff = [128 * t for t in plan.tile_off]   # slot offset of window in core

    plan.devnode = node_window * WIN + node_slot

    # edge -> slot within its dst window. Edges whose SOURCE lies in any
    # core's last window ("fix" edges) go to the tail of the window's slot
    # range (the last tile): the other tiles' gather then depends only on
    # h windows 0..wpc-2 and overlaps the last window's compute.
    devnode = node_window * WIN + node_slot
    edge_win = node_window[dst]
    is_fix = (devnode[src] % (wpc * WIN)) >= (wpc - 1) * WIN
    ord_e = np.argsort(edge_win, kind="stable")
    fill = np.zeros(n_windows, dtype=np.int64)
    fillb = np.zeros(n_windows, dtype=np.int64)
    eslot = np.zeros(E, dtype=np.int64)
    for e in ord_e:
        w = edge_win[e]
        core, wl = divmod(w, wpc)
        capw = nt_w[wl] * 128
        if is_fix[e]:
            fillb[w] += 1
            eslot[e] = core * epc + woff[wl] + capw - fillb[w]
        else:
            eslot[e] = core * epc + woff[wl] + fill[w]
            fill[w] += 1
    assert all(fill[w] + fillb[w] <= nt_w[w % wpc] * 128
               for w in range(n_windows))
    assert fillb.max() <= 128, "fix edges must fit the last tile"

    tot_slots = n_cores * epc
    slot_src = np.zeros(tot_slots, dtype=np.int64)
    slot_used = np.zeros(tot_slots, dtype=bool)
    slot_vloc = np.zeros(tot_slots, dtype=np.int64)
    slot_invdeg = np.zeros(tot_slots, dtype=np.float32)
    slot_ea = np.zeros((tot_slots, 3), dtype=np.float32)
    slot_src[eslot] = devnode[src]
    del devnode
    slot_used[eslot] = True
    slot_vloc[eslot] = node_slot[dst]
    slot_invdeg[eslot] = invdeg_node[dst]
    slot_ea[eslot] = np.asarray(edge_attr, dtype=np.float32)

    # weight repacks: T_cm [66*64, 64]: rows 0..4095 = k2_W, rows
    # 4096..4159 = k2_b (paired with z's invdeg column), rest zero.
    # chunk layout: T_sb[p, k*64+o] = T_cm[k*128+p, o]
    T_cm = np.zeros((66 * 64, W), dtype=np.float32)
    T_cm[: 64 * 64] = np.ascontiguousarray(
        np.asarray(k2_W, dtype=np.float32).reshape(64, 64, 64)
    ).reshape(64 * 64, W)
    T_cm[64 * 64 : 65 * 64] = np.asarray(k2_b, dtype=np.float32).reshape(64, 64)
    T_sb = np.ascontiguousarray(
        T_cm.reshape(NCH, 128, W).transpose(1, 0, 2)
    ).reshape(128, NCH * W).astype(np.float16)

    # k1 extended to 66 cols: 0-63 = [k1_W; k1_b], 64 = bias-row one (the
    # invdeg ACT-scale turns it into the invdeg column), 65 = zero pad.
    k1_Wb = np.zeros((4, 66), dtype=np.float16)
    k1_Wb[:3, :64] = np.asarray(k1_W, dtype=np.float16)
    k1_Wb[3, :64] = np.asarray(k1_b, dtype=np.float16)
    k1_Wb[3, 64] = 1.0

    # h rows padded to 128 f16 (=256B) so SWDGE dma_gather's 256B-multiple
    # row-stride restriction is met; cols 64.. are never read by compute.
    h0 = np.zeros((nodes_pad, 2 * W), dtype=np.float16)
    h0[plan.devnode, :W] = (np.asarray(x, np.float32) @ np.asarray(fc1_W, np.float32)
                       + np.asarray(fc1_b, np.float32)).astype(np.float16)

    ident = np.eye(64, dtype=np.float16)
    root_np = np.asarray(root, dtype=np.float16)
    fc2_np = np.asarray(fc2_W, dtype=np.float16).reshape(W, 1)

    for r in range(n_cores):
        sl = slice(r * epc, (r + 1) * epc)
        c_ea = slot_ea[sl]
        c_used = slot_used[sl]
        c_invd = slot_invdeg[sl]
        c_vloc = slot_vloc[sl]
        c_src = slot_src[sl]

        eaT = np.zeros((4, epc), dtype=np.float16)
        eaT[:3] = c_ea.T.astype(np.float16)
        eaT[3] = 1.0
        # invdeg in [partition, tile] layout (slot s -> (s//128, s%128))
        invd = np.ascontiguousarray(
            c_invd.reshape(ntiles, 128).T)                       # [128, ntiles]
        tt = np.arange(epc) // 128
        pp = np.arange(epc) % 128
        segT = np.zeros((ntiles, 128, WIN), dtype=np.float16)
        segT[tt[c_used], pp[c_used], c_vloc[c_used]] = 1.0
        segT = np.ascontiguousarray(segT.transpose(1, 0, 2)).reshape(128, ntiles * WIN)

        idx = np.zeros((128, epc // 16), dtype=np.int16)
        base = c_src.astype(np.int16).reshape(epc // 16, 16).T   # [16, epc/16]
        for g in range(8):
            idx[16 * g : 16 * (g + 1)] = base

        h0T = np.ascontiguousarray(
            h0[r * wpc * WIN : (r + 1) * wpc * WIN, :W].T)       # [64, wpc*WIN]


        plan.in_maps.append({
            "eaT": eaT,
            "invdeg": invd,
            "segmatT": segT,
            "idx": idx,
            "h0": h0,
            "h0T": h0T,
            "T_sb": T_sb,
            "k1_Wb": k1_Wb,
            "root": root_np,
            "fc2_W": fc2_np,
            "fc2_b": np.full((WIN, 1), plan.fc2_b, dtype=np.float32),
            "ident": ident,
        })
    return plan


def build_program(plan: Plan, debug=False, single_core=False):
    """Build the SPMD Bass program (one program, run on all cores).

    single_core=True replaces the AllGather with direct local h_full writes
    (and drops addr_space="Shared") so the program can run under TimelineSim
    for cost modeling."""
    W = WIDTH
    WPC = plan.wpc
    WIN = plan.win
    NT_W = plan.nt_w
    TOFF = plan.tile_off
    NTILES = plan.ntiles
    EPC = plan.epc
    NPAD = plan.nodes_pad
    DEP = plan.depth
    NC_ = plan.n_cores
    Relu = mybir.ActivationFunctionType.Relu

    nc = bacc.Bacc("TRN2", target_bir_lowering=False, debug=debug,
                   num_devices=NC_)

    # ---- I/O ----
    eaT_d = nc.dram_tensor("eaT", [4, EPC], F16, kind="ExternalInput")
    invd_d = nc.dram_tensor("invdeg", [128, NTILES], F32, kind="ExternalInput")
    segT_d = nc.dram_tensor("segmatT", [128, NTILES * WIN], F16, kind="ExternalInput")
    idx_d = nc.dram_tensor("idx", [128, EPC // 16], I16, kind="ExternalInput")
    h0_d = nc.dram_tensor("h0", [NPAD, 2 * W], F16, kind="ExternalInput")
    h0T_d = nc.dram_tensor("h0T", [W, WPC * WIN], F16, kind="ExternalInput")
    Tsb_d = nc.dram_tensor("T_sb", [128, NCH * W], F16, kind="ExternalInput")
    k1_d = nc.dram_tensor("k1_Wb", [4, 66], F16, kind="ExternalInput")
    root_d = nc.dram_tensor("root", [W, W], F16, kind="ExternalInput")
    fc2_d = nc.dram_tensor("fc2_W", [W, 1], F16, kind="ExternalInput")
    fc2b_d = nc.dram_tensor("fc2_b", [WIN, 1], F32, kind="ExternalInput")
    id_d = nc.dram_tensor("ident", [64, 64], F16, kind="ExternalInput")
    y_d = nc.dram_tensor("y", [WPC * WIN, 1], F32, kind="ExternalOutput")

    # internal DRAM for the h exchange
    h_slice = [nc.dram_tensor(f"h_slice{i}", [WPC * WIN, 2 * W], F16)
               for i in range(DEP - 1)]
    if single_core:
        h_fullA = [nc.dram_tensor(f"h_fullA{i}", [NPAD, 2 * W], F16)
                   for i in range(DEP - 1)]
        h_full = [nc.dram_tensor(f"h_fullB{i}", [NPAD, 2 * W], F16)
                  for i in range(DEP - 1)]
    else:
        h_full = [nc.dram_tensor(f"h_full{i}", [NPAD, 2 * W], F16,
                                 addr_space="Shared")
                  for i in range(DEP - 1)]
        h_fullA = h_full

    MAXNT = max(NT_W)
    with tile.TileContext(nc) as tc:
        with (
            tc.tile_pool(name="const", bufs=1) as cpool,
            tc.tile_pool(name="hsrc", bufs=2) as hsrc_pool,
            tc.tile_pool(name="z", bufs=2 * MAXNT + 1) as zpool,
            tc.tile_pool(name="zsum_sb", bufs=2) as zsum_sb_pool,
            tc.tile_pool(name="hT", bufs=2) as hT_pool,
            tc.tile_pool(name="small", bufs=4) as spool,
            tc.tile_pool(name="zsum_ps", bufs=2, space="PSUM") as zsum_ps_pool,
            tc.tile_pool(name="agg_ps", bufs=2, space="PSUM") as agg_ps_pool,
            tc.tile_pool(name="tr_ps", bufs=1, space="PSUM") as tr_ps_pool,
        ):
            nc.gpsimd.load_library(library_config.mlp)

            # preload the ACT function table (1.3us) under the const DMAs;
            # Copy needs no bias const-AP (whose DMA would land late)
            warm = cpool.tile([1, 1], F32)
            nc.vector.memset(warm[:], 0.0)
            nc.scalar.activation(warm[:], warm[:],
                                 mybir.ActivationFunctionType.Copy)

            # ---- constants; gather-critical tensors first so window 0's
            # gather + e2 chain + first zsum start ASAP ----
            n0 = NT_W[0] * 128 // 16
            idx0 = cpool.tile([128, n0], I16)
            nc.sync.dma_start(idx0[:], idx_d[:, :n0])
            idx = cpool.tile([128, EPC // 16], I16)
            nc.sync.dma_start(idx[:], idx_d[:])
            eaT = cpool.tile([4, EPC], F16)
            nc.sync.dma_start(eaT[:], eaT_d[:])
            invd = cpool.tile([128, NTILES], F32)
            nc.sync.dma_start(invd[:], invd_d[:])
            k1 = cpool.tile([4, 66], F16)
            nc.sync.dma_start(k1[:], k1_d[:])
            segT = cpool.tile([128, NTILES * WIN], F16)
            nc.sync.dma_start(segT[:], segT_d[:])
            Tsb = cpool.tile([128, NCH * W], F16)
            nc.sync.dma_start(Tsb[:], Tsb_d[:])
            h0T = cpool.tile([W, WPC * WIN], F16)
            nc.sync.dma_start(h0T[:], h0T_d[:])
            rootW = cpool.tile([W, W], F16)
            nc.sync.dma_start(rootW[:], root_d[:])
            fc2 = cpool.tile([W, 1], F16)
            nc.sync.dma_start(fc2[:], fc2_d[:])
            fc2b = cpool.tile([WIN, 1], F32)
            nc.sync.dma_start(fc2b[:], fc2b_d[:])
            ident = cpool.tile([64, 64], F16)
            nc.sync.dma_start(ident[:], id_d[:])

            # ---- e2dup: [128, NTILES*64*2] fp16, every value twice so the
            # z-build APs end in a packed (stride 1, count 2) dim on ALL
            # operands -> DVE 2x mode. relu(x*invdeg) = invdeg*relu(x) folds
            # the scatter-mean denominator into the ACT scale. ----
            e2dup = cpool.tile([128, NTILES * 66 * 2], F16)

            def build_e2dup(t):
                e2_ps = agg_ps_pool.tile([128, 66], F32, tag="a")
                nc.tensor.matmul(e2_ps[:], eaT[:, t * 128:(t + 1) * 128],
                                 k1[:], start=True, stop=True)
                dup = e2dup[:, t * 132:(t + 1) * 132] \
                    .rearrange("p (c b) -> p c b", b=2)
                for b in range(2):
                    nc.scalar.activation(dup[:, :, b], e2_ps[:], Relu,
                                         scale=invd[:, t: t + 1])

            def build_e2dup_win(w):
                for et in range(NT_W[w]):
                    build_e2dup(TOFF[w] + et)

            # only window 0's e2dup up front: emitting all of it here would
            # queue 24us of ACT work ahead of iteration 0's PSUM drains (ACT
            # executes in order) and stall the whole pipeline; z(w) also
            # waits on every e2dup write emitted before it (tile-granular
            # dependency tracking), so later windows' builds are staggered
            # through iteration 0.
            build_e2dup_win(0)

            hT_cur = h0T
            y_sb = spool.tile([WIN, WPC], F32, tag="y")

            for it in range(DEP):
                gsrcA = h0_d if it == 0 else h_fullA[it - 1]
                gsrcB = h0_d if it == 0 else h_full[it - 1]
                # Two gathers per window into its own tiles: the MAIN gather
                # (tiles 0..nt-2, whose edges by construction have sources in
                # windows 0..wpc-2) runs as soon as those h windows land and
                # overlaps the last window's compute; only the small FIX
                # gather (last tile) waits for the final h window.
                h_srcs = [None] * WPC
                h_fix = [None] * WPC

                def issue_gather(w):
                    nt = NT_W[w]
                    o = TOFF[w] * 128
                    nm = (nt - 1) * 128
                    ix = idx0 if (w == 0 and it == 0) else idx
                    hs_w = hsrc_pool.tile([128, nt - 1, 2 * W], F16,
                                          tag=f"h{w}")
                    nc.gpsimd.dma_gather(
                        hs_w[:], gsrcA[:],
                        ix[:, o // 16:(o + nm) // 16], nm, nm, 2 * W)
                    h_srcs[w] = hs_w
                    hf_w = hsrc_pool.tile([128, 1, 2 * W], F16, tag=f"hf{w}")
                    nc.gpsimd.dma_gather(
                        hf_w[:], gsrcB[:],
                        ix[:, (o + nm) // 16:(o + nm + 128) // 16], 128, 128,
                        2 * W)
                    h_fix[w] = hf_w

                for _w0 in range(4):
                    issue_gather(_w0)
                hT_next = hT_pool.tile([W, WPC * WIN], F16)

                def write_h(w):
                    # transpose hT_next[w] and write it to DRAM. Deferred by
                    # one window (emitted after the NEXT window's zsum
                    # passes) so the in-order PE never stalls waiting for
                    # relu(w) on ACT; the last window is emitted inline since
                    # its write gates the next iteration's fix gathers.
                    h_ps = tr_ps_pool.tile([WIN, 64], F16, tag="tr")
                    nc.tensor.transpose(h_ps[:],
                                        hT_next[:, w * WIN:(w + 1) * WIN],
                                        ident[:])
                    h_sb = spool.tile([WIN, 64], F16, tag="hnew")
                    nc.scalar.copy(h_sb[:], h_ps[:])
                    if single_core:
                        if w < WPC - 1:
                            nc.sync.dma_start(
                                h_fullA[it][w * WIN:(w + 1) * WIN, :W],
                                h_sb[:])
                            if w == WPC - 2:
                                # B gets windows 0..wpc-2 via one bulk
                                # copy (fix gathers wait for the last
                                # window anyway, so this is off the
                                # critical path)
                                nc.sync.dma_start(
                                    h_full[it][: (WPC - 1) * WIN, :],
                                    h_fullA[it][: (WPC - 1) * WIN, :])
                        else:
                            nc.sync.dma_start(
                                h_full[it][w * WIN:(w + 1) * WIN, :W],
                                h_sb[:])
                    else:
                        nc.sync.dma_start(
                            h_slice[it][w * WIN:(w + 1) * WIN, :W], h_sb[:])

                pending_tr = []
                for w in range(WPC):
                    nt = NT_W[w]
                    t0 = TOFF[w]
                    zs = []
                    for et in range(nt):
                        t = t0 + et
                        z = zpool.tile([128, NCH * 128], F16)
                        # all-fp16 operands with packed (1,2) last dims hit
                        # the DVE 2x perf mode (stride-0 last dims do not)
                        zv = z[:].rearrange("p (c a b) -> p c a b", c=66, b=2)
                        h_t = h_srcs[w] if et < nt - 1 else h_fix[w]
                        e_t = et if et < nt - 1 else 0
                        hs = h_t[:, e_t, :W] \
                            .rearrange("p (a b) -> p a b", b=2) \
                            .unsqueeze(1).broadcast_to((128, 66, 32, 2))
                        e2 = e2dup[:, t * 132:(t + 1) * 132] \
                            .rearrange("p (c b) -> p c b", b=2) \
                            .unsqueeze(2).broadcast_to((128, 66, 32, 2))
                        if et == nt - 1 and w == 0:
                            # window 0's fix tile sits on the iteration
                            # boundary critical path: build it in KH-chunk
                            # pieces so pass 0 can start after the first one
                            for c0 in range(0, 66, 16):
                                c1 = min(c0 + 16, 66)
                                nc.vector.tensor_mul(
                                    zv[:, c0:c1, :, :], hs[:, c0:c1, :, :],
                                    e2[:, c0:c1, :, :])
                            zs.append(z)
                            continue
                        # offload part of the first tile of each window to
                        # the (idle) GPSIMD engine; DVE builds the rest.
                        # Not in iteration 0's first windows: Pool is still
                        # busy with the initial gather burst there.
                        if et == 0 and not (it == 0 and w < 3):
                            nc.gpsimd.tensor_mul(
                                zv[:, :30, :, :], hs[:, :30, :, :],
                                e2[:, :30, :, :])
                            nc.vector.tensor_mul(
                                zv[:, 30:, :, :], hs[:, 30:, :, :],
                                e2[:, 30:, :, :])
                        else:
                            nc.vector.tensor_mul(zv, hs, e2)
                        zs.append(z)
                    if w + 4 < WPC:
                        issue_gather(w + 4)
                    if it == 0 and w + 1 < WPC:
                        build_e2dup_win(w + 1)
                    # zsum in KH-chunk PSUM passes (back-to-back on PE; the
                    # drains pipeline on ACT), then the T-contract block.
                    zsum_sb = zsum_sb_pool.tile([128, NCH * WIN], F16)
                    for p0 in range(0, NCH, KH):
                        p1 = min(p0 + KH, NCH)
                        zsum_ps = zsum_ps_pool.tile([128, KH * WIN], F32)
                        for k in range(p0, p1):
                            for et in range(nt):
                                nc.tensor.matmul(
                                    zsum_ps[:, (k - p0) * WIN:(k - p0 + 1) * WIN],
                                    zs[et][:, k * 128:(k + 1) * 128],
                                    segT[:, (t0 + et) * WIN:(t0 + et + 1) * WIN],
                                    start=(et == 0), stop=(et == nt - 1))
                        # keep the DVE free for z-builds (critical engine) —
                        # drain PSUM on ACT
                        nc.scalar.copy(zsum_sb[:, p0 * WIN:p1 * WIN],
                                       zsum_ps[:, :(p1 - p0) * WIN])
                    if pending_tr:
                        write_h(pending_tr.pop())
                    agg_ps = agg_ps_pool.tile([64, WIN], F32, tag="a")
                    for k in range(NCH):
                        nc.tensor.matmul(agg_ps[:],
                                         Tsb[:, k * W:(k + 1) * W],
                                         zsum_sb[:, k * WIN:(k + 1) * WIN],
                                         start=(k == 0), stop=False)
                    nc.tensor.matmul(agg_ps[:], rootW[:],
                                     hT_cur[:, w * WIN:(w + 1) * WIN],
                                     start=False, stop=True)
                    nc.scalar.activation(hT_next[:, w * WIN:(w + 1) * WIN],
                                         agg_ps[:], Relu)
                    if it == DEP - 1:
                        y_ps = agg_ps_pool.tile([WIN, 1], F32, tag="a")
                        nc.tensor.matmul(y_ps[:],
                                         hT_next[:, w * WIN:(w + 1) * WIN],
                                         fc2[:], start=True, stop=True)
                        nc.vector.tensor_add(y_sb[:, w: w + 1], y_ps[:], fc2b[:])
                    else:
                        if w >= WPC - 2:
                            # w8's write gates the next iteration's main
                            # gathers, w9's its fix gathers: keep both inline
                            if pending_tr:
                                write_h(pending_tr.pop())
                            write_h(w)
                        else:
                            pending_tr.append(w)
                hT_cur = hT_next
                if it < DEP - 1 and not single_core:
                    nc.gpsimd.collective_compute(
                        "AllGather",
                        mybir.AluOpType.bypass,
                        ins=[h_slice[it][:].opt()],
                        outs=[h_full[it][:].opt()],
                        replica_groups=[list(range(NC_))],
                    )

            # ---- output ----
            y_view = y_d[:].rearrange("(w v) o -> v (w o)", w=WPC)
            nc.sync.dma_start(y_view, y_sb[:])

    nc.compile()
    return nc


def kernel(**inputs) -> np.ndarray:
    from concourse.bass_utils import run_bass_kernel_spmd

    plan = make_plan(**{k: np.asarray(v) for k, v in inputs.items()})
    nc = build_program(plan)
    core_ids = list(range(plan.n_cores))
    res = run_bass_kernel_spmd(nc, plan.in_maps, core_ids,
                               trace=bool(int(os.environ.get("KERNEL_TRACE", "0"))))
    y = np.concatenate([res.results[r]["y"] for r in range(plan.n_cores)], axis=0)
    out = y[plan.devnode]
    kernel.last_results = res
    kernel.last_plan = plan
    return out


# revision 24
# speedup vs baseline: 2.0187x; 1.0078x over previous
"""Trainium2 Bass kernel for nn_Net_MP_68805376082308 (NNConv-style GNN).

Reference computation (see problem statement):
    h = x@fc1 + b
    e2 = relu(edge_attr@k1 + b1)                     # [E, 64]
    ew = (e2 @ k2 + b2).reshape(E, 64, 64)           # never materialized here!
    for 4 iters:
        msg  = einsum('ei,eio->eo', h[src], ew)
        agg  = segment_sum(msg, dst) / max(deg,1)
        h    = relu(agg + h@root)
    out = h @ fc2 + b

Device algorithm (per core, node-sharded, dst-grouped edge slots):
    e2s[e, c]    = relu((edge_attr@k1+b1)[e,c]) * invdeg[dst[e]]  (c in 0..63)
    z[e, c*64+i] = e2s[e,c] * h[src[e], i]        # DVE, fp16 pair-trick APs
    zsumT[ci, v] = sum_e z[e,ci] * SegMat[e,v]    # PE, z stationary (scatter
                                                  #  commutes with k2 contract)
    aggT[o, v]   = T_cm.T @ zsumT + root.T @ hT   # PE (T_cm rows 4096..4159
                                                  #  carry k2_b, paired with
                                                  #  z's invdeg column)
    hT           = relu(aggT)                     # ACT
    h[src] gather via SWDGE dma_gather; h exchanged across 8 cores with an
    AllGather after each iteration.

Windows hold 128 dst-node slots and 5 edge tiles each. Edges whose SOURCE
node lies in any core's last window are segregated into each window's last
("fix") tile: the other tiles' gathers then depend only on h windows 0..8 and
overlap the last window's compute at the iteration boundary (split h_fullA/B
tensors express this to the dependency tracker in the single-core cost model;
the real 8-core path keeps one h_full fed by an AllGather).

kernel(**inputs) takes the FULL unsharded inputs and returns [10000, 1] fp32.
"""

import math
import os
import sys
from dataclasses import dataclass, field

import numpy as np

sys.path.insert(0, "/opt/trn_rl_repo")

import concourse.bacc as bacc
import concourse.bass as bass
import concourse.mybir as mybir
import concourse.tile as tile
from concourse import library_config

F32 = mybir.dt.float32
F16 = mybir.dt.float16
I16 = mybir.dt.int16

WIDTH = 64
DEPTH = 4
NCH = 33                # ci chunks of 128 (66*64/128)
KH = 8                  # chunks per PSUM pass


@dataclass
class Plan:
    """Host-side preprocessing result: all per-core device input arrays plus
    the compile-time structure constants."""

    n_cores: int
    wpc: int                 # windows per core
    nt_w: list = None        # tiles per window (same layout for every core)
    nodes_pad: int = 0
    depth: int = DEPTH
    win: int = 128           # nodes per scatter window
    devnode: np.ndarray = None     # [N] original node -> device row
    in_maps: list = field(default_factory=list)
    fc2_b: float = 0.0

    @property
    def ntiles(self):        # edge tiles per core
        return sum(self.nt_w)

    @property
    def epc(self):           # edge slots per core
        return self.ntiles * 128

    @property
    def tile_off(self):      # first tile index of each window
        off, out = 0, []
        for n in self.nt_w:
            out.append(off)
            off += n
        return out


def make_plan(x, edge_index, edge_attr, fc1_W, fc1_b, k1_W, k1_b, k2_W, k2_b,
              root, conv_b, fc2_W, fc2_b, n_cores=8, depth=DEPTH):
    W = WIDTH
    N = x.shape[0]
    E = edge_index.shape[1]
    src = np.asarray(edge_index[0], dtype=np.int64)
    dst = np.asarray(edge_index[1], dtype=np.int64)
    assert np.all(np.asarray(conv_b) == 0.0), "kernel assumes conv_b == 0"

    WIN = 128
    wpc = max(1, int(math.ceil(N / WIN / n_cores)))
    n_windows = n_cores * wpc
    nodes_pad = n_windows * WIN

    counts = np.bincount(dst, minlength=N).astype(np.float64)
    denom = np.where(counts > 0, counts, 1.0)
    invdeg_node = (1.0 / denom).astype(np.float32)

    # Per-window edge-tile capacities (uniform; asymmetric layouts lose:
    # low-degree tail nodes still bring >1 tile of edges and the greedy
    # overflow then costs more steady-state tiles than the tail saves).
    base_nt = [5] * 10
    if wpc != 10:  # generic fallback: balanced with one tiny last window
        per = int(math.ceil(E / n_cores / max(1, wpc - 1) / 128)) + 1
        base_nt = [per] * (wpc - 1) + [1]
    cap = np.array([nt * 128 for _ in range(n_cores) for nt in base_nt],
                   dtype=np.int64)

    order = np.argsort(-counts, kind="stable")
    win_edges = np.zeros(n_windows, dtype=np.int64)
    win_fill = np.zeros(n_windows, dtype=np.int64)
    node_window = np.zeros(N, dtype=np.int64)
    node_slot = np.zeros(N, dtype=np.int64)
    NEG = -(1 << 60)
    # greedy: place desc-degree nodes into the window with the most remaining
    # edge capacity that still has node slots; grow a window's capacity by a
    # tile if nothing fits.
    rem = cap.copy()
    for n in order:
        d = int(counts[n])
        w = int(np.argmax(rem))
        if rem[w] < d:
            cap[w] += 128 * int(math.ceil((d - rem[w]) / 128))
            rem[w] = cap[w] - win_edges[w]
        node_window[n] = w
        node_slot[n] = win_fill[w]
        win_fill[w] += 1
        win_edges[w] += d
        rem[w] = cap[w] - win_edges[w] if win_fill[w] < WIN else NEG
    nt_all = (cap // 128).reshape(n_cores, wpc)
    # every core runs one compiled program -> shared nt layout: per-position max
    nt_w = [int(nt_all[:, i].max()) for i in range(wpc)]

    plan = Plan(n_cores=n_cores, wpc=wpc, nt_w=nt_w, nodes_pad=nodes_pad,
                depth=depth, win=WIN,
                fc2_b=float(np.asarray(fc2_b).reshape(())))
    ntiles = plan.ntiles
    epc = plan.epc
    woff = [128 * t for t in plan.tile_off]   # slot offset of window in core

    plan.devnode = node_window * WIN + node_slot

    # edge -> slot within its dst window. Edges whose SOURCE lies in any
    # core's last window ("fix" edges) go to the tail of the window's slot
    # range (the last tile): the other tiles' gather then depends only on
    # h windows 0..wpc-2 and overlaps the last window's compute.
    devnode = node_window * WIN + node_slot
    edge_win = node_window[dst]
    is_fix = (devnode[src] % (wpc * WIN)) >= (wpc - 1) * WIN
    ord_e = np.argsort(edge_win, kind="stable")
    fill = np.zeros(n_windows, dtype=np.int64)
    fillb = np.zeros(n_windows, dtype=np.int64)
    eslot = np.zeros(E, dtype=np.int64)
    for e in ord_e:
        w = edge_win[e]
        core, wl = divmod(w, wpc)
        capw = nt_w[wl] * 128
        if is_fix[e]:
            fillb[w] += 1
            eslot[e] = core * epc + woff[wl] + capw - fillb[w]
        else:
            eslot[e] = core * epc + woff[wl] + fill[w]
            fill[w] += 1
    assert all(fill[w] + fillb[w] <= nt_w[w % wpc] * 128
               for w in range(n_windows))
    assert fillb.max() <= 128, "fix edges must fit the last tile"

    tot_slots = n_cores * epc
    slot_src = np.zeros(tot_slots, dtype=np.int64)
    slot_used = np.zeros(tot_slots, dtype=bool)
    slot_vloc = np.zeros(tot_slots, dtype=np.int64)
    slot_invdeg = np.zeros(tot_slots, dtype=np.float32)
    slot_ea = np.zeros((tot_slots, 3), dtype=np.float32)
    slot_src[eslot] = devnode[src]
    del devnode
    slot_used[eslot] = True
    slot_vloc[eslot] = node_slot[dst]
    slot_invdeg[eslot] = invdeg_node[dst]
    slot_ea[eslot] = np.asarray(edge_attr, dtype=np.float32)

    # weight repacks: T_cm [66*64, 64]: rows 0..4095 = k2_W, rows
    # 4096..4159 = k2_b (paired with z's invdeg column), rest zero.
    # chunk layout: T_sb[p, k*64+o] = T_cm[k*128+p, o]
    T_cm = np.zeros((66 * 64, W), dtype=np.float32)
    T_cm[: 64 * 64] = np.ascontiguousarray(
        np.asarray(k2_W, dtype=np.float32).reshape(64, 64, 64)
    ).reshape(64 * 64, W)
    T_cm[64 * 64 : 65 * 64] = np.asarray(k2_b, dtype=np.float32).reshape(64, 64)
    T_sb = np.ascontiguousarray(
        T_cm.reshape(NCH, 128, W).transpose(1, 0, 2)
    ).reshape(128, NCH * W).astype(np.float16)

    # k1 extended to 66 cols: 0-63 = [k1_W; k1_b], 64 = bias-row one (the
    # invdeg ACT-scale turns it into the invdeg column), 65 = zero pad.
    k1_Wb = np.zeros((4, 66), dtype=np.float16)
    k1_Wb[:3, :64] = np.asarray(k1_W, dtype=np.float16)
    k1_Wb[3, :64] = np.asarray(k1_b, dtype=np.float16)
    k1_Wb[3, 64] = 1.0

    # h rows padded to 128 f16 (=256B) so SWDGE dma_gather's 256B-multiple
    # row-stride restriction is met; cols 64.. are never read by compute.
    h0 = np.zeros((nodes_pad, 2 * W), dtype=np.float16)
    h0[plan.devnode, :W] = (np.asarray(x, np.float32) @ np.asarray(fc1_W, np.float32)
                       + np.asarray(fc1_b, np.float32)).astype(np.float16)

    ident = np.eye(64, dtype=np.float16)
    root_np = np.asarray(root, dtype=np.float16)
    fc2_np = np.asarray(fc2_W, dtype=np.float16).reshape(W, 1)

    for r in range(n_cores):
        sl = slice(r * epc, (r + 1) * epc)
        c_ea = slot_ea[sl]
        c_used = slot_used[sl]
        c_invd = slot_invdeg[sl]
        c_vloc = slot_vloc[sl]
        c_src = slot_src[sl]

        eaT = np.zeros((4, epc), dtype=np.float16)
        eaT[:3] = c_ea.T.astype(np.float16)
        eaT[3] = 1.0
        # invdeg in [partition, tile] layout (slot s -> (s//128, s%128))
        invd = np.ascontiguousarray(
            c_invd.reshape(ntiles, 128).T)                       # [128, ntiles]
        tt = np.arange(epc) // 128
        pp = np.arange(epc) % 128
        segT = np.zeros((ntiles, 128, WIN), dtype=np.float16)
        segT[tt[c_used], pp[c_used], c_vloc[c_used]] = 1.0
        segT = np.ascontiguousarray(segT.transpose(1, 0, 2)).reshape(128, ntiles * WIN)

        idx = np.zeros((128, epc // 16), dtype=np.int16)
        base = c_src.astype(np.int16).reshape(epc // 16, 16).T   # [16, epc/16]
        for g in range(8):
            idx[16 * g : 16 * (g + 1)] = base

        h0T = np.ascontiguousarray(
            h0[r * wpc * WIN : (r + 1) * wpc * WIN, :W].T)       # [64, wpc*WIN]


        plan.in_maps.append({
            "eaT": eaT,
            "invdeg": invd,
            "segmatT": segT,
            "idx": idx,
            "h0": h0,
            "h0T": h0T,
            "T_sb": T_sb,
            "k1_Wb": k1_Wb,
            "root": root_np,
            "fc2_W": fc2_np,
            "fc2_b": np.full((WIN, 1), plan.fc2_b, dtype=np.float32),
            "ident": ident,
        })
    return plan


def build_program(plan: Plan, debug=False, single_core=False):
    """Build the SPMD Bass program (one program, run on all cores).

    single_core=True replaces the AllGather with direct local h_full writes
    (and drops addr_space="Shared") so the program can run under TimelineSim
    for cost modeling."""
    W = WIDTH
    WPC = plan.wpc
    WIN = plan.win
    NT_W = plan.nt_w
    TOFF = plan.tile_off
    NTILES = plan.ntiles
    EPC = plan.epc
    NPAD = plan.nodes_pad
    DEP = plan.depth
    NC_ = plan.n_cores
    Relu = mybir.ActivationFunctionType.Relu

    nc = bacc.Bacc("TRN2", target_bir_lowering=False, debug=debug,
                   num_devices=NC_)

    # ---- I/O ----
    eaT_d = nc.dram_tensor("eaT", [4, EPC], F16, kind="ExternalInput")
    invd_d = nc.dram_tensor("invdeg", [128, NTILES], F32, kind="ExternalInput")
    segT_d = nc.dram_tensor("segmatT", [128, NTILES * WIN], F16, kind="ExternalInput")
    idx_d = nc.dram_tensor("idx", [128, EPC // 16], I16, kind="ExternalInput")
    h0_d = nc.dram_tensor("h0", [NPAD, 2 * W], F16, kind="ExternalInput")
    h0T_d = nc.dram_tensor("h0T", [W, WPC * WIN], F16, kind="ExternalInput")
    Tsb_d = nc.dram_tensor("T_sb", [128, NCH * W], F16, kind="ExternalInput")
    k1_d = nc.dram_tensor("k1_Wb", [4, 66], F16, kind="ExternalInput")
    root_d = nc.dram_tensor("root", [W, W], F16, kind="ExternalInput")
    fc2_d = nc.dram_tensor("fc2_W", [W, 1], F16, kind="ExternalInput")
    fc2b_d = nc.dram_tensor("fc2_b", [WIN, 1], F32, kind="ExternalInput")
    id_d = nc.dram_tensor("ident", [64, 64], F16, kind="ExternalInput")
    y_d = nc.dram_tensor("y", [WPC * WIN, 1], F32, kind="ExternalOutput")

    # internal DRAM for the h exchange
    h_slice = [nc.dram_tensor(f"h_slice{i}", [WPC * WIN, 2 * W], F16)
               for i in range(DEP - 1)]
    if single_core:
        h_fullA = [nc.dram_tensor(f"h_fullA{i}", [NPAD, 2 * W], F16)
                   for i in range(DEP - 1)]
        h_full = [nc.dram_tensor(f"h_fullB{i}", [NPAD, 2 * W], F16)
                  for i in range(DEP - 1)]
    else:
        h_full = [nc.dram_tensor(f"h_full{i}", [NPAD, 2 * W], F16,
                                 addr_space="Shared")
                  for i in range(DEP - 1)]
        h_fullA = h_full

    MAXNT = max(NT_W)
    with tile.TileContext(nc) as tc:
        with (
            tc.tile_pool(name="const", bufs=1) as cpool,
            tc.tile_pool(name="hsrc", bufs=2) as hsrc_pool,
            tc.tile_pool(name="z", bufs=2 * MAXNT + 1) as zpool,
            tc.tile_pool(name="zsum_sb", bufs=2) as zsum_sb_pool,
            tc.tile_pool(name="hT", bufs=2) as hT_pool,
            tc.tile_pool(name="small", bufs=4) as spool,
            tc.tile_pool(name="zsum_ps", bufs=2, space="PSUM") as zsum_ps_pool,
            tc.tile_pool(name="agg_ps", bufs=2, space="PSUM") as agg_ps_pool,
            tc.tile_pool(name="tr_ps", bufs=1, space="PSUM") as tr_ps_pool,
        ):
            nc.gpsimd.load_library(library_config.mlp)

            # preload the ACT function table (1.3us) under the const DMAs;
            # Copy needs no bias const-AP (whose DMA would land late)
            warm = cpool.tile([1, 1], F32)
            nc.vector.memset(warm[:], 0.0)
            nc.scalar.activation(warm[:], warm[:],
                                 mybir.ActivationFunctionType.Copy)

            # ---- constants; gather-critical tensors first so window 0's
            # gather + e2 chain + first zsum start ASAP ----
            n0 = NT_W[0] * 128 // 16
            idx0 = cpool.tile([128, n0], I16)
            nc.sync.dma_start(idx0[:], idx_d[:, :n0])
            idx = cpool.tile([128, EPC // 16], I16)
            nc.sync.dma_start(idx[:], idx_d[:])
            eaT = cpool.tile([4, EPC], F16)
            nc.sync.dma_start(eaT[:], eaT_d[:])
            invd = cpool.tile([128, NTILES], F32)
            nc.sync.dma_start(invd[:], invd_d[:])
            k1 = cpool.tile([4, 66], F16)
            nc.sync.dma_start(k1[:], k1_d[:])
            segT = cpool.tile([128, NTILES * WIN], F16)
            nc.sync.dma_start(segT[:], segT_d[:])
            Tsb = cpool.tile([128, NCH * W], F16)
            nc.sync.dma_start(Tsb[:], Tsb_d[:])
            h0T = cpool.tile([W, WPC * WIN], F16)
            nc.sync.dma_start(h0T[:], h0T_d[:])
            rootW = cpool.tile([W, W], F16)
            nc.sync.dma_start(rootW[:], root_d[:])
            fc2 = cpool.tile([W, 1], F16)
            nc.sync.dma_start(fc2[:], fc2_d[:])
            fc2b = cpool.tile([WIN, 1], F32)
            nc.sync.dma_start(fc2b[:], fc2b_d[:])
            ident = cpool.tile([64, 64], F16)
            nc.sync.dma_start(ident[:], id_d[:])

            # ---- e2dup: [128, NTILES*64*2] fp16, every value twice so the
            # z-build APs end in a packed (stride 1, count 2) dim on ALL
            # operands -> DVE 2x mode. relu(x*invdeg) = invdeg*relu(x) folds
            # the scatter-mean denominator into the ACT scale. ----
            e2dup = cpool.tile([128, NTILES * 66 * 2], F16)

            def build_e2dup(t):
                e2_ps = agg_ps_pool.tile([128, 66], F32, tag="a")
                nc.tensor.matmul(e2_ps[:], eaT[:, t * 128:(t + 1) * 128],
                                 k1[:], start=True, stop=True)
                dup = e2dup[:, t * 132:(t + 1) * 132] \
                    .rearrange("p (c b) -> p c b", b=2)
                for b in range(2):
                    nc.scalar.activation(dup[:, :, b], e2_ps[:], Relu,
                                         scale=invd[:, t: t + 1])

            def build_e2dup_win(w):
                for et in range(NT_W[w]):
                    build_e2dup(TOFF[w] + et)

            # only window 0's e2dup up front: emitting all of it here would
            # queue 24us of ACT work ahead of iteration 0's PSUM drains (ACT
            # executes in order) and stall the whole pipeline; z(w) also
            # waits on every e2dup write emitted before it (tile-granular
            # dependency tracking), so later windows' builds are staggered
            # through iteration 0.
            build_e2dup_win(0)

            hT_cur = h0T
            y_sb = spool.tile([WIN, WPC], F32, tag="y")

            for it in range(DEP):
                gsrcA = h0_d if it == 0 else h_fullA[it - 1]
                gsrcB = h0_d if it == 0 else h_full[it - 1]
                # Two gathers per window into its own tiles: the MAIN gather
                # (tiles 0..nt-2, whose edges by construction have sources in
                # windows 0..wpc-2) runs as soon as those h windows land and
                # overlaps the last window's compute; only the small FIX
                # gather (last tile) waits for the final h window.
                h_srcs = [None] * WPC
                h_fix = [None] * WPC

                def issue_gather(w):
                    nt = NT_W[w]
                    o = TOFF[w] * 128
                    nm = (nt - 1) * 128
                    ix = idx0 if (w == 0 and it == 0) else idx
                    hs_w = hsrc_pool.tile([128, nt - 1, 2 * W], F16,
                                          tag=f"h{w}")
                    nc.gpsimd.dma_gather(
                        hs_w[:], gsrcA[:],
                        ix[:, o // 16:(o + nm) // 16], nm, nm, 2 * W)
                    h_srcs[w] = hs_w
                    hf_w = hsrc_pool.tile([128, 1, 2 * W], F16, tag=f"hf{w}")
                    nc.gpsimd.dma_gather(
                        hf_w[:], gsrcB[:],
                        ix[:, (o + nm) // 16:(o + nm + 128) // 16], 128, 128,
                        2 * W)
                    h_fix[w] = hf_w

                for _w0 in range(4):
                    issue_gather(_w0)
                hT_next = hT_pool.tile([W, WPC * WIN], F16)

                def write_h(w):
                    # transpose hT_next[w] and write it to DRAM. Deferred by
                    # one window (emitted after the NEXT window's zsum
                    # passes) so the in-order PE never stalls waiting for
                    # relu(w) on ACT; the last window is emitted inline since
                    # its write gates the next iteration's fix gathers.
                    h_ps = tr_ps_pool.tile([WIN, 64], F16, tag="tr")
                    nc.tensor.transpose(h_ps[:],
                                        hT_next[:, w * WIN:(w + 1) * WIN],
                                        ident[:])
                    h_sb = spool.tile([WIN, 64], F16, tag="hnew")
                    nc.scalar.copy(h_sb[:], h_ps[:])
                    if single_core:
                        if w < WPC - 1:
                            nc.sync.dma_start(
                                h_fullA[it][w * WIN:(w + 1) * WIN, :W],
                                h_sb[:])
                            if w == WPC - 2:
                                # B gets windows 0..wpc-2 via one bulk
                                # copy (fix gathers wait for the last
                                # window anyway, so this is off the
                                # critical path)
                                nc.sync.dma_start(
                                    h_full[it][: (WPC - 1) * WIN, :],
                                    h_fullA[it][: (WPC - 1) * WIN, :])
                        else:
                            nc.sync.dma_start(
                                h_full[it][w * WIN:(w + 1) * WIN, :W],
                                h_sb[:])
                    else:
                        nc.sync.dma_start(
                            h_slice[it][w * WIN:(w + 1) * WIN, :W], h_sb[:])

                pending_tr = []
                for w in range(WPC):
                    nt = NT_W[w]
                    t0 = TOFF[w]
                    zs = []
                    for et in range(nt):
                        t = t0 + et
                        z = zpool.tile([128, NCH * 128], F16)
                        # all-fp16 operands with packed (1,2) last dims hit
                        # the DVE 2x perf mode (stride-0 last dims do not)
                        zv = z[:].rearrange("p (c a b) -> p c a b", c=66, b=2)
                        h_t = h_srcs[w] if et < nt - 1 else h_fix[w]
                        e_t = et if et < nt - 1 else 0
                        hs = h_t[:, e_t, :W] \
                            .rearrange("p (a b) -> p a b", b=2) \
                            .unsqueeze(1).broadcast_to((128, 66, 32, 2))
                        e2 = e2dup[:, t * 132:(t + 1) * 132] \
                            .rearrange("p (c b) -> p c b", b=2) \
                            .unsqueeze(2).broadcast_to((128, 66, 32, 2))
                        if et == nt - 1 and w == 0:
                            # window 0's fix tile sits on the iteration
                            # boundary critical path: build it in KH-chunk
                            # pieces so pass 0 can start after the first one
                            for c0 in range(0, 66, 8):
                                c1 = min(c0 + 8, 66)
                                nc.vector.tensor_mul(
                                    zv[:, c0:c1, :, :], hs[:, c0:c1, :, :],
                                    e2[:, c0:c1, :, :])
                            zs.append(z)
                            continue
                        # offload part of the first tile of each window to
                        # the (idle) GPSIMD engine; DVE builds the rest.
                        # Not in iteration 0's first windows: Pool is still
                        # busy with the initial gather burst there.
                        if et == 0 and not (it == 0 and w < 3):
                            nc.gpsimd.tensor_mul(
                                zv[:, :24, :, :], hs[:, :24, :, :],
                                e2[:, :24, :, :])
                            nc.vector.tensor_mul(
                                zv[:, 24:, :, :], hs[:, 24:, :, :],
                                e2[:, 24:, :, :])
                        else:
                            nc.vector.tensor_mul(zv, hs, e2)
                        zs.append(z)
                    if w + 4 < WPC:
                        issue_gather(w + 4)
                    if it == 0:
                        if w == 0 and WPC > 1:
                            build_e2dup_win(1)
                        if w + 2 < WPC:
                            build_e2dup_win(w + 2)
                    # zsum in KH-chunk PSUM passes (back-to-back on PE; the
                    # drains pipeline on ACT), then the T-contract block.
                    zsum_sb = zsum_sb_pool.tile([128, NCH * WIN], F16)
                    for p0 in range(0, NCH, KH):
                        p1 = min(p0 + KH, NCH)
                        zsum_ps = zsum_ps_pool.tile([128, KH * WIN], F32)
                        for k in range(p0, p1):
                            for et in range(nt):
                                nc.tensor.matmul(
                                    zsum_ps[:, (k - p0) * WIN:(k - p0 + 1) * WIN],
                                    zs[et][:, k * 128:(k + 1) * 128],
                                    segT[:, (t0 + et) * WIN:(t0 + et + 1) * WIN],
                                    start=(et == 0), stop=(et == nt - 1))
                        # keep the DVE free for z-builds (critical engine) —
                        # drain PSUM on ACT
                        nc.scalar.copy(zsum_sb[:, p0 * WIN:p1 * WIN],
                                       zsum_ps[:, :(p1 - p0) * WIN])
                    if pending_tr:
                        write_h(pending_tr.pop())
                    agg_ps = agg_ps_pool.tile([64, WIN], F32, tag="a")
                    for k in range(NCH):
                        nc.tensor.matmul(agg_ps[:],
                                         Tsb[:, k * W:(k + 1) * W],
                                         zsum_sb[:, k * WIN:(k + 1) * WIN],
                                         start=(k == 0), stop=False)
                    nc.tensor.matmul(agg_ps[:], rootW[:],
                                     hT_cur[:, w * WIN:(w + 1) * WIN],
                                     start=False, stop=True)
                    nc.scalar.activation(hT_next[:, w * WIN:(w + 1) * WIN],
                                         agg_ps[:], Relu)
                    if it == DEP - 1:
                        y_ps = agg_ps_pool.tile([WIN, 1], F32, tag="a")
                        nc.tensor.matmul(y_ps[:],
                                         hT_next[:, w * WIN:(w + 1) * WIN],
                                         fc2[:], start=True, stop=True)
                        nc.vector.tensor_add(y_sb[:, w: w + 1], y_ps[:], fc2b[:])
                    else:
                        if w >= WPC - 2:
                            # w8's write gates the next iteration's main
                            # gathers, w9's its fix gathers: keep both inline
                            if pending_tr:
                                write_h(pending_tr.pop())
                            write_h(w)
                        else:
                            pending_tr.append(w)
                hT_cur = hT_next
                if it < DEP - 1 and not single_core:
                    nc.gpsimd.collective_compute(
                        "AllGather",
                        mybir.AluOpType.bypass,
                        ins=[h_slice[it][:].opt()],
                        outs=[h_full[it][:].opt()],
                        replica_groups=[list(range(NC_))],
                    )

            # ---- output ----
            y_view = y_d[:].rearrange("(w v) o -> v (w o)", w=WPC)
            nc.sync.dma_start(y_view, y_sb[:])

    nc.compile()
    return nc


def kernel(**inputs) -> np.ndarray:
    from concourse.bass_utils import run_bass_kernel_spmd

    plan = make_plan(**{k: np.asarray(v) for k, v in inputs.items()})
    nc = build_program(plan)
    core_ids = list(range(plan.n_cores))
    res = run_bass_kernel_spmd(nc, plan.in_maps, core_ids,
                               trace=bool(int(os.environ.get("KERNEL_TRACE", "0"))))
    y = np.concatenate([res.results[r]["y"] for r in range(plan.n_cores)], axis=0)
    out = y[plan.devnode]
    kernel.last_results = res
    kernel.last_plan = plan
    return out


# revision 32
# speedup vs baseline: 2.0561x; 1.0185x over previous
"""Trainium2 Bass kernel for nn_Net_MP_68805376082308 (NNConv-style GNN).

Reference computation (see problem statement):
    h = x@fc1 + b
    e2 = relu(edge_attr@k1 + b1)                     # [E, 64]
    ew = (e2 @ k2 + b2).reshape(E, 64, 64)           # never materialized here!
    for 4 iters:
        msg  = einsum('ei,eio->eo', h[src], ew)
        agg  = segment_sum(msg, dst) / max(deg,1)
        h    = relu(agg + h@root)
    out = h @ fc2 + b

Device algorithm (per core, node-sharded, dst-grouped edge slots):
    e2s[e, c]    = relu((edge_attr@k1+b1)[e,c]) * invdeg[dst[e]]  (c in 0..63)
    z[e, c*64+i] = e2s[e,c] * h[src[e], i]        # DVE, fp16 pair-trick APs
    zsumT[ci, v] = sum_e z[e,ci] * SegMat[e,v]    # PE, z stationary (scatter
                                                  #  commutes with k2 contract)
    aggT[o, v]   = T_cm.T @ zsumT + root.T @ hT   # PE (T_cm rows 4096..4159
                                                  #  carry k2_b, paired with
                                                  #  z's invdeg column)
    hT           = relu(aggT)                     # ACT
    h[src] gather via SWDGE dma_gather; h exchanged across 8 cores with an
    AllGather after each iteration.

Windows hold 128 dst-node slots and 5 edge tiles each. Edges whose SOURCE
node lies in any core's last window are segregated into each window's last
("fix") tile: the other tiles' gathers then depend only on h windows 0..8 and
overlap the last window's compute at the iteration boundary (split h_fullA/B
tensors express this to the dependency tracker in the single-core cost model;
the real 8-core path keeps one h_full fed by an AllGather).

kernel(**inputs) takes the FULL unsharded inputs and returns [10000, 1] fp32.
"""

import math
import os
import sys
from dataclasses import dataclass, field

import numpy as np

sys.path.insert(0, "/opt/trn_rl_repo")

import concourse.bacc as bacc
import concourse.bass as bass
import concourse.mybir as mybir
import concourse.tile as tile
from concourse import library_config

F32 = mybir.dt.float32
F16 = mybir.dt.float16
I16 = mybir.dt.int16

WIDTH = 64
DEPTH = 4
NCH = 33                # ci chunks of 128 (66*64/128)
KH = 8                  # chunks per PSUM pass


@dataclass
class Plan:
    """Host-side preprocessing result: all per-core device input arrays plus
    the compile-time structure constants."""

    n_cores: int
    wpc: int                 # windows per core
    nt_w: list = None        # tiles per window (same layout for every core)
    nodes_pad: int = 0
    depth: int = DEPTH
    win: int = 128           # nodes per scatter window
    devnode: np.ndarray = None     # [N] original node -> device row
    in_maps: list = field(default_factory=list)
    fc2_b: float = 0.0

    @property
    def ntiles(self):        # edge tiles per core
        return sum(self.nt_w)

    @property
    def epc(self):           # edge slots per core
        return self.ntiles * 128

    @property
    def tile_off(self):      # first tile index of each window
        off, out = 0, []
        for n in self.nt_w:
            out.append(off)
            off += n
        return out


def make_plan(x, edge_index, edge_attr, fc1_W, fc1_b, k1_W, k1_b, k2_W, k2_b,
              root, conv_b, fc2_W, fc2_b, n_cores=8, depth=DEPTH):
    W = WIDTH
    N = x.shape[0]
    E = edge_index.shape[1]
    src = np.asarray(edge_index[0], dtype=np.int64)
    dst = np.asarray(edge_index[1], dtype=np.int64)
    assert np.all(np.asarray(conv_b) == 0.0), "kernel assumes conv_b == 0"

    WIN = 128
    wpc = max(1, int(math.ceil(N / WIN / n_cores)))
    n_windows = n_cores * wpc
    nodes_pad = n_windows * WIN

    counts = np.bincount(dst, minlength=N).astype(np.float64)
    denom = np.where(counts > 0, counts, 1.0)
    invdeg_node = (1.0 / denom).astype(np.float32)

    # Per-window edge-tile capacities: 49 tiles (6272 slots) is the minimum
    # that fits 6250 edges; the 4-tile window sits second-to-last (measured
    # best position).
    base_nt = [5, 5, 5, 5, 5, 5, 5, 5, 4, 5]
    if wpc != 10:  # generic fallback: balanced with one tiny last window
        per = int(math.ceil(E / n_cores / max(1, wpc - 1) / 128)) + 1
        base_nt = [per] * (wpc - 1) + [1]
    cap = np.array([nt * 128 for _ in range(n_cores) for nt in base_nt],
                   dtype=np.int64)

    order = np.argsort(-counts, kind="stable")
    win_edges = np.zeros(n_windows, dtype=np.int64)
    win_fill = np.zeros(n_windows, dtype=np.int64)
    node_window = np.zeros(N, dtype=np.int64)
    node_slot = np.zeros(N, dtype=np.int64)
    NEG = -(1 << 60)
    # greedy: place desc-degree nodes into the window with the most remaining
    # edge capacity that still has node slots; grow a window's capacity by a
    # tile if nothing fits.
    rem = cap.copy()
    for n in order:
        d = int(counts[n])
        w = int(np.argmax(rem))
        if rem[w] < d:
            cap[w] += 128 * int(math.ceil((d - rem[w]) / 128))
            rem[w] = cap[w] - win_edges[w]
        node_window[n] = w
        node_slot[n] = win_fill[w]
        win_fill[w] += 1
        win_edges[w] += d
        rem[w] = cap[w] - win_edges[w] if win_fill[w] < WIN else NEG
    nt_all = (cap // 128).reshape(n_cores, wpc)
    # every core runs one compiled program -> shared nt layout: per-position max
    nt_w = [int(nt_all[:, i].max()) for i in range(wpc)]

    plan = Plan(n_cores=n_cores, wpc=wpc, nt_w=nt_w, nodes_pad=nodes_pad,
                depth=depth, win=WIN,
                fc2_b=float(np.asarray(fc2_b).reshape(())))
    ntiles = plan.ntiles
    epc = plan.epc
    woff = [128 * t for t in plan.tile_off]   # slot offset of window in core

    plan.devnode = node_window * WIN + node_slot

    # edge -> slot within its dst window. Edges whose SOURCE lies in any
    # core's last window ("fix" edges) go to the tail of the window's slot
    # range (the last tile): the other tiles' gather then depends only on
    # h windows 0..wpc-2 and overlaps the last window's compute.
    devnode = node_window * WIN + node_slot
    edge_win = node_window[dst]
    is_fix = (devnode[src] % (wpc * WIN)) >= (wpc - 1) * WIN
    ord_e = np.argsort(edge_win, kind="stable")
    fill = np.zeros(n_windows, dtype=np.int64)
    fillb = np.zeros(n_windows, dtype=np.int64)
    eslot = np.zeros(E, dtype=np.int64)
    for e in ord_e:
        w = edge_win[e]
        core, wl = divmod(w, wpc)
        capw = nt_w[wl] * 128
        if is_fix[e]:
            fillb[w] += 1
            eslot[e] = core * epc + woff[wl] + capw - fillb[w]
        else:
            eslot[e] = core * epc + woff[wl] + fill[w]
            fill[w] += 1
    assert all(fill[w] + fillb[w] <= nt_w[w % wpc] * 128
               for w in range(n_windows))
    assert fillb.max() <= 128, "fix edges must fit the last tile"

    tot_slots = n_cores * epc
    slot_src = np.zeros(tot_slots, dtype=np.int64)
    slot_used = np.zeros(tot_slots, dtype=bool)
    slot_vloc = np.zeros(tot_slots, dtype=np.int64)
    slot_invdeg = np.zeros(tot_slots, dtype=np.float32)
    slot_ea = np.zeros((tot_slots, 3), dtype=np.float32)
    slot_src[eslot] = devnode[src]
    del devnode
    slot_used[eslot] = True
    slot_vloc[eslot] = node_slot[dst]
    slot_invdeg[eslot] = invdeg_node[dst]
    slot_ea[eslot] = np.asarray(edge_attr, dtype=np.float32)

    # weight repacks: T_cm [66*64, 64]: rows 0..4095 = k2_W, rows
    # 4096..4159 = k2_b (paired with z's invdeg column), rest zero.
    # chunk layout: T_sb[p, k*64+o] = T_cm[k*128+p, o]
    T_cm = np.zeros((66 * 64, W), dtype=np.float32)
    T_cm[: 64 * 64] = np.ascontiguousarray(
        np.asarray(k2_W, dtype=np.float32).reshape(64, 64, 64)
    ).reshape(64 * 64, W)
    T_cm[64 * 64 : 65 * 64] = np.asarray(k2_b, dtype=np.float32).reshape(64, 64)
    T_sb = np.ascontiguousarray(
        T_cm.reshape(NCH, 128, W).transpose(1, 0, 2)
    ).reshape(128, NCH * W).astype(np.float16)

    # k1 extended to 66 cols: 0-63 = [k1_W; k1_b], 64 = bias-row one (the
    # invdeg ACT-scale turns it into the invdeg column), 65 = zero pad.
    k1_Wb = np.zeros((4, 66), dtype=np.float16)
    k1_Wb[:3, :64] = np.asarray(k1_W, dtype=np.float16)
    k1_Wb[3, :64] = np.asarray(k1_b, dtype=np.float16)
    k1_Wb[3, 64] = 1.0

    # h rows padded to 128 f16 (=256B) so SWDGE dma_gather's 256B-multiple
    # row-stride restriction is met; cols 64.. are never read by compute.
    h0 = np.zeros((nodes_pad, 2 * W), dtype=np.float16)
    h0[plan.devnode, :W] = (np.asarray(x, np.float32) @ np.asarray(fc1_W, np.float32)
                       + np.asarray(fc1_b, np.float32)).astype(np.float16)

    ident = np.eye(64, dtype=np.float16)
    root_np = np.asarray(root, dtype=np.float16)
    fc2_np = np.asarray(fc2_W, dtype=np.float16).reshape(W, 1)

    for r in range(n_cores):
        sl = slice(r * epc, (r + 1) * epc)
        c_ea = slot_ea[sl]
        c_used = slot_used[sl]
        c_invd = slot_invdeg[sl]
        c_vloc = slot_vloc[sl]
        c_src = slot_src[sl]

        eaT = np.zeros((4, epc), dtype=np.float16)
        eaT[:3] = c_ea.T.astype(np.float16)
        eaT[3] = 1.0
        # invdeg in [partition, tile] layout (slot s -> (s//128, s%128))
        invd = np.ascontiguousarray(
            c_invd.reshape(ntiles, 128).T)                       # [128, ntiles]
        tt = np.arange(epc) // 128
        pp = np.arange(epc) % 128
        segT = np.zeros((ntiles, 128, WIN), dtype=np.float16)
        segT[tt[c_used], pp[c_used], c_vloc[c_used]] = 1.0
        segT = np.ascontiguousarray(segT.transpose(1, 0, 2)).reshape(128, ntiles * WIN)

        idx = np.zeros((128, epc // 16), dtype=np.int16)
        base = c_src.astype(np.int16).reshape(epc // 16, 16).T   # [16, epc/16]
        for g in range(8):
            idx[16 * g : 16 * (g + 1)] = base

        h0T = np.ascontiguousarray(
            h0[r * wpc * WIN : (r + 1) * wpc * WIN, :W].T)       # [64, wpc*WIN]


        plan.in_maps.append({
            "eaT": eaT,
            "invdeg": invd,
            "segmatT": segT,
            "idx": idx,
            "h0": h0,
            "h0T": h0T,
            "T_sb": T_sb,
            "k1_Wb": k1_Wb,
            "root": root_np,
            "fc2_W": fc2_np,
            "fc2_b": np.full((WIN, 1), plan.fc2_b, dtype=np.float32),
            "ident": ident,
        })
    return plan


def build_program(plan: Plan, debug=False, single_core=False):
    """Build the SPMD Bass program (one program, run on all cores).

    single_core=True replaces the AllGather with direct local h_full writes
    (and drops addr_space="Shared") so the program can run under TimelineSim
    for cost modeling."""
    W = WIDTH
    WPC = plan.wpc
    WIN = plan.win
    NT_W = plan.nt_w
    TOFF = plan.tile_off
    NTILES = plan.ntiles
    EPC = plan.epc
    NPAD = plan.nodes_pad
    DEP = plan.depth
    NC_ = plan.n_cores
    Relu = mybir.ActivationFunctionType.Relu

    nc = bacc.Bacc("TRN2", target_bir_lowering=False, debug=debug,
                   num_devices=NC_)

    # ---- I/O ----
    eaT_d = nc.dram_tensor("eaT", [4, EPC], F16, kind="ExternalInput")
    invd_d = nc.dram_tensor("invdeg", [128, NTILES], F32, kind="ExternalInput")
    segT_d = nc.dram_tensor("segmatT", [128, NTILES * WIN], F16, kind="ExternalInput")
    idx_d = nc.dram_tensor("idx", [128, EPC // 16], I16, kind="ExternalInput")
    h0_d = nc.dram_tensor("h0", [NPAD, 2 * W], F16, kind="ExternalInput")
    h0T_d = nc.dram_tensor("h0T", [W, WPC * WIN], F16, kind="ExternalInput")
    Tsb_d = nc.dram_tensor("T_sb", [128, NCH * W], F16, kind="ExternalInput")
    k1_d = nc.dram_tensor("k1_Wb", [4, 66], F16, kind="ExternalInput")
    root_d = nc.dram_tensor("root", [W, W], F16, kind="ExternalInput")
    fc2_d = nc.dram_tensor("fc2_W", [W, 1], F16, kind="ExternalInput")
    fc2b_d = nc.dram_tensor("fc2_b", [WIN, 1], F32, kind="ExternalInput")
    id_d = nc.dram_tensor("ident", [64, 64], F16, kind="ExternalInput")
    y_d = nc.dram_tensor("y", [WPC * WIN, 1], F32, kind="ExternalOutput")

    # internal DRAM for the h exchange
    h_slice = [nc.dram_tensor(f"h_slice{i}", [WPC * WIN, 2 * W], F16)
               for i in range(DEP - 1)]
    if single_core:
        h_fullA = [nc.dram_tensor(f"h_fullA{i}", [NPAD, 2 * W], F16)
                   for i in range(DEP - 1)]
        h_full = [nc.dram_tensor(f"h_fullB{i}", [NPAD, 2 * W], F16)
                  for i in range(DEP - 1)]
    else:
        h_full = [nc.dram_tensor(f"h_full{i}", [NPAD, 2 * W], F16,
                                 addr_space="Shared")
                  for i in range(DEP - 1)]
        h_fullA = h_full

    MAXNT = max(NT_W)
    with tile.TileContext(nc) as tc:
        with (
            tc.tile_pool(name="const", bufs=1) as cpool,
            tc.tile_pool(name="hsrc", bufs=2) as hsrc_pool,
            tc.tile_pool(name="z", bufs=2 * MAXNT + 1) as zpool,
            tc.tile_pool(name="zsum_sb", bufs=2) as zsum_sb_pool,
            tc.tile_pool(name="hT", bufs=2) as hT_pool,
            tc.tile_pool(name="small", bufs=4) as spool,
            tc.tile_pool(name="zsum_ps", bufs=2, space="PSUM") as zsum_ps_pool,
            tc.tile_pool(name="agg_ps", bufs=2, space="PSUM") as agg_ps_pool,
            tc.tile_pool(name="tr_ps", bufs=1, space="PSUM") as tr_ps_pool,
        ):
            nc.gpsimd.load_library(library_config.mlp)

            # preload the ACT function table (1.3us) under the const DMAs;
            # Copy needs no bias const-AP (whose DMA would land late)
            warm = cpool.tile([1, 1], F32)
            nc.vector.memset(warm[:], 0.0)
            nc.scalar.activation(warm[:], warm[:],
                                 mybir.ActivationFunctionType.Copy)

            # ---- constants; gather-critical tensors first so window 0's
            # gather + e2 chain + first zsum start ASAP ----
            n0 = NT_W[0] * 128 // 16
            idx0 = cpool.tile([128, n0], I16)
            nc.sync.dma_start(idx0[:], idx_d[:, :n0])
            idx = cpool.tile([128, EPC // 16], I16)
            nc.sync.dma_start(idx[:], idx_d[:])
            eaT = cpool.tile([4, EPC], F16)
            nc.sync.dma_start(eaT[:], eaT_d[:])
            invd = cpool.tile([128, NTILES], F32)
            nc.sync.dma_start(invd[:], invd_d[:])
            k1 = cpool.tile([4, 66], F16)
            nc.sync.dma_start(k1[:], k1_d[:])
            segT = cpool.tile([128, NTILES * WIN], F16)
            nc.sync.dma_start(segT[:], segT_d[:])
            Tsb = cpool.tile([128, NCH * W], F16)
            nc.sync.dma_start(Tsb[:], Tsb_d[:])
            h0T = cpool.tile([W, WPC * WIN], F16)
            nc.sync.dma_start(h0T[:], h0T_d[:])
            rootW = cpool.tile([W, W], F16)
            nc.sync.dma_start(rootW[:], root_d[:])
            fc2 = cpool.tile([W, 1], F16)
            nc.sync.dma_start(fc2[:], fc2_d[:])
            fc2b = cpool.tile([WIN, 1], F32)
            nc.sync.dma_start(fc2b[:], fc2b_d[:])
            ident = cpool.tile([64, 64], F16)
            nc.sync.dma_start(ident[:], id_d[:])

            # ---- e2dup: [128, NTILES*64*2] fp16, every value twice so the
            # z-build APs end in a packed (stride 1, count 2) dim on ALL
            # operands -> DVE 2x mode. relu(x*invdeg) = invdeg*relu(x) folds
            # the scatter-mean denominator into the ACT scale. ----
            e2dup = cpool.tile([128, NTILES * 66 * 2], F16)

            def build_e2dup(t):
                e2_ps = agg_ps_pool.tile([128, 66], F32, tag="a")
                nc.tensor.matmul(e2_ps[:], eaT[:, t * 128:(t + 1) * 128],
                                 k1[:], start=True, stop=True)
                dup = e2dup[:, t * 132:(t + 1) * 132] \
                    .rearrange("p (c b) -> p c b", b=2)
                for b in range(2):
                    nc.scalar.activation(dup[:, :, b], e2_ps[:], Relu,
                                         scale=invd[:, t: t + 1])

            def build_e2dup_win(w):
                for et in range(NT_W[w]):
                    build_e2dup(TOFF[w] + et)

            # only window 0's e2dup up front: emitting all of it here would
            # queue 24us of ACT work ahead of iteration 0's PSUM drains (ACT
            # executes in order) and stall the whole pipeline; z(w) also
            # waits on every e2dup write emitted before it (tile-granular
            # dependency tracking), so later windows' builds are staggered
            # through iteration 0.
            build_e2dup_win(0)

            hT_cur = h0T
            y_sb = spool.tile([WIN, WPC], F32, tag="y")

            for it in range(DEP):
                gsrcA = h0_d if it == 0 else h_fullA[it - 1]
                gsrcB = h0_d if it == 0 else h_full[it - 1]
                # Two gathers per window into its own tiles: the MAIN gather
                # (tiles 0..nt-2, whose edges by construction have sources in
                # windows 0..wpc-2) runs as soon as those h windows land and
                # overlaps the last window's compute; only the small FIX
                # gather (last tile) waits for the final h window.
                h_srcs = [None] * WPC
                h_fix = [None] * WPC

                def issue_gather(w):
                    nt = NT_W[w]
                    o = TOFF[w] * 128
                    nm = (nt - 1) * 128
                    ix = idx0 if (w == 0 and it == 0) else idx
                    hs_w = hsrc_pool.tile([128, nt - 1, 2 * W], F16,
                                          tag=f"h{w}")
                    nc.gpsimd.dma_gather(
                        hs_w[:], gsrcA[:],
                        ix[:, o // 16:(o + nm) // 16], nm, nm, 2 * W)
                    h_srcs[w] = hs_w
                    hf_w = hsrc_pool.tile([128, 1, 2 * W], F16, tag=f"hf{w}")
                    nc.gpsimd.dma_gather(
                        hf_w[:], gsrcB[:],
                        ix[:, (o + nm) // 16:(o + nm + 128) // 16], 128, 128,
                        2 * W)
                    h_fix[w] = hf_w

                for _w0 in range(4):
                    issue_gather(_w0)
                hT_next = hT_pool.tile([W, WPC * WIN], F16)

                def write_h(w):
                    # transpose hT_next[w] and write it to DRAM. Deferred by
                    # one window (emitted after the NEXT window's zsum
                    # passes) so the in-order PE never stalls waiting for
                    # relu(w) on ACT; the last two windows are emitted inline
                    # since their writes gate the next iteration's gathers.
                    h_ps = tr_ps_pool.tile([WIN, 64], F16, tag="tr")
                    nc.tensor.transpose(h_ps[:],
                                        hT_next[:, w * WIN:(w + 1) * WIN],
                                        ident[:])
                    h_sb = spool.tile([WIN, 64], F16, tag="hnew")
                    nc.scalar.copy(h_sb[:], h_ps[:])
                    if single_core:
                        if w < WPC - 1:
                            nc.sync.dma_start(
                                h_fullA[it][w * WIN:(w + 1) * WIN, :W],
                                h_sb[:])
                            if w == WPC - 2:
                                # B gets windows 0..wpc-2 via one bulk
                                # copy (fix gathers wait for the last
                                # window anyway, so this is off the
                                # critical path)
                                nc.sync.dma_start(
                                    h_full[it][: (WPC - 1) * WIN, :],
                                    h_fullA[it][: (WPC - 1) * WIN, :])
                        else:
                            nc.sync.dma_start(
                                h_full[it][w * WIN:(w + 1) * WIN, :W],
                                h_sb[:])
                    else:
                        nc.sync.dma_start(
                            h_slice[it][w * WIN:(w + 1) * WIN, :W], h_sb[:])

                def emit_y(w):
                    y_ps = agg_ps_pool.tile([WIN, 1], F32, tag="a")
                    nc.tensor.matmul(y_ps[:],
                                     hT_next[:, w * WIN:(w + 1) * WIN],
                                     fc2[:], start=True, stop=True)
                    nc.vector.tensor_add(y_sb[:, w: w + 1], y_ps[:], fc2b[:])

                pending_tr = []
                pending_y = []
                for w in range(WPC):
                    nt = NT_W[w]
                    t0 = TOFF[w]
                    zs = []
                    for et in range(nt):
                        t = t0 + et
                        z = zpool.tile([128, NCH * 128], F16)
                        # all-fp16 operands with packed (1,2) last dims hit
                        # the DVE 2x perf mode (stride-0 last dims do not)
                        zv = z[:].rearrange("p (c a b) -> p c a b", c=66, b=2)
                        h_t = h_srcs[w] if et < nt - 1 else h_fix[w]
                        e_t = et if et < nt - 1 else 0
                        hs = h_t[:, e_t, :W] \
                            .rearrange("p (a b) -> p a b", b=2) \
                            .unsqueeze(1).broadcast_to((128, 66, 32, 2))
                        e2 = e2dup[:, t * 132:(t + 1) * 132] \
                            .rearrange("p (c b) -> p c b", b=2) \
                            .unsqueeze(2).broadcast_to((128, 66, 32, 2))
                        if et == nt - 1 and w == 0:
                            # window 0's fix tile sits on the iteration
                            # boundary critical path: build it in KH-chunk
                            # pieces so pass 0 can start after the first one
                            for c0 in range(0, 66, 8):
                                c1 = min(c0 + 8, 66)
                                nc.vector.tensor_mul(
                                    zv[:, c0:c1, :, :], hs[:, c0:c1, :, :],
                                    e2[:, c0:c1, :, :])
                            zs.append(z)
                            continue
                        # offload part of the first tile of each window to
                        # the (idle) GPSIMD engine; DVE builds the rest.
                        # Not in iteration 0's first windows: Pool is still
                        # busy with the initial gather burst there.
                        if et == 0 and not (it == 0 and w < 3):
                            nc.gpsimd.tensor_mul(
                                zv[:, :24, :, :], hs[:, :24, :, :],
                                e2[:, :24, :, :])
                            nc.vector.tensor_mul(
                                zv[:, 24:, :, :], hs[:, 24:, :, :],
                                e2[:, 24:, :, :])
                        else:
                            nc.vector.tensor_mul(zv, hs, e2)
                        zs.append(z)
                    if w + 4 < WPC:
                        issue_gather(w + 4)
                    if it == 0:
                        if w == 0 and WPC > 1:
                            build_e2dup_win(1)
                        if w + 2 < WPC:
                            build_e2dup_win(w + 2)
                    # zsum in KH-chunk PSUM passes (back-to-back on PE; the
                    # drains pipeline on ACT), then the T-contract block.
                    zsum_sb = zsum_sb_pool.tile([128, NCH * WIN], F16)
                    for p0 in range(0, NCH, KH):
                        p1 = min(p0 + KH, NCH)
                        zsum_ps = zsum_ps_pool.tile([128, KH * WIN], F32)
                        for k in range(p0, p1):
                            for et in range(nt):
                                nc.tensor.matmul(
                                    zsum_ps[:, (k - p0) * WIN:(k - p0 + 1) * WIN],
                                    zs[et][:, k * 128:(k + 1) * 128],
                                    segT[:, (t0 + et) * WIN:(t0 + et + 1) * WIN],
                                    start=(et == 0), stop=(et == nt - 1))
                        # keep the DVE free for z-builds (critical engine) —
                        # drain PSUM on ACT
                        nc.scalar.copy(zsum_sb[:, p0 * WIN:p1 * WIN],
                                       zsum_ps[:, :(p1 - p0) * WIN])
                    if pending_tr:
                        write_h(pending_tr.pop())
                    while len(pending_y) > 1:
                        emit_y(pending_y.pop(0))
                    agg_ps = agg_ps_pool.tile([64, WIN], F32, tag="a")
                    for k in range(NCH):
                        nc.tensor.matmul(agg_ps[:],
                                         Tsb[:, k * W:(k + 1) * W],
                                         zsum_sb[:, k * WIN:(k + 1) * WIN],
                                         start=(k == 0), stop=False)
                    nc.tensor.matmul(agg_ps[:], rootW[:],
                                     hT_cur[:, w * WIN:(w + 1) * WIN],
                                     start=False, stop=True)
                    nc.scalar.activation(hT_next[:, w * WIN:(w + 1) * WIN],
                                         agg_ps[:], Relu)
                    if it == DEP - 1:
                        pending_y.append(w)
                        if w == WPC - 1:
                            while pending_y:
                                emit_y(pending_y.pop(0))
                    else:
                        if w >= WPC - 2:
                            # w8's write gates the next iteration's main
                            # gathers, w9's its fix gathers: keep both inline
                            if pending_tr:
                                write_h(pending_tr.pop())
                            write_h(w)
                        else:
                            pending_tr.append(w)
                hT_cur = hT_next
                if it < DEP - 1 and not single_core:
                    nc.gpsimd.collective_compute(
                        "AllGather",
                        mybir.AluOpType.bypass,
                        ins=[h_slice[it][:].opt()],
                        outs=[h_full[it][:].opt()],
                        replica_groups=[list(range(NC_))],
                    )

            # ---- output ----
            y_view = y_d[:].rearrange("(w v) o -> v (w o)", w=WPC)
            nc.sync.dma_start(y_view, y_sb[:])

    nc.compile()
    return nc


def kernel(**inputs) -> np.ndarray:
    from concourse.bass_utils import run_bass_kernel_spmd

    plan = make_plan(**{k: np.asarray(v) for k, v in inputs.items()})
    nc = build_program(plan)
    core_ids = list(range(plan.n_cores))
    res = run_bass_kernel_spmd(nc, plan.in_maps, core_ids,
                               trace=bool(int(os.environ.get("KERNEL_TRACE", "0"))))
    y = np.concatenate([res.results[r]["y"] for r in range(plan.n_cores)], axis=0)
    out = y[plan.devnode]
    kernel.last_results = res
    kernel.last_plan = plan
    return out


# revision 36
# speedup vs baseline: 2.1553x; 1.0482x over previous
"""Trainium2 Bass kernel for nn_Net_MP_68805376082308 (NNConv-style GNN).

Reference computation (see problem statement):
    h = x@fc1 + b
    e2 = relu(edge_attr@k1 + b1)                     # [E, 64]
    ew = (e2 @ k2 + b2).reshape(E, 64, 64)           # never materialized here!
    for 4 iters:
        msg  = einsum('ei,eio->eo', h[src], ew)
        agg  = segment_sum(msg, dst) / max(deg,1)
        h    = relu(agg + h@root)
    out = h @ fc2 + b

Device algorithm (per core, node-sharded, dst-grouped edge slots):
    e2s[e, c]    = relu((edge_attr@k1+b1)[e,c]) * invdeg[dst[e]]  (c in 0..63)
    z[e, c*64+i] = e2s[e,c] * h[src[e], i]        # DVE, fp16 pair-trick APs
    zsumT[ci, v] = sum_e z[e,ci] * SegMat[e,v]    # PE, z stationary (scatter
                                                  #  commutes with k2 contract)
    aggT[o, v]   = T_cm.T @ zsumT + root.T @ hT   # PE (T_cm rows 4096..4159
                                                  #  carry k2_b, paired with
                                                  #  z's invdeg column)
    hT           = relu(aggT)                     # ACT
    h[src] gather via SWDGE dma_gather; h exchanged across 8 cores with an
    AllGather after each iteration.

Windows hold 128 dst-node slots and 5 edge tiles each. Edges whose SOURCE
node lies in any core's last window are segregated into each window's last
("fix") tile: the other tiles' gathers then depend only on h windows 0..8 and
overlap the last window's compute at the iteration boundary (split h_fullA/B
tensors express this to the dependency tracker in the single-core cost model;
the real 8-core path keeps one h_full fed by an AllGather).

kernel(**inputs) takes the FULL unsharded inputs and returns [10000, 1] fp32.
"""

import math
import os
import sys
from dataclasses import dataclass, field

import numpy as np

sys.path.insert(0, "/opt/trn_rl_repo")

import concourse.bacc as bacc
import concourse.bass as bass
import concourse.mybir as mybir
import concourse.tile as tile
from concourse import library_config

F32 = mybir.dt.float32
F16 = mybir.dt.float16
I16 = mybir.dt.int16

WIDTH = 64
DEPTH = 4
NCH = 33                # ci chunks of 128 (66*64/128)
KH = 8                  # chunks per PSUM pass


@dataclass
class Plan:
    """Host-side preprocessing result: all per-core device input arrays plus
    the compile-time structure constants."""

    n_cores: int
    wpc: int                 # windows per core
    nt_w: list = None        # tiles per window (same layout for every core)
    nodes_pad: int = 0
    depth: int = DEPTH
    win: int = 128           # nodes per scatter window
    devnode: np.ndarray = None     # [N] original node -> device row
    fix0_free: bool = False
    in_maps: list = field(default_factory=list)
    fc2_b: float = 0.0

    @property
    def ntiles(self):        # edge tiles per core
        return sum(self.nt_w)

    @property
    def epc(self):           # edge slots per core
        return self.ntiles * 128

    @property
    def tile_off(self):      # first tile index of each window
        off, out = 0, []
        for n in self.nt_w:
            out.append(off)
            off += n
        return out


def make_plan(x, edge_index, edge_attr, fc1_W, fc1_b, k1_W, k1_b, k2_W, k2_b,
              root, conv_b, fc2_W, fc2_b, n_cores=8, depth=DEPTH):
    W = WIDTH
    N = x.shape[0]
    E = edge_index.shape[1]
    src = np.asarray(edge_index[0], dtype=np.int64)
    dst = np.asarray(edge_index[1], dtype=np.int64)
    assert np.all(np.asarray(conv_b) == 0.0), "kernel assumes conv_b == 0"

    WIN = 128
    wpc = max(1, int(math.ceil(N / WIN / n_cores)))
    n_windows = n_cores * wpc
    nodes_pad = n_windows * WIN

    counts = np.bincount(dst, minlength=N).astype(np.float64)
    denom = np.where(counts > 0, counts, 1.0)
    invdeg_node = (1.0 / denom).astype(np.float32)

    # Per-window edge-tile capacities: 49 tiles (6272 slots) is the minimum
    # that fits 6250 edges; the 4-tile window sits second-to-last (measured
    # best position).
    base_nt = [5, 5, 5, 5, 5, 5, 5, 5, 4, 5]
    if wpc != 10:  # generic fallback: balanced with one tiny last window
        per = int(math.ceil(E / n_cores / max(1, wpc - 1) / 128)) + 1
        base_nt = [per] * (wpc - 1) + [1]
    cap = np.array([nt * 128 for _ in range(n_cores) for nt in base_nt],
                   dtype=np.int64)

    order = np.argsort(-counts, kind="stable")
    win_edges = np.zeros(n_windows, dtype=np.int64)
    win_fill = np.zeros(n_windows, dtype=np.int64)
    node_window = np.zeros(N, dtype=np.int64)
    node_slot = np.zeros(N, dtype=np.int64)
    NEG = -(1 << 60)
    # greedy: place desc-degree nodes into the window with the most remaining
    # edge capacity that still has node slots; grow a window's capacity by a
    # tile if nothing fits.
    rem = cap.copy()
    for n in order:
        d = int(counts[n])
        w = int(np.argmax(rem))
        if rem[w] < d:
            cap[w] += 128 * int(math.ceil((d - rem[w]) / 128))
            rem[w] = cap[w] - win_edges[w]
        node_window[n] = w
        node_slot[n] = win_fill[w]
        win_fill[w] += 1
        win_edges[w] += d
        rem[w] = cap[w] - win_edges[w] if win_fill[w] < WIN else NEG
    nt_all = (cap // 128).reshape(n_cores, wpc)
    # every core runs one compiled program -> shared nt layout: per-position max
    nt_w = [int(nt_all[:, i].max()) for i in range(wpc)]

    # Repair pass: make window 0 of every core free of "fix" in-edges (edges
    # sourced from any core's last window). Its whole gather then depends
    # only on h windows 0..wpc-2 and the first window of the next iteration
    # starts right at the boundary. Swap dirty w0 members with clean nodes
    # of similar degree from the same core's windows 0..wpc-2.
    lastwin = (node_window % wpc) == (wpc - 1)
    has_fix_in = np.zeros(N, dtype=bool)
    np.logical_or.at(has_fix_in, dst, lastwin[src])
    import bisect

    def try_repair(w0):
        """Swap w0's dirty nodes (fix in-edges) out; True on full success.
        Mutates node_window/node_slot/win_edges only for committed swaps."""
        r = w0 // wpc
        members = np.where(node_window == w0)[0]
        dirty = [n for n in members if has_fix_in[n]]
        cand = [n for n in np.where((node_window // wpc == r)
                                    & (node_window != w0)
                                    & ~lastwin[np.arange(N)]
                                    & ~has_fix_in)[0]]
        cand.sort(key=lambda n: counts[n])
        cdeg = [counts[n] for n in cand]
        swaps = []
        for n_out in dirty:
            d_out = counts[n_out]
            placed = False
            i0 = bisect.bisect_left(cdeg, d_out)
            for i in sorted(range(len(cand)),
                            key=lambda i: abs(i - i0)):
                n_in = cand[i]
                if n_in < 0:
                    continue
                wb = node_window[n_in]
                d_in = counts[n_in]
                if (win_edges[w0] + d_in - d_out <= cap[w0]
                        and win_edges[wb] + d_out - d_in <= cap[wb]):
                    node_window[n_out], node_window[n_in] = wb, w0
                    node_slot[n_out], node_slot[n_in] = \
                        node_slot[n_in], node_slot[n_out]
                    win_edges[w0] += d_in - d_out
                    win_edges[wb] += d_out - d_in
                    cand[i] = -1
                    swaps.append((n_out, n_in, wb))
                    placed = True
                    break
            if not placed:
                for n_out2, n_in2, wb2 in reversed(swaps):
                    node_window[n_out2], node_window[n_in2] = w0, wb2
                    node_slot[n_out2], node_slot[n_in2] = \
                        node_slot[n_in2], node_slot[n_out2]
                    win_edges[w0] += counts[n_out2] - counts[n_in2]
                    win_edges[wb2] += counts[n_in2] - counts[n_out2]
                return False
        return True

    fix0_free = True
    for r in range(n_cores):
        if try_repair(r * wpc):
            continue
        # relabel: swap position 0 with another nt=5 position whose window
        # repairs cleanly (window identity is just a label per core)
        done = False
        for alt in range(1, wpc - 1):
            if nt_w[alt] != nt_w[0]:
                continue
            wa, w0 = r * wpc + alt, r * wpc
            sel0 = node_window == w0
            sela = node_window == wa
            node_window[sel0], node_window[sela] = wa, w0
            win_edges[w0], win_edges[wa] = win_edges[wa], win_edges[w0]
            if try_repair(w0):
                done = True
                break
            sel0 = node_window == w0
            sela = node_window == wa
            node_window[sel0], node_window[sela] = wa, w0
            win_edges[w0], win_edges[wa] = win_edges[wa], win_edges[w0]
        if not done:
            fix0_free = False
    plan_fix0_free = fix0_free

    plan = Plan(n_cores=n_cores, wpc=wpc, nt_w=nt_w, nodes_pad=nodes_pad,
                depth=depth, win=WIN,
                fc2_b=float(np.asarray(fc2_b).reshape(())))
    plan.fix0_free = plan_fix0_free
    ntiles = plan.ntiles
    epc = plan.epc
    woff = [128 * t for t in plan.tile_off]   # slot offset of window in core

    plan.devnode = node_window * WIN + node_slot

    # edge -> slot within its dst window. Edges whose SOURCE lies in any
    # core's last window ("fix" edges) go to the tail of the window's slot
    # range (the last tile): the other tiles' gather then depends only on
    # h windows 0..wpc-2 and overlaps the last window's compute.
    devnode = node_window * WIN + node_slot
    edge_win = node_window[dst]
    is_fix = (devnode[src] % (wpc * WIN)) >= (wpc - 1) * WIN
    ord_e = np.argsort(edge_win, kind="stable")
    fill = np.zeros(n_windows, dtype=np.int64)
    fillb = np.zeros(n_windows, dtype=np.int64)
    eslot = np.zeros(E, dtype=np.int64)
    for e in ord_e:
        w = edge_win[e]
        core, wl = divmod(w, wpc)
        capw = nt_w[wl] * 128
        if is_fix[e]:
            fillb[w] += 1
            eslot[e] = core * epc + woff[wl] + capw - fillb[w]
        else:
            eslot[e] = core * epc + woff[wl] + fill[w]
            fill[w] += 1
    assert all(fill[w] + fillb[w] <= nt_w[w % wpc] * 128
               for w in range(n_windows))
    assert fillb.max() <= 128, "fix edges must fit the last tile"
    if plan_fix0_free:
        assert all(fillb[r * wpc] == 0 for r in range(n_cores))

    tot_slots = n_cores * epc
    slot_src = np.zeros(tot_slots, dtype=np.int64)
    slot_used = np.zeros(tot_slots, dtype=bool)
    slot_vloc = np.zeros(tot_slots, dtype=np.int64)
    slot_invdeg = np.zeros(tot_slots, dtype=np.float32)
    slot_ea = np.zeros((tot_slots, 3), dtype=np.float32)
    slot_src[eslot] = devnode[src]
    del devnode
    slot_used[eslot] = True
    slot_vloc[eslot] = node_slot[dst]
    slot_invdeg[eslot] = invdeg_node[dst]
    slot_ea[eslot] = np.asarray(edge_attr, dtype=np.float32)

    # weight repacks: T_cm [66*64, 64]: rows 0..4095 = k2_W, rows
    # 4096..4159 = k2_b (paired with z's invdeg column), rest zero.
    # chunk layout: T_sb[p, k*64+o] = T_cm[k*128+p, o]
    T_cm = np.zeros((66 * 64, W), dtype=np.float32)
    T_cm[: 64 * 64] = np.ascontiguousarray(
        np.asarray(k2_W, dtype=np.float32).reshape(64, 64, 64)
    ).reshape(64 * 64, W)
    T_cm[64 * 64 : 65 * 64] = np.asarray(k2_b, dtype=np.float32).reshape(64, 64)
    T_sb = np.ascontiguousarray(
        T_cm.reshape(NCH, 128, W).transpose(1, 0, 2)
    ).reshape(128, NCH * W).astype(np.float16)

    # k1 extended to 66 cols: 0-63 = [k1_W; k1_b], 64 = bias-row one (the
    # invdeg ACT-scale turns it into the invdeg column), 65 = zero pad.
    k1_Wb = np.zeros((4, 66), dtype=np.float16)
    k1_Wb[:3, :64] = np.asarray(k1_W, dtype=np.float16)
    k1_Wb[3, :64] = np.asarray(k1_b, dtype=np.float16)
    k1_Wb[3, 64] = 1.0

    # h rows padded to 128 f16 (=256B) so SWDGE dma_gather's 256B-multiple
    # row-stride restriction is met; cols 64.. are never read by compute.
    h0 = np.zeros((nodes_pad, 2 * W), dtype=np.float16)
    h0[plan.devnode, :W] = (np.asarray(x, np.float32) @ np.asarray(fc1_W, np.float32)
                       + np.asarray(fc1_b, np.float32)).astype(np.float16)

    ident = np.eye(64, dtype=np.float16)
    root_np = np.asarray(root, dtype=np.float16)
    fc2_np = np.asarray(fc2_W, dtype=np.float16).reshape(W, 1)

    for r in range(n_cores):
        sl = slice(r * epc, (r + 1) * epc)
        c_ea = slot_ea[sl]
        c_used = slot_used[sl]
        c_invd = slot_invdeg[sl]
        c_vloc = slot_vloc[sl]
        c_src = slot_src[sl]

        eaT = np.zeros((4, epc), dtype=np.float16)
        eaT[:3] = c_ea.T.astype(np.float16)
        eaT[3] = 1.0
        # invdeg in [partition, tile] layout (slot s -> (s//128, s%128))
        invd = np.ascontiguousarray(
            c_invd.reshape(ntiles, 128).T)                       # [128, ntiles]
        tt = np.arange(epc) // 128
        pp = np.arange(epc) % 128
        segT = np.zeros((ntiles, 128, WIN), dtype=np.float16)
        segT[tt[c_used], pp[c_used], c_vloc[c_used]] = 1.0
        segT = np.ascontiguousarray(segT.transpose(1, 0, 2)).reshape(128, ntiles * WIN)

        idx = np.zeros((128, epc // 16), dtype=np.int16)
        base = c_src.astype(np.int16).reshape(epc // 16, 16).T   # [16, epc/16]
        for g in range(8):
            idx[16 * g : 16 * (g + 1)] = base

        h0T = np.ascontiguousarray(
            h0[r * wpc * WIN : (r + 1) * wpc * WIN, :W].T)       # [64, wpc*WIN]


        plan.in_maps.append({
            "eaT": eaT,
            "invdeg": invd,
            "segmatT": segT,
            "idx": idx,
            "h0": h0,
            "h0T": h0T,
            "T_sb": T_sb,
            "k1_Wb": k1_Wb,
            "root": root_np,
            "fc2_W": fc2_np,
            "fc2_b": np.full((WIN, 1), plan.fc2_b, dtype=np.float32),
            "ident": ident,
        })
    return plan


def build_program(plan: Plan, debug=False, single_core=False):
    """Build the SPMD Bass program (one program, run on all cores).

    single_core=True replaces the AllGather with direct local h_full writes
    (and drops addr_space="Shared") so the program can run under TimelineSim
    for cost modeling."""
    W = WIDTH
    WPC = plan.wpc
    WIN = plan.win
    NT_W = plan.nt_w
    TOFF = plan.tile_off
    NTILES = plan.ntiles
    EPC = plan.epc
    NPAD = plan.nodes_pad
    DEP = plan.depth
    NC_ = plan.n_cores
    Relu = mybir.ActivationFunctionType.Relu

    nc = bacc.Bacc("TRN2", target_bir_lowering=False, debug=debug,
                   num_devices=NC_)

    # ---- I/O ----
    eaT_d = nc.dram_tensor("eaT", [4, EPC], F16, kind="ExternalInput")
    invd_d = nc.dram_tensor("invdeg", [128, NTILES], F32, kind="ExternalInput")
    segT_d = nc.dram_tensor("segmatT", [128, NTILES * WIN], F16, kind="ExternalInput")
    idx_d = nc.dram_tensor("idx", [128, EPC // 16], I16, kind="ExternalInput")
    h0_d = nc.dram_tensor("h0", [NPAD, 2 * W], F16, kind="ExternalInput")
    h0T_d = nc.dram_tensor("h0T", [W, WPC * WIN], F16, kind="ExternalInput")
    Tsb_d = nc.dram_tensor("T_sb", [128, NCH * W], F16, kind="ExternalInput")
    k1_d = nc.dram_tensor("k1_Wb", [4, 66], F16, kind="ExternalInput")
    root_d = nc.dram_tensor("root", [W, W], F16, kind="ExternalInput")
    fc2_d = nc.dram_tensor("fc2_W", [W, 1], F16, kind="ExternalInput")
    fc2b_d = nc.dram_tensor("fc2_b", [WIN, 1], F32, kind="ExternalInput")
    id_d = nc.dram_tensor("ident", [64, 64], F16, kind="ExternalInput")
    y_d = nc.dram_tensor("y", [WPC * WIN, 1], F32, kind="ExternalOutput")

    # internal DRAM for the h exchange
    h_slice = [nc.dram_tensor(f"h_slice{i}", [WPC * WIN, 2 * W], F16)
               for i in range(DEP - 1)]
    if single_core:
        h_fullA = [nc.dram_tensor(f"h_fullA{i}", [NPAD, 2 * W], F16)
                   for i in range(DEP - 1)]
        h_full = [nc.dram_tensor(f"h_fullB{i}", [NPAD, 2 * W], F16)
                  for i in range(DEP - 1)]
    else:
        h_full = [nc.dram_tensor(f"h_full{i}", [NPAD, 2 * W], F16,
                                 addr_space="Shared")
                  for i in range(DEP - 1)]
        h_fullA = h_full

    MAXNT = max(NT_W)
    with tile.TileContext(nc) as tc:
        with (
            tc.tile_pool(name="const", bufs=1) as cpool,
            tc.tile_pool(name="hsrc", bufs=2) as hsrc_pool,
            tc.tile_pool(name="z", bufs=2 * MAXNT + 1) as zpool,
            tc.tile_pool(name="zsum_sb", bufs=2) as zsum_sb_pool,
            tc.tile_pool(name="hT", bufs=2) as hT_pool,
            tc.tile_pool(name="small", bufs=4) as spool,
            tc.tile_pool(name="zsum_ps", bufs=2, space="PSUM") as zsum_ps_pool,
            tc.tile_pool(name="agg_ps", bufs=2, space="PSUM") as agg_ps_pool,
            tc.tile_pool(name="tr_ps", bufs=1, space="PSUM") as tr_ps_pool,
        ):
            nc.gpsimd.load_library(library_config.mlp)

            # preload the ACT function table (1.3us) under the const DMAs;
            # Copy needs no bias const-AP (whose DMA would land late)
            warm = cpool.tile([1, 1], F32)
            nc.vector.memset(warm[:], 0.0)
            nc.scalar.activation(warm[:], warm[:],
                                 mybir.ActivationFunctionType.Copy)

            # ---- constants; gather-critical tensors first so window 0's
            # gather + e2 chain + first zsum start ASAP ----
            n0 = NT_W[0] * 128 // 16
            idx0 = cpool.tile([128, n0], I16)
            nc.sync.dma_start(idx0[:], idx_d[:, :n0])
            idx = cpool.tile([128, EPC // 16], I16)
            nc.sync.dma_start(idx[:], idx_d[:])
            eaT = cpool.tile([4, EPC], F16)
            nc.sync.dma_start(eaT[:], eaT_d[:])
            invd = cpool.tile([128, NTILES], F32)
            nc.sync.dma_start(invd[:], invd_d[:])
            k1 = cpool.tile([4, 66], F16)
            nc.sync.dma_start(k1[:], k1_d[:])
            segT = cpool.tile([128, NTILES * WIN], F16)
            nc.sync.dma_start(segT[:], segT_d[:])
            Tsb = cpool.tile([128, NCH * W], F16)
            nc.sync.dma_start(Tsb[:], Tsb_d[:])
            h0T = cpool.tile([W, WPC * WIN], F16)
            nc.sync.dma_start(h0T[:], h0T_d[:])
            rootW = cpool.tile([W, W], F16)
            nc.sync.dma_start(rootW[:], root_d[:])
            fc2 = cpool.tile([W, 1], F16)
            nc.sync.dma_start(fc2[:], fc2_d[:])
            fc2b = cpool.tile([WIN, 1], F32)
            nc.sync.dma_start(fc2b[:], fc2b_d[:])
            ident = cpool.tile([64, 64], F16)
            nc.sync.dma_start(ident[:], id_d[:])

            # ---- e2dup: [128, NTILES*64*2] fp16, every value twice so the
            # z-build APs end in a packed (stride 1, count 2) dim on ALL
            # operands -> DVE 2x mode. relu(x*invdeg) = invdeg*relu(x) folds
            # the scatter-mean denominator into the ACT scale. ----
            e2dup = cpool.tile([128, NTILES * 66 * 2], F16)

            def build_e2dup(t):
                e2_ps = agg_ps_pool.tile([128, 66], F32, tag="a")
                nc.tensor.matmul(e2_ps[:], eaT[:, t * 128:(t + 1) * 128],
                                 k1[:], start=True, stop=True)
                dup = e2dup[:, t * 132:(t + 1) * 132] \
                    .rearrange("p (c b) -> p c b", b=2)
                for b in range(2):
                    nc.scalar.activation(dup[:, :, b], e2_ps[:], Relu,
                                         scale=invd[:, t: t + 1])

            def build_e2dup_win(w):
                for et in range(NT_W[w]):
                    build_e2dup(TOFF[w] + et)

            # only window 0's e2dup up front: emitting all of it here would
            # queue 24us of ACT work ahead of iteration 0's PSUM drains (ACT
            # executes in order) and stall the whole pipeline; z(w) also
            # waits on every e2dup write emitted before it (tile-granular
            # dependency tracking), so later windows' builds are staggered
            # through iteration 0.
            build_e2dup_win(0)

            hT_cur = h0T
            y_sb = spool.tile([WIN, WPC], F32, tag="y")

            for it in range(DEP):
                gsrcA = h0_d if it == 0 else h_fullA[it - 1]
                gsrcB = h0_d if it == 0 else h_full[it - 1]
                # Two gathers per window into its own tiles: the MAIN gather
                # (tiles 0..nt-2, whose edges by construction have sources in
                # windows 0..wpc-2) runs as soon as those h windows land and
                # overlaps the last window's compute; only the small FIX
                # gather (last tile) waits for the final h window.
                h_srcs = [None] * WPC
                h_fix = [None] * WPC

                def issue_gather(w):
                    nt = NT_W[w]
                    o = TOFF[w] * 128
                    ix = idx0 if (w == 0 and it == 0) else idx
                    if w == 0 and plan.fix0_free:
                        # window 0 has no fix edges: one full gather from A
                        hs_w = hsrc_pool.tile([128, nt, 2 * W], F16, tag="h0f")
                        n = nt * 128
                        nc.gpsimd.dma_gather(
                            hs_w[:], gsrcA[:],
                            ix[:, o // 16:(o + n) // 16], n, n, 2 * W)
                        h_srcs[0] = hs_w
                        h_fix[0] = None
                        return
                    nm = (nt - 1) * 128
                    hs_w = hsrc_pool.tile([128, nt - 1, 2 * W], F16,
                                          tag=f"h{w}")
                    nc.gpsimd.dma_gather(
                        hs_w[:], gsrcA[:],
                        ix[:, o // 16:(o + nm) // 16], nm, nm, 2 * W)
                    h_srcs[w] = hs_w
                    hf_w = hsrc_pool.tile([128, 1, 2 * W], F16, tag=f"hf{w}")
                    nc.gpsimd.dma_gather(
                        hf_w[:], gsrcB[:],
                        ix[:, (o + nm) // 16:(o + nm + 128) // 16], 128, 128,
                        2 * W)
                    h_fix[w] = hf_w

                for _w0 in range(4):
                    issue_gather(_w0)
                hT_next = hT_pool.tile([W, WPC * WIN], F16)

                def write_h(w):
                    # transpose hT_next[w] and write it to DRAM. Deferred by
                    # one window (emitted after the NEXT window's zsum
                    # passes) so the in-order PE never stalls waiting for
                    # relu(w) on ACT; the last two windows are emitted inline
                    # since their writes gate the next iteration's gathers.
                    h_ps = tr_ps_pool.tile([WIN, 64], F16, tag="tr")
                    nc.tensor.transpose(h_ps[:],
                                        hT_next[:, w * WIN:(w + 1) * WIN],
                                        ident[:])
                    h_sb = spool.tile([WIN, 64], F16, tag="hnew")
                    nc.scalar.copy(h_sb[:], h_ps[:])
                    if single_core:
                        if w < WPC - 1:
                            nc.sync.dma_start(
                                h_fullA[it][w * WIN:(w + 1) * WIN, :W],
                                h_sb[:])
                            if w == WPC - 2:
                                # B gets windows 0..wpc-2 via one bulk
                                # copy (fix gathers wait for the last
                                # window anyway, so this is off the
                                # critical path)
                                nc.sync.dma_start(
                                    h_full[it][: (WPC - 1) * WIN, :],
                                    h_fullA[it][: (WPC - 1) * WIN, :])
                        else:
                            nc.sync.dma_start(
                                h_full[it][w * WIN:(w + 1) * WIN, :W],
                                h_sb[:])
                    else:
                        nc.sync.dma_start(
                            h_slice[it][w * WIN:(w + 1) * WIN, :W], h_sb[:])

                def emit_y(w):
                    y_ps = agg_ps_pool.tile([WIN, 1], F32, tag="a")
                    nc.tensor.matmul(y_ps[:],
                                     hT_next[:, w * WIN:(w + 1) * WIN],
                                     fc2[:], start=True, stop=True)
                    nc.vector.tensor_add(y_sb[:, w: w + 1], y_ps[:], fc2b[:])

                pending_tr = []
                pending_y = []
                for w in range(WPC):
                    nt = NT_W[w]
                    t0 = TOFF[w]
                    zs = []
                    for et in range(nt):
                        t = t0 + et
                        z = zpool.tile([128, NCH * 128], F16)
                        # all-fp16 operands with packed (1,2) last dims hit
                        # the DVE 2x perf mode (stride-0 last dims do not)
                        zv = z[:].rearrange("p (c a b) -> p c a b", c=66, b=2)
                        full = w == 0 and plan.fix0_free
                        h_t = h_srcs[w] if (full or et < nt - 1) else h_fix[w]
                        e_t = et if (full or et < nt - 1) else 0
                        hs = h_t[:, e_t, :W] \
                            .rearrange("p (a b) -> p a b", b=2) \
                            .unsqueeze(1).broadcast_to((128, 66, 32, 2))
                        e2 = e2dup[:, t * 132:(t + 1) * 132] \
                            .rearrange("p (c b) -> p c b", b=2) \
                            .unsqueeze(2).broadcast_to((128, 66, 32, 2))
                        if et == nt - 1 and w == 0 \
                                and not plan.fix0_free:
                            # window 0's fix tile sits on the iteration
                            # boundary critical path: build it in KH-chunk
                            # pieces so pass 0 can start after the first one
                            for c0 in range(0, 66, 8):
                                c1 = min(c0 + 8, 66)
                                nc.vector.tensor_mul(
                                    zv[:, c0:c1, :, :], hs[:, c0:c1, :, :],
                                    e2[:, c0:c1, :, :])
                            zs.append(z)
                            continue
                        # offload part of the first tile of each window to
                        # the (idle) GPSIMD engine; DVE builds the rest.
                        # Not in iteration 0's first windows: Pool is still
                        # busy with the initial gather burst there.
                        if et == 0 and not (it == 0 and w < 3):
                            nc.gpsimd.tensor_mul(
                                zv[:, :24, :, :], hs[:, :24, :, :],
                                e2[:, :24, :, :])
                            nc.vector.tensor_mul(
                                zv[:, 24:, :, :], hs[:, 24:, :, :],
                                e2[:, 24:, :, :])
                        else:
                            nc.vector.tensor_mul(zv, hs, e2)
                        zs.append(z)
                    if w + 4 < WPC:
                        issue_gather(w + 4)
                    if it == 0:
                        if w == 0 and WPC > 1:
                            build_e2dup_win(1)
                        if w + 2 < WPC:
                            build_e2dup_win(w + 2)
                    # zsum in KH-chunk PSUM passes (back-to-back on PE; the
                    # drains pipeline on ACT), then the T-contract block.
                    zsum_sb = zsum_sb_pool.tile([128, NCH * WIN], F16)
                    for p0 in range(0, NCH, KH):
                        p1 = min(p0 + KH, NCH)
                        zsum_ps = zsum_ps_pool.tile([128, KH * WIN], F32)
                        for k in range(p0, p1):
                            for et in range(nt):
                                nc.tensor.matmul(
                                    zsum_ps[:, (k - p0) * WIN:(k - p0 + 1) * WIN],
                                    zs[et][:, k * 128:(k + 1) * 128],
                                    segT[:, (t0 + et) * WIN:(t0 + et + 1) * WIN],
                                    start=(et == 0), stop=(et == nt - 1))
                        # keep the DVE free for z-builds (critical engine) —
                        # drain PSUM on ACT
                        nc.scalar.copy(zsum_sb[:, p0 * WIN:p1 * WIN],
                                       zsum_ps[:, :(p1 - p0) * WIN])
                    if pending_tr:
                        write_h(pending_tr.pop())
                    while len(pending_y) > 1:
                        emit_y(pending_y.pop(0))
                    agg_ps = agg_ps_pool.tile([64, WIN], F32, tag="a")
                    for k in range(NCH):
                        nc.tensor.matmul(agg_ps[:],
                                         Tsb[:, k * W:(k + 1) * W],
                                         zsum_sb[:, k * WIN:(k + 1) * WIN],
                                         start=(k == 0), stop=False)
                    nc.tensor.matmul(agg_ps[:], rootW[:],
                                     hT_cur[:, w * WIN:(w + 1) * WIN],
                                     start=False, stop=True)
                    nc.scalar.activation(hT_next[:, w * WIN:(w + 1) * WIN],
                                         agg_ps[:], Relu)
                    if it == DEP - 1:
                        pending_y.append(w)
                        if w == WPC - 1:
                            while pending_y:
                                emit_y(pending_y.pop(0))
                    else:
                        if w >= WPC - 2:
                            # w8's write gates the next iteration's main
                            # gathers, w9's its fix gathers: keep both inline
                            if pending_tr:
                                write_h(pending_tr.pop())
                            write_h(w)
                        else:
                            pending_tr.append(w)
                hT_cur = hT_next
                if it < DEP - 1 and not single_core:
                    nc.gpsimd.collective_compute(
                        "AllGather",
                        mybir.AluOpType.bypass,
                        ins=[h_slice[it][:].opt()],
                        outs=[h_full[it][:].opt()],
                        replica_groups=[list(range(NC_))],
                    )

            # ---- output ----
            y_view = y_d[:].rearrange("(w v) o -> v (w o)", w=WPC)
            nc.sync.dma_start(y_view, y_sb[:])

    nc.compile()
    return nc


def kernel(**inputs) -> np.ndarray:
    from concourse.bass_utils import run_bass_kernel_spmd

    plan = make_plan(**{k: np.asarray(v) for k, v in inputs.items()})
    nc = build_program(plan)
    core_ids = list(range(plan.n_cores))
    res = run_bass_kernel_spmd(nc, plan.in_maps, core_ids,
                               trace=bool(int(os.environ.get("KERNEL_TRACE", "0"))))
    y = np.concatenate([res.results[r]["y"] for r in range(plan.n_cores)], axis=0)
    out = y[plan.devnode]
    kernel.last_results = res
    kernel.last_plan = plan
    return out


# revision 39
# speedup vs baseline: 2.1574x; 1.0010x over previous
"""Trainium2 Bass kernel for nn_Net_MP_68805376082308 (NNConv-style GNN).

Reference computation (see problem statement):
    h = x@fc1 + b
    e2 = relu(edge_attr@k1 + b1)                     # [E, 64]
    ew = (e2 @ k2 + b2).reshape(E, 64, 64)           # never materialized here!
    for 4 iters:
        msg  = einsum('ei,eio->eo', h[src], ew)
        agg  = segment_sum(msg, dst) / max(deg,1)
        h    = relu(agg + h@root)
    out = h @ fc2 + b

Device algorithm (per core, node-sharded, dst-grouped edge slots):
    e2s[e, c]    = relu((edge_attr@k1+b1)[e,c]) * invdeg[dst[e]]  (c in 0..63)
    z[e, c*64+i] = e2s[e,c] * h[src[e], i]        # DVE, fp16 pair-trick APs
    zsumT[ci, v] = sum_e z[e,ci] * SegMat[e,v]    # PE, z stationary (scatter
                                                  #  commutes with k2 contract)
    aggT[o, v]   = T_cm.T @ zsumT + root.T @ hT   # PE (T_cm rows 4096..4159
                                                  #  carry k2_b, paired with
                                                  #  z's invdeg column)
    hT           = relu(aggT)                     # ACT
    h[src] gather via SWDGE dma_gather; h exchanged across 8 cores with an
    AllGather after each iteration.

Windows hold 128 dst-node slots and 5 edge tiles each. Edges whose SOURCE
node lies in any core's last window are segregated into each window's last
("fix") tile: the other tiles' gathers then depend only on h windows 0..8 and
overlap the last window's compute at the iteration boundary (split h_fullA/B
tensors express this to the dependency tracker in the single-core cost model;
the real 8-core path keeps one h_full fed by an AllGather).

kernel(**inputs) takes the FULL unsharded inputs and returns [10000, 1] fp32.
"""

import math
import os
import sys
from dataclasses import dataclass, field

import numpy as np

sys.path.insert(0, "/opt/trn_rl_repo")

import concourse.bacc as bacc
import concourse.bass as bass
import concourse.mybir as mybir
import concourse.tile as tile
from concourse import library_config

F32 = mybir.dt.float32
F16 = mybir.dt.float16
I16 = mybir.dt.int16

WIDTH = 64
DEPTH = 4
NCH = 33                # ci chunks of 128 (66*64/128)
KH = 8                  # chunks per PSUM pass


@dataclass
class Plan:
    """Host-side preprocessing result: all per-core device input arrays plus
    the compile-time structure constants."""

    n_cores: int
    wpc: int                 # windows per core
    nt_w: list = None        # tiles per window (same layout for every core)
    nodes_pad: int = 0
    depth: int = DEPTH
    win: int = 128           # nodes per scatter window
    devnode: np.ndarray = None     # [N] original node -> device row
    fix0_free: bool = False
    vf_w: list = None
    in_maps: list = field(default_factory=list)
    fc2_b: float = 0.0

    @property
    def ntiles(self):        # edge tiles per core
        return sum(self.nt_w)

    @property
    def epc(self):           # edge slots per core
        return self.ntiles * 128

    @property
    def tile_off(self):      # first tile index of each window
        off, out = 0, []
        for n in self.nt_w:
            out.append(off)
            off += n
        return out


def make_plan(x, edge_index, edge_attr, fc1_W, fc1_b, k1_W, k1_b, k2_W, k2_b,
              root, conv_b, fc2_W, fc2_b, n_cores=8, depth=DEPTH):
    W = WIDTH
    N = x.shape[0]
    E = edge_index.shape[1]
    src = np.asarray(edge_index[0], dtype=np.int64)
    dst = np.asarray(edge_index[1], dtype=np.int64)
    assert np.all(np.asarray(conv_b) == 0.0), "kernel assumes conv_b == 0"

    WIN = 128
    wpc = max(1, int(math.ceil(N / WIN / n_cores)))
    n_windows = n_cores * wpc
    nodes_pad = n_windows * WIN
    # per-position node caps: the zsum/T/root moving dim is the node count
    # VF_W[w] (< 128) of each window; the 4-tile window gets fewer nodes so
    # its smaller edge capacity still suffices. Total = N / n_cores.
    per_core = N // n_cores
    VF_W = [128] * wpc
    if wpc == 10:
        VF_W = [128, 128, 128, 128, 128, 128, 128, 128, 101,
                per_core - 8 * 128 - 101]
    else:
        base = per_core // wpc
        VF_W = [base] * wpc
        VF_W[-1] += per_core - base * wpc
    assert sum(VF_W) == per_core and all(0 < v <= 128 for v in VF_W)
    ncap = np.array([v for _ in range(n_cores) for v in VF_W], dtype=np.int64)

    counts = np.bincount(dst, minlength=N).astype(np.float64)
    denom = np.where(counts > 0, counts, 1.0)
    invdeg_node = (1.0 / denom).astype(np.float32)

    # Per-window edge-tile capacities: 49 tiles (6272 slots) is the minimum
    # that fits 6250 edges; the 4-tile window sits second-to-last (measured
    # best position).
    base_nt = [5, 5, 5, 5, 5, 5, 5, 5, 4, 5]
    if wpc != 10:  # generic fallback: balanced with one tiny last window
        per = int(math.ceil(E / n_cores / max(1, wpc - 1) / 128)) + 1
        base_nt = [per] * (wpc - 1) + [1]
    cap = np.array([nt * 128 for _ in range(n_cores) for nt in base_nt],
                   dtype=np.int64)

    order = np.argsort(-counts, kind="stable")
    win_edges = np.zeros(n_windows, dtype=np.int64)
    win_fill = np.zeros(n_windows, dtype=np.int64)
    node_window = np.zeros(N, dtype=np.int64)
    node_slot = np.zeros(N, dtype=np.int64)
    NEG = -(1 << 60)
    # greedy: place desc-degree nodes into the window with the most remaining
    # edge capacity that still has node slots; grow a window's capacity by a
    # tile if nothing fits.
    rem = cap.copy()
    for n in order:
        d = int(counts[n])
        w = int(np.argmax(rem))
        if rem[w] < d:
            cap[w] += 128 * int(math.ceil((d - rem[w]) / 128))
            rem[w] = cap[w] - win_edges[w]
        node_window[n] = w
        node_slot[n] = win_fill[w]
        win_fill[w] += 1
        win_edges[w] += d
        rem[w] = cap[w] - win_edges[w] if win_fill[w] < ncap[w] else NEG
    nt_all = (cap // 128).reshape(n_cores, wpc)
    # every core runs one compiled program -> shared nt layout: per-position max
    nt_w = [int(nt_all[:, i].max()) for i in range(wpc)]

    # Repair pass: make window 0 of every core free of "fix" in-edges (edges
    # sourced from any core's last window). Its whole gather then depends
    # only on h windows 0..wpc-2 and the first window of the next iteration
    # starts right at the boundary. Swap dirty w0 members with clean nodes
    # of similar degree from the same core's windows 0..wpc-2.
    lastwin = (node_window % wpc) == (wpc - 1)
    has_fix_in = np.zeros(N, dtype=bool)
    np.logical_or.at(has_fix_in, dst, lastwin[src])
    import bisect

    def try_repair(w0):
        """Swap w0's dirty nodes (fix in-edges) out; True on full success.
        Mutates node_window/node_slot/win_edges only for committed swaps."""
        r = w0 // wpc
        members = np.where(node_window == w0)[0]
        dirty = [n for n in members if has_fix_in[n]]
        cand = [n for n in np.where((node_window // wpc == r)
                                    & (node_window != w0)
                                    & ~lastwin[np.arange(N)]
                                    & ~has_fix_in)[0]]
        cand.sort(key=lambda n: counts[n])
        cdeg = [counts[n] for n in cand]
        swaps = []
        for n_out in dirty:
            d_out = counts[n_out]
            placed = False
            i0 = bisect.bisect_left(cdeg, d_out)
            for i in sorted(range(len(cand)),
                            key=lambda i: abs(i - i0)):
                n_in = cand[i]
                if n_in < 0:
                    continue
                wb = node_window[n_in]
                d_in = counts[n_in]
                if (win_edges[w0] + d_in - d_out <= cap[w0]
                        and win_edges[wb] + d_out - d_in <= cap[wb]):
                    node_window[n_out], node_window[n_in] = wb, w0
                    node_slot[n_out], node_slot[n_in] = \
                        node_slot[n_in], node_slot[n_out]
                    win_edges[w0] += d_in - d_out
                    win_edges[wb] += d_out - d_in
                    cand[i] = -1
                    swaps.append((n_out, n_in, wb))
                    placed = True
                    break
            if not placed:
                for n_out2, n_in2, wb2 in reversed(swaps):
                    node_window[n_out2], node_window[n_in2] = w0, wb2
                    node_slot[n_out2], node_slot[n_in2] = \
                        node_slot[n_in2], node_slot[n_out2]
                    win_edges[w0] += counts[n_out2] - counts[n_in2]
                    win_edges[wb2] += counts[n_in2] - counts[n_out2]
                return False
        return True

    fix0_free = True
    for r in range(n_cores):
        if try_repair(r * wpc):
            continue
        # relabel: swap position 0 with another nt=5 position whose window
        # repairs cleanly (window identity is just a label per core)
        done = False
        for alt in range(1, wpc - 1):
            if nt_w[alt] != nt_w[0]:
                continue
            wa, w0 = r * wpc + alt, r * wpc
            sel0 = node_window == w0
            sela = node_window == wa
            node_window[sel0], node_window[sela] = wa, w0
            win_edges[w0], win_edges[wa] = win_edges[wa], win_edges[w0]
            if try_repair(w0):
                done = True
                break
            sel0 = node_window == w0
            sela = node_window == wa
            node_window[sel0], node_window[sela] = wa, w0
            win_edges[w0], win_edges[wa] = win_edges[wa], win_edges[w0]
        if not done:
            fix0_free = False
    plan_fix0_free = fix0_free

    assert np.all(win_fill <= ncap)
    plan = Plan(n_cores=n_cores, wpc=wpc, nt_w=nt_w, nodes_pad=nodes_pad,
                depth=depth, win=WIN,
                fc2_b=float(np.asarray(fc2_b).reshape(())))
    plan.fix0_free = plan_fix0_free
    plan.vf_w = VF_W
    ntiles = plan.ntiles
    epc = plan.epc
    woff = [128 * t for t in plan.tile_off]   # slot offset of window in core

    plan.devnode = node_window * WIN + node_slot

    # edge -> slot within its dst window. Edges whose SOURCE lies in any
    # core's last window ("fix" edges) go to the tail of the window's slot
    # range (the last tile): the other tiles' gather then depends only on
    # h windows 0..wpc-2 and overlaps the last window's compute.
    devnode = node_window * WIN + node_slot
    edge_win = node_window[dst]
    is_fix = (devnode[src] % (wpc * WIN)) >= (wpc - 1) * WIN
    ord_e = np.argsort(edge_win, kind="stable")
    fill = np.zeros(n_windows, dtype=np.int64)
    fillb = np.zeros(n_windows, dtype=np.int64)
    eslot = np.zeros(E, dtype=np.int64)
    for e in ord_e:
        w = edge_win[e]
        core, wl = divmod(w, wpc)
        capw = nt_w[wl] * 128
        if is_fix[e]:
            fillb[w] += 1
            eslot[e] = core * epc + woff[wl] + capw - fillb[w]
        else:
            eslot[e] = core * epc + woff[wl] + fill[w]
            fill[w] += 1
    assert all(fill[w] + fillb[w] <= nt_w[w % wpc] * 128
               for w in range(n_windows))
    assert fillb.max() <= 128, "fix edges must fit the last tile"
    if plan_fix0_free:
        assert all(fillb[r * wpc] == 0 for r in range(n_cores))

    tot_slots = n_cores * epc
    slot_src = np.zeros(tot_slots, dtype=np.int64)
    slot_used = np.zeros(tot_slots, dtype=bool)
    slot_vloc = np.zeros(tot_slots, dtype=np.int64)
    slot_invdeg = np.zeros(tot_slots, dtype=np.float32)
    slot_ea = np.zeros((tot_slots, 3), dtype=np.float32)
    slot_src[eslot] = devnode[src]
    del devnode
    slot_used[eslot] = True
    slot_vloc[eslot] = node_slot[dst]
    slot_invdeg[eslot] = invdeg_node[dst]
    slot_ea[eslot] = np.asarray(edge_attr, dtype=np.float32)

    # weight repacks: T_cm [66*64, 64]: rows 0..4095 = k2_W, rows
    # 4096..4159 = k2_b (paired with z's invdeg column), rest zero.
    # chunk layout: T_sb[p, k*64+o] = T_cm[k*128+p, o]
    T_cm = np.zeros((66 * 64, W), dtype=np.float32)
    T_cm[: 64 * 64] = np.ascontiguousarray(
        np.asarray(k2_W, dtype=np.float32).reshape(64, 64, 64)
    ).reshape(64 * 64, W)
    T_cm[64 * 64 : 65 * 64] = np.asarray(k2_b, dtype=np.float32).reshape(64, 64)
    T_sb = np.ascontiguousarray(
        T_cm.reshape(NCH, 128, W).transpose(1, 0, 2)
    ).reshape(128, NCH * W).astype(np.float16)

    # k1 extended to 66 cols: 0-63 = [k1_W; k1_b], 64 = bias-row one (the
    # invdeg ACT-scale turns it into the invdeg column), 65 = zero pad.
    k1_Wb = np.zeros((4, 66), dtype=np.float16)
    k1_Wb[:3, :64] = np.asarray(k1_W, dtype=np.float16)
    k1_Wb[3, :64] = np.asarray(k1_b, dtype=np.float16)
    k1_Wb[3, 64] = 1.0

    # h rows padded to 128 f16 (=256B) so SWDGE dma_gather's 256B-multiple
    # row-stride restriction is met; cols 64.. are never read by compute.
    h0 = np.zeros((nodes_pad, 2 * W), dtype=np.float16)
    h0[plan.devnode, :W] = (np.asarray(x, np.float32) @ np.asarray(fc1_W, np.float32)
                       + np.asarray(fc1_b, np.float32)).astype(np.float16)

    ident = np.eye(64, dtype=np.float16)
    root_np = np.asarray(root, dtype=np.float16)
    fc2_np = np.asarray(fc2_W, dtype=np.float16).reshape(W, 1)

    for r in range(n_cores):
        sl = slice(r * epc, (r + 1) * epc)
        c_ea = slot_ea[sl]
        c_used = slot_used[sl]
        c_invd = slot_invdeg[sl]
        c_vloc = slot_vloc[sl]
        c_src = slot_src[sl]

        eaT = np.zeros((4, epc), dtype=np.float16)
        eaT[:3] = c_ea.T.astype(np.float16)
        eaT[3] = 1.0
        # invdeg in [partition, tile] layout (slot s -> (s//128, s%128))
        invd = np.ascontiguousarray(
            c_invd.reshape(ntiles, 128).T)                       # [128, ntiles]
        tt = np.arange(epc) // 128
        pp = np.arange(epc) % 128
        segT = np.zeros((ntiles, 128, WIN), dtype=np.float16)
        segT[tt[c_used], pp[c_used], c_vloc[c_used]] = 1.0
        segT = np.ascontiguousarray(segT.transpose(1, 0, 2)).reshape(128, ntiles * WIN)

        idx = np.zeros((128, epc // 16), dtype=np.int16)
        base = c_src.astype(np.int16).reshape(epc // 16, 16).T   # [16, epc/16]
        for g in range(8):
            idx[16 * g : 16 * (g + 1)] = base

        h0T = np.ascontiguousarray(
            h0[r * wpc * WIN : (r + 1) * wpc * WIN, :W].T)       # [64, wpc*WIN]


        plan.in_maps.append({
            "eaT": eaT,
            "invdeg": invd,
            "segmatT": segT,
            "idx": idx,
            "h0": h0,
            "h0T": h0T,
            "T_sb": T_sb,
            "k1_Wb": k1_Wb,
            "root": root_np,
            "fc2_W": fc2_np,
            "fc2_b": np.full((WIN, 1), plan.fc2_b, dtype=np.float32),
            "ident": ident,
        })
    return plan


def build_program(plan: Plan, debug=False, single_core=False):
    """Build the SPMD Bass program (one program, run on all cores).

    single_core=True replaces the AllGather with direct local h_full writes
    (and drops addr_space="Shared") so the program can run under TimelineSim
    for cost modeling."""
    W = WIDTH
    WPC = plan.wpc
    WIN = plan.win
    NT_W = plan.nt_w
    TOFF = plan.tile_off
    NTILES = plan.ntiles
    EPC = plan.epc
    NPAD = plan.nodes_pad
    VF_W = plan.vf_w
    DEP = plan.depth
    NC_ = plan.n_cores
    Relu = mybir.ActivationFunctionType.Relu

    nc = bacc.Bacc("TRN2", target_bir_lowering=False, debug=debug,
                   num_devices=NC_)

    # ---- I/O ----
    eaT_d = nc.dram_tensor("eaT", [4, EPC], F16, kind="ExternalInput")
    invd_d = nc.dram_tensor("invdeg", [128, NTILES], F32, kind="ExternalInput")
    segT_d = nc.dram_tensor("segmatT", [128, NTILES * WIN], F16, kind="ExternalInput")
    idx_d = nc.dram_tensor("idx", [128, EPC // 16], I16, kind="ExternalInput")
    h0_d = nc.dram_tensor("h0", [NPAD, 2 * W], F16, kind="ExternalInput")
    h0T_d = nc.dram_tensor("h0T", [W, WPC * WIN], F16, kind="ExternalInput")
    Tsb_d = nc.dram_tensor("T_sb", [128, NCH * W], F16, kind="ExternalInput")
    k1_d = nc.dram_tensor("k1_Wb", [4, 66], F16, kind="ExternalInput")
    root_d = nc.dram_tensor("root", [W, W], F16, kind="ExternalInput")
    fc2_d = nc.dram_tensor("fc2_W", [W, 1], F16, kind="ExternalInput")
    fc2b_d = nc.dram_tensor("fc2_b", [WIN, 1], F32, kind="ExternalInput")
    id_d = nc.dram_tensor("ident", [64, 64], F16, kind="ExternalInput")
    y_d = nc.dram_tensor("y", [WPC * WIN, 1], F32, kind="ExternalOutput")

    # internal DRAM for the h exchange
    h_slice = [nc.dram_tensor(f"h_slice{i}", [WPC * WIN, 2 * W], F16)
               for i in range(DEP - 1)]
    if single_core:
        h_fullA = [nc.dram_tensor(f"h_fullA{i}", [NPAD, 2 * W], F16)
                   for i in range(DEP - 1)]
        h_full = [nc.dram_tensor(f"h_fullB{i}", [NPAD, 2 * W], F16)
                  for i in range(DEP - 1)]
    else:
        h_full = [nc.dram_tensor(f"h_full{i}", [NPAD, 2 * W], F16,
                                 addr_space="Shared")
                  for i in range(DEP - 1)]
        h_fullA = h_full

    MAXNT = max(NT_W)
    with tile.TileContext(nc) as tc:
        with (
            tc.tile_pool(name="const", bufs=1) as cpool,
            tc.tile_pool(name="hsrc", bufs=2) as hsrc_pool,
            tc.tile_pool(name="z", bufs=2 * MAXNT + 1) as zpool,
            tc.tile_pool(name="zsum_sb", bufs=2) as zsum_sb_pool,
            tc.tile_pool(name="hT", bufs=2) as hT_pool,
            tc.tile_pool(name="small", bufs=4) as spool,
            tc.tile_pool(name="zsum_ps", bufs=2, space="PSUM") as zsum_ps_pool,
            tc.tile_pool(name="agg_ps", bufs=2, space="PSUM") as agg_ps_pool,
            tc.tile_pool(name="tr_ps", bufs=1, space="PSUM") as tr_ps_pool,
        ):
            nc.gpsimd.load_library(library_config.mlp)

            # preload the ACT function table (1.3us) under the const DMAs;
            # Copy needs no bias const-AP (whose DMA would land late)
            warm = cpool.tile([1, 1], F32)
            nc.vector.memset(warm[:], 0.0)
            nc.scalar.activation(warm[:], warm[:],
                                 mybir.ActivationFunctionType.Copy)

            # ---- constants; gather-critical tensors first so window 0's
            # gather + e2 chain + first zsum start ASAP ----
            n0 = NT_W[0] * 128 // 16
            idx0 = cpool.tile([128, n0], I16)
            nc.sync.dma_start(idx0[:], idx_d[:, :n0])
            idx = cpool.tile([128, EPC // 16], I16)
            nc.sync.dma_start(idx[:], idx_d[:])
            eaT = cpool.tile([4, EPC], F16)
            nc.sync.dma_start(eaT[:], eaT_d[:])
            invd = cpool.tile([128, NTILES], F32)
            nc.sync.dma_start(invd[:], invd_d[:])
            k1 = cpool.tile([4, 66], F16)
            nc.sync.dma_start(k1[:], k1_d[:])
            segT = cpool.tile([128, NTILES * WIN], F16)
            nc.sync.dma_start(segT[:], segT_d[:])
            Tsb = cpool.tile([128, NCH * W], F16)
            nc.sync.dma_start(Tsb[:], Tsb_d[:])
            h0T = cpool.tile([W, WPC * WIN], F16)
            nc.sync.dma_start(h0T[:], h0T_d[:])
            rootW = cpool.tile([W, W], F16)
            nc.sync.dma_start(rootW[:], root_d[:])
            fc2 = cpool.tile([W, 1], F16)
            nc.sync.dma_start(fc2[:], fc2_d[:])
            fc2b = cpool.tile([WIN, 1], F32)
            nc.sync.dma_start(fc2b[:], fc2b_d[:])
            ident = cpool.tile([64, 64], F16)
            nc.sync.dma_start(ident[:], id_d[:])

            # ---- e2dup: [128, NTILES*64*2] fp16, every value twice so the
            # z-build APs end in a packed (stride 1, count 2) dim on ALL
            # operands -> DVE 2x mode. relu(x*invdeg) = invdeg*relu(x) folds
            # the scatter-mean denominator into the ACT scale. ----
            e2dup = cpool.tile([128, NTILES * 66 * 2], F16)

            def build_e2dup(t):
                e2_ps = agg_ps_pool.tile([128, 66], F32, tag="a")
                nc.tensor.matmul(e2_ps[:], eaT[:, t * 128:(t + 1) * 128],
                                 k1[:], start=True, stop=True)
                dup = e2dup[:, t * 132:(t + 1) * 132] \
                    .rearrange("p (c b) -> p c b", b=2)
                for b in range(2):
                    nc.scalar.activation(dup[:, :, b], e2_ps[:], Relu,
                                         scale=invd[:, t: t + 1])

            def build_e2dup_win(w):
                for et in range(NT_W[w]):
                    build_e2dup(TOFF[w] + et)

            # only window 0's e2dup up front: emitting all of it here would
            # queue 24us of ACT work ahead of iteration 0's PSUM drains (ACT
            # executes in order) and stall the whole pipeline; z(w) also
            # waits on every e2dup write emitted before it (tile-granular
            # dependency tracking), so later windows' builds are staggered
            # through iteration 0.
            build_e2dup_win(0)

            hT_cur = h0T
            y_sb = spool.tile([WIN, WPC], F32, tag="y")

            for it in range(DEP):
                gsrcA = h0_d if it == 0 else h_fullA[it - 1]
                gsrcB = h0_d if it == 0 else h_full[it - 1]
                # Two gathers per window into its own tiles: the MAIN gather
                # (tiles 0..nt-2, whose edges by construction have sources in
                # windows 0..wpc-2) runs as soon as those h windows land and
                # overlaps the last window's compute; only the small FIX
                # gather (last tile) waits for the final h window.
                h_srcs = [None] * WPC
                h_fix = [None] * WPC

                def issue_gather(w):
                    nt = NT_W[w]
                    o = TOFF[w] * 128
                    ix = idx0 if (w == 0 and it == 0) else idx
                    if w == 0 and plan.fix0_free:
                        # window 0 has no fix edges: one full gather from A
                        hs_w = hsrc_pool.tile([128, nt, 2 * W], F16, tag="h0f")
                        n = nt * 128
                        nc.gpsimd.dma_gather(
                            hs_w[:], gsrcA[:],
                            ix[:, o // 16:(o + n) // 16], n, n, 2 * W)
                        h_srcs[0] = hs_w
                        h_fix[0] = None
                        return
                    nm = (nt - 1) * 128
                    hs_w = hsrc_pool.tile([128, nt - 1, 2 * W], F16,
                                          tag=f"h{w}")
                    nc.gpsimd.dma_gather(
                        hs_w[:], gsrcA[:],
                        ix[:, o // 16:(o + nm) // 16], nm, nm, 2 * W)
                    h_srcs[w] = hs_w
                    hf_w = hsrc_pool.tile([128, 1, 2 * W], F16, tag=f"hf{w}")
                    nc.gpsimd.dma_gather(
                        hf_w[:], gsrcB[:],
                        ix[:, (o + nm) // 16:(o + nm + 128) // 16], 128, 128,
                        2 * W)
                    h_fix[w] = hf_w

                for _w0 in range(4):
                    issue_gather(_w0)
                hT_next = hT_pool.tile([W, WPC * WIN], F16)

                def write_h(w):
                    VF = VF_W[w]
                    # transpose hT_next[w] and write it to DRAM. Deferred by
                    # one window (emitted after the NEXT window's zsum
                    # passes) so the in-order PE never stalls waiting for
                    # relu(w) on ACT; the last two windows are emitted inline
                    # since their writes gate the next iteration's gathers.
                    h_ps = tr_ps_pool.tile([WIN, 64], F16, tag="tr")
                    nc.tensor.transpose(h_ps[:VF, :],
                                        hT_next[:, w * WIN:w * WIN + VF],
                                        ident[:])
                    h_sb = spool.tile([WIN, 64], F16, tag="hnew")
                    nc.scalar.copy(h_sb[:VF, :], h_ps[:VF, :])
                    if single_core:
                        if w < WPC - 1:
                            nc.sync.dma_start(
                                h_fullA[it][w * WIN:w * WIN + VF, :W],
                                h_sb[:VF, :])
                            if w == WPC - 2:
                                # B gets windows 0..wpc-2 via one bulk
                                # copy (fix gathers wait for the last
                                # window anyway, so this is off the
                                # critical path)
                                nc.sync.dma_start(
                                    h_full[it][: (WPC - 1) * WIN, :],
                                    h_fullA[it][: (WPC - 1) * WIN, :])
                        else:
                            nc.sync.dma_start(
                                h_full[it][w * WIN:w * WIN + VF, :W],
                                h_sb[:VF, :])
                    else:
                        nc.sync.dma_start(
                            h_slice[it][w * WIN:w * WIN + VF, :W],
                            h_sb[:VF, :])

                def emit_y(w):
                    VF = VF_W[w]
                    y_ps = agg_ps_pool.tile([WIN, 1], F32, tag="a")
                    nc.tensor.matmul(y_ps[:VF, :],
                                     hT_next[:, w * WIN:w * WIN + VF],
                                     fc2[:], start=True, stop=True)
                    nc.vector.tensor_add(y_sb[:VF, w: w + 1], y_ps[:VF, :],
                                         fc2b[:VF, :])

                pending_tr = []
                pending_y = []
                for w in range(WPC):
                    nt = NT_W[w]
                    t0 = TOFF[w]
                    VF = VF_W[w]
                    zs = []
                    for et in range(nt):
                        t = t0 + et
                        z = zpool.tile([128, NCH * 128], F16)
                        # all-fp16 operands with packed (1,2) last dims hit
                        # the DVE 2x perf mode (stride-0 last dims do not)
                        zv = z[:].rearrange("p (c a b) -> p c a b", c=66, b=2)
                        full = w == 0 and plan.fix0_free
                        h_t = h_srcs[w] if (full or et < nt - 1) else h_fix[w]
                        e_t = et if (full or et < nt - 1) else 0
                        hs = h_t[:, e_t, :W] \
                            .rearrange("p (a b) -> p a b", b=2) \
                            .unsqueeze(1).broadcast_to((128, 66, 32, 2))
                        e2 = e2dup[:, t * 132:(t + 1) * 132] \
                            .rearrange("p (c b) -> p c b", b=2) \
                            .unsqueeze(2).broadcast_to((128, 66, 32, 2))
                        if et == nt - 1 and w == 0 \
                                and not plan.fix0_free:
                            # window 0's fix tile sits on the iteration
                            # boundary critical path: build it in KH-chunk
                            # pieces so pass 0 can start after the first one
                            for c0 in range(0, 66, 8):
                                c1 = min(c0 + 8, 66)
                                nc.vector.tensor_mul(
                                    zv[:, c0:c1, :, :], hs[:, c0:c1, :, :],
                                    e2[:, c0:c1, :, :])
                            zs.append(z)
                            continue
                        # offload part of the first tile of each window to
                        # the (idle) GPSIMD engine; DVE builds the rest.
                        # Not in iteration 0's first windows: Pool is still
                        # busy with the initial gather burst there.
                        if et == 0 and not (it == 0 and w < 3):
                            nc.gpsimd.tensor_mul(
                                zv[:, :28, :, :], hs[:, :28, :, :],
                                e2[:, :28, :, :])
                            nc.vector.tensor_mul(
                                zv[:, 28:, :, :], hs[:, 28:, :, :],
                                e2[:, 28:, :, :])
                        else:
                            nc.vector.tensor_mul(zv, hs, e2)
                        zs.append(z)
                    if w + 4 < WPC:
                        issue_gather(w + 4)
                    if it == 0:
                        if w == 0 and WPC > 1:
                            build_e2dup_win(1)
                        if w + 2 < WPC:
                            build_e2dup_win(w + 2)
                    # zsum in KH-chunk PSUM passes (back-to-back on PE; the
                    # drains pipeline on ACT), then the T-contract block.
                    zsum_sb = zsum_sb_pool.tile([128, NCH * WIN], F16)
                    for p0 in range(0, NCH, KH):
                        p1 = min(p0 + KH, NCH)
                        zsum_ps = zsum_ps_pool.tile([128, KH * WIN], F32)
                        for k in range(p0, p1):
                            for et in range(nt):
                                nc.tensor.matmul(
                                    zsum_ps[:, (k - p0) * WIN:(k - p0 + 1) * WIN],
                                    zs[et][:, k * 128:(k + 1) * 128],
                                    segT[:, (t0 + et) * WIN:(t0 + et + 1) * WIN],
                                    start=(et == 0), stop=(et == nt - 1))
                        # keep the DVE free for z-builds (critical engine) —
                        # drain PSUM on ACT
                        nc.scalar.copy(zsum_sb[:, p0 * WIN:p1 * WIN],
                                       zsum_ps[:, :(p1 - p0) * WIN])
                    if pending_tr:
                        write_h(pending_tr.pop())
                    while len(pending_y) > 1:
                        emit_y(pending_y.pop(0))
                    agg_ps = agg_ps_pool.tile([64, WIN], F32, tag="a")
                    for k in range(NCH):
                        nc.tensor.matmul(agg_ps[:],
                                         Tsb[:, k * W:(k + 1) * W],
                                         zsum_sb[:, k * WIN:(k + 1) * WIN],
                                         start=(k == 0), stop=False)
                    nc.tensor.matmul(agg_ps[:], rootW[:],
                                     hT_cur[:, w * WIN:(w + 1) * WIN],
                                     start=False, stop=True)
                    nc.scalar.activation(hT_next[:, w * WIN:(w + 1) * WIN],
                                         agg_ps[:], Relu)
                    if it == DEP - 1:
                        pending_y.append(w)
                        if w == WPC - 1:
                            while pending_y:
                                emit_y(pending_y.pop(0))
                    else:
                        if w >= WPC - 2:
                            # w8's write gates the next iteration's main
                            # gathers, w9's its fix gathers: keep both inline
                            if pending_tr:
                                write_h(pending_tr.pop())
                            write_h(w)
                        else:
                            pending_tr.append(w)
                hT_cur = hT_next
                if it < DEP - 1 and not single_core:
                    nc.gpsimd.collective_compute(
                        "AllGather",
                        mybir.AluOpType.bypass,
                        ins=[h_slice[it][:].opt()],
                        outs=[h_full[it][:].opt()],
                        replica_groups=[list(range(NC_))],
                    )

            # ---- output ----
            y_view = y_d[:].rearrange("(w v) o -> v (w o)", w=WPC)
            nc.sync.dma_start(y_view, y_sb[:])

    nc.compile()
    return nc


def kernel(**inputs) -> np.ndarray:
    from concourse.bass_utils import run_bass_kernel_spmd

    plan = make_plan(**{k: np.asarray(v) for k, v in inputs.items()})
    nc = build_program(plan)
    core_ids = list(range(plan.n_cores))
    res = run_bass_kernel_spmd(nc, plan.in_maps, core_ids,
                               trace=bool(int(os.environ.get("KERNEL_TRACE", "0"))))
    y = np.concatenate([res.results[r]["y"] for r in range(plan.n_cores)], axis=0)
    out = y[plan.devnode]
    kernel.last_results = res
    kernel.last_plan = plan
    return out


# revision 45
# speedup vs baseline: 2.1951x; 1.0175x over previous
"""Trainium2 Bass kernel for nn_Net_MP_68805376082308 (NNConv-style GNN).

Reference computation (see problem statement):
    h = x@fc1 + b
    e2 = relu(edge_attr@k1 + b1)                     # [E, 64]
    ew = (e2 @ k2 + b2).reshape(E, 64, 64)           # never materialized here!
    for 4 iters:
        msg  = einsum('ei,eio->eo', h[src], ew)
        agg  = segment_sum(msg, dst) / max(deg,1)
        h    = relu(agg + h@root)
    out = h @ fc2 + b

Device algorithm (per core, node-sharded, dst-grouped edge slots):
    e2s[e, c]    = relu((edge_attr@k1+b1)[e,c]) * invdeg[dst[e]]  (c in 0..63)
    z[e, c*64+i] = e2s[e,c] * h[src[e], i]        # DVE, fp16 pair-trick APs
    zsumT[ci, v] = sum_e z[e,ci] * SegMat[e,v]    # PE, z stationary (scatter
                                                  #  commutes with k2 contract)
    aggT[o, v]   = T_cm.T @ zsumT + root.T @ hT   # PE (T_cm rows 4096..4159
                                                  #  carry k2_b, paired with
                                                  #  z's invdeg column)
    hT           = relu(aggT)                     # ACT
    h[src] gather via SWDGE dma_gather; h exchanged across 8 cores with an
    AllGather after each iteration.

Windows hold 128 dst-node slots and 5 edge tiles each. Edges whose SOURCE
node lies in any core's last window are segregated into each window's last
("fix") tile: the other tiles' gathers then depend only on h windows 0..8 and
overlap the last window's compute at the iteration boundary (split h_fullA/B
tensors express this to the dependency tracker in the single-core cost model;
the real 8-core path keeps one h_full fed by an AllGather).

kernel(**inputs) takes the FULL unsharded inputs and returns [10000, 1] fp32.
"""

import math
import os
import sys
from dataclasses import dataclass, field

import numpy as np

sys.path.insert(0, "/opt/trn_rl_repo")

import concourse.bacc as bacc
import concourse.bass as bass
import concourse.mybir as mybir
import concourse.tile as tile
from concourse import library_config

F32 = mybir.dt.float32
F16 = mybir.dt.float16
I16 = mybir.dt.int16

WIDTH = 64
DEPTH = 4
NCH = 33                # ci chunks of 128 (66*64/128)
KH = 8                  # chunks per PSUM pass


@dataclass
class Plan:
    """Host-side preprocessing result: all per-core device input arrays plus
    the compile-time structure constants."""

    n_cores: int
    wpc: int                 # windows per core
    nt_w: list = None        # tiles per window (same layout for every core)
    nodes_pad: int = 0
    depth: int = DEPTH
    win: int = 128           # nodes per scatter window
    devnode: np.ndarray = None     # [N] original node -> device row
    fix0_free: bool = False
    vf_w: list = None
    in_maps: list = field(default_factory=list)
    fc2_b: float = 0.0

    @property
    def ntiles(self):        # edge tiles per core
        return sum(self.nt_w)

    @property
    def epc(self):           # edge slots per core
        return self.ntiles * 128

    @property
    def tile_off(self):      # first tile index of each window
        off, out = 0, []
        for n in self.nt_w:
            out.append(off)
            off += n
        return out


def make_plan(x, edge_index, edge_attr, fc1_W, fc1_b, k1_W, k1_b, k2_W, k2_b,
              root, conv_b, fc2_W, fc2_b, n_cores=8, depth=DEPTH):
    W = WIDTH
    N = x.shape[0]
    E = edge_index.shape[1]
    src = np.asarray(edge_index[0], dtype=np.int64)
    dst = np.asarray(edge_index[1], dtype=np.int64)
    assert np.all(np.asarray(conv_b) == 0.0), "kernel assumes conv_b == 0"

    WIN = 128
    wpc = max(1, int(math.ceil(N / WIN / n_cores)))
    n_windows = n_cores * wpc
    nodes_pad = n_windows * WIN
    # per-position node caps: the zsum/T/root moving dim is the node count
    # VF_W[w] (< 128) of each window; the 4-tile window gets fewer nodes so
    # its smaller edge capacity still suffices. Total = N / n_cores.
    per_core = N // n_cores
    VF_W = [128] * wpc
    if wpc == 10:
        VF_W = [128, 128, 128, 128, 128, 128, 128, 128, 101,
                per_core - 8 * 128 - 101]
    else:
        base = per_core // wpc
        VF_W = [base] * wpc
        VF_W[-1] += per_core - base * wpc
    assert sum(VF_W) == per_core and all(0 < v <= 128 for v in VF_W)
    ncap = np.array([v for _ in range(n_cores) for v in VF_W], dtype=np.int64)

    counts = np.bincount(dst, minlength=N).astype(np.float64)
    denom = np.where(counts > 0, counts, 1.0)
    invdeg_node = (1.0 / denom).astype(np.float32)

    # Per-window edge-tile capacities: 49 tiles (6272 slots) is the minimum
    # that fits 6250 edges; the 4-tile window sits second-to-last (measured
    # best position).
    base_nt = [5, 5, 5, 5, 5, 5, 5, 5, 4, 5]
    if wpc != 10:  # generic fallback: balanced with one tiny last window
        per = int(math.ceil(E / n_cores / max(1, wpc - 1) / 128)) + 1
        base_nt = [per] * (wpc - 1) + [1]
    cap = np.array([nt * 128 for _ in range(n_cores) for nt in base_nt],
                   dtype=np.int64)

    order = np.argsort(-counts, kind="stable")
    win_edges = np.zeros(n_windows, dtype=np.int64)
    win_fill = np.zeros(n_windows, dtype=np.int64)
    node_window = np.zeros(N, dtype=np.int64)
    node_slot = np.zeros(N, dtype=np.int64)
    NEG = -(1 << 60)
    # greedy: place desc-degree nodes into the window with the most remaining
    # edge capacity that still has node slots; grow a window's capacity by a
    # tile if nothing fits.
    rem = cap.copy()
    for n in order:
        d = int(counts[n])
        w = int(np.argmax(rem))
        if rem[w] < d:
            cap[w] += 128 * int(math.ceil((d - rem[w]) / 128))
            rem[w] = cap[w] - win_edges[w]
        node_window[n] = w
        node_slot[n] = win_fill[w]
        win_fill[w] += 1
        win_edges[w] += d
        rem[w] = cap[w] - win_edges[w] if win_fill[w] < ncap[w] else NEG
    nt_all = (cap // 128).reshape(n_cores, wpc)
    # every core runs one compiled program -> shared nt layout: per-position max
    nt_w = [int(nt_all[:, i].max()) for i in range(wpc)]

    # Repair pass: make window 0 of every core free of "fix" in-edges (edges
    # sourced from any core's last window). Its whole gather then depends
    # only on h windows 0..wpc-2 and the first window of the next iteration
    # starts right at the boundary. Swap dirty w0 members with clean nodes
    # of similar degree from the same core's windows 0..wpc-2.
    lastwin = (node_window % wpc) == (wpc - 1)
    has_fix_in = np.zeros(N, dtype=bool)
    np.logical_or.at(has_fix_in, dst, lastwin[src])
    import bisect

    def try_repair(w0):
        """Swap w0's dirty nodes (fix in-edges) out; True on full success.
        Mutates node_window/node_slot/win_edges only for committed swaps."""
        r = w0 // wpc
        members = np.where(node_window == w0)[0]
        dirty = [n for n in members if has_fix_in[n]]
        cand = [n for n in np.where((node_window // wpc == r)
                                    & (node_window != w0)
                                    & ~lastwin[np.arange(N)]
                                    & ~has_fix_in)[0]]
        cand.sort(key=lambda n: counts[n])
        cdeg = [counts[n] for n in cand]
        swaps = []
        for n_out in dirty:
            d_out = counts[n_out]
            placed = False
            i0 = bisect.bisect_left(cdeg, d_out)
            for i in sorted(range(len(cand)),
                            key=lambda i: abs(i - i0)):
                n_in = cand[i]
                if n_in < 0:
                    continue
                wb = node_window[n_in]
                d_in = counts[n_in]
                if (win_edges[w0] + d_in - d_out <= cap[w0]
                        and win_edges[wb] + d_out - d_in <= cap[wb]):
                    node_window[n_out], node_window[n_in] = wb, w0
                    node_slot[n_out], node_slot[n_in] = \
                        node_slot[n_in], node_slot[n_out]
                    win_edges[w0] += d_in - d_out
                    win_edges[wb] += d_out - d_in
                    cand[i] = -1
                    swaps.append((n_out, n_in, wb))
                    placed = True
                    break
            if not placed:
                for n_out2, n_in2, wb2 in reversed(swaps):
                    node_window[n_out2], node_window[n_in2] = w0, wb2
                    node_slot[n_out2], node_slot[n_in2] = \
                        node_slot[n_in2], node_slot[n_out2]
                    win_edges[w0] += counts[n_out2] - counts[n_in2]
                    win_edges[wb2] += counts[n_in2] - counts[n_out2]
                return False
        return True

    fix0_free = True
    for r in range(n_cores):
        if try_repair(r * wpc):
            continue
        # relabel: swap position 0 with another nt=5 position whose window
        # repairs cleanly (window identity is just a label per core)
        done = False
        for alt in range(1, wpc - 1):
            if nt_w[alt] != nt_w[0]:
                continue
            wa, w0 = r * wpc + alt, r * wpc
            sel0 = node_window == w0
            sela = node_window == wa
            node_window[sel0], node_window[sela] = wa, w0
            win_edges[w0], win_edges[wa] = win_edges[wa], win_edges[w0]
            if try_repair(w0):
                done = True
                break
            sel0 = node_window == w0
            sela = node_window == wa
            node_window[sel0], node_window[sela] = wa, w0
            win_edges[w0], win_edges[wa] = win_edges[wa], win_edges[w0]
        if not done:
            fix0_free = False
    plan_fix0_free = fix0_free

    assert np.all(win_fill <= ncap)
    plan = Plan(n_cores=n_cores, wpc=wpc, nt_w=nt_w, nodes_pad=nodes_pad,
                depth=depth, win=WIN,
                fc2_b=float(np.asarray(fc2_b).reshape(())))
    plan.fix0_free = plan_fix0_free
    plan.vf_w = VF_W
    ntiles = plan.ntiles
    epc = plan.epc
    woff = [128 * t for t in plan.tile_off]   # slot offset of window in core

    plan.devnode = node_window * WIN + node_slot

    # edge -> slot within its dst window. Edges whose SOURCE lies in any
    # core's last window ("fix" edges) go to the tail of the window's slot
    # range (the last tile): the other tiles' gather then depends only on
    # h windows 0..wpc-2 and overlaps the last window's compute.
    devnode = node_window * WIN + node_slot
    edge_win = node_window[dst]
    is_fix = (devnode[src] % (wpc * WIN)) >= (wpc - 1) * WIN
    ord_e = np.argsort(edge_win, kind="stable")
    fill = np.zeros(n_windows, dtype=np.int64)
    fillb = np.zeros(n_windows, dtype=np.int64)
    eslot = np.zeros(E, dtype=np.int64)
    for e in ord_e:
        w = edge_win[e]
        core, wl = divmod(w, wpc)
        capw = nt_w[wl] * 128
        if is_fix[e]:
            fillb[w] += 1
            eslot[e] = core * epc + woff[wl] + capw - fillb[w]
        else:
            eslot[e] = core * epc + woff[wl] + fill[w]
            fill[w] += 1
    assert all(fill[w] + fillb[w] <= nt_w[w % wpc] * 128
               for w in range(n_windows))
    assert fillb.max() <= 128, "fix edges must fit the last tile"
    if plan_fix0_free:
        assert all(fillb[r * wpc] == 0 for r in range(n_cores))

    tot_slots = n_cores * epc
    slot_src = np.zeros(tot_slots, dtype=np.int64)
    slot_used = np.zeros(tot_slots, dtype=bool)
    slot_vloc = np.zeros(tot_slots, dtype=np.int64)
    slot_invdeg = np.zeros(tot_slots, dtype=np.float32)
    slot_ea = np.zeros((tot_slots, 3), dtype=np.float32)
    slot_src[eslot] = devnode[src]
    del devnode
    slot_used[eslot] = True
    slot_vloc[eslot] = node_slot[dst]
    slot_invdeg[eslot] = invdeg_node[dst]
    slot_ea[eslot] = np.asarray(edge_attr, dtype=np.float32)

    # weight repacks: T_cm [66*64, 64]: rows 0..4095 = k2_W, rows
    # 4096..4159 = k2_b (paired with z's invdeg column), rest zero.
    # chunk layout: T_sb[p, k*64+o] = T_cm[k*128+p, o]
    T_cm = np.zeros((66 * 64, W), dtype=np.float32)
    T_cm[: 64 * 64] = np.ascontiguousarray(
        np.asarray(k2_W, dtype=np.float32).reshape(64, 64, 64)
    ).reshape(64 * 64, W)
    T_cm[64 * 64 : 65 * 64] = np.asarray(k2_b, dtype=np.float32).reshape(64, 64)
    T_sb = np.ascontiguousarray(
        T_cm.reshape(NCH, 128, W).transpose(1, 0, 2)
    ).reshape(128, NCH * W).astype(np.float16)

    # k1 extended to 65 cols: 0-63 = [k1_W; k1_b], 64 = bias-row one (the
    # invdeg ACT-scale turns it into the invdeg column).
    k1_Wb = np.zeros((4, 65), dtype=np.float16)
    k1_Wb[:3, :64] = np.asarray(k1_W, dtype=np.float16)
    k1_Wb[3, :64] = np.asarray(k1_b, dtype=np.float16)
    k1_Wb[3, 64] = 1.0

    # h rows padded to 128 f16 (=256B) so SWDGE dma_gather's 256B-multiple
    # row-stride restriction is met; cols 64.. are never read by compute.
    h0 = np.zeros((nodes_pad, 2 * W), dtype=np.float16)
    h0[plan.devnode, :W] = (np.asarray(x, np.float32) @ np.asarray(fc1_W, np.float32)
                       + np.asarray(fc1_b, np.float32)).astype(np.float16)

    ident = np.eye(64, dtype=np.float16)
    root_np = np.asarray(root, dtype=np.float16)
    fc2_np = np.asarray(fc2_W, dtype=np.float16).reshape(W, 1)

    for r in range(n_cores):
        sl = slice(r * epc, (r + 1) * epc)
        c_ea = slot_ea[sl]
        c_used = slot_used[sl]
        c_invd = slot_invdeg[sl]
        c_vloc = slot_vloc[sl]
        c_src = slot_src[sl]

        eaT = np.zeros((4, epc), dtype=np.float16)
        eaT[:3] = c_ea.T.astype(np.float16)
        eaT[3] = 1.0
        # invdeg in [partition, tile] layout (slot s -> (s//128, s%128))
        invd = np.ascontiguousarray(
            c_invd.reshape(ntiles, 128).T)                       # [128, ntiles]
        tt = np.arange(epc) // 128
        pp = np.arange(epc) % 128
        segT = np.zeros((ntiles, 128, WIN), dtype=np.float16)
        segT[tt[c_used], pp[c_used], c_vloc[c_used]] = 1.0
        segT = np.ascontiguousarray(segT.transpose(1, 0, 2)).reshape(128, ntiles * WIN)

        idx = np.zeros((128, epc // 16), dtype=np.int16)
        base = c_src.astype(np.int16).reshape(epc // 16, 16).T   # [16, epc/16]
        for g in range(8):
            idx[16 * g : 16 * (g + 1)] = base

        h0T = np.ascontiguousarray(
            h0[r * wpc * WIN : (r + 1) * wpc * WIN, :W].T)       # [64, wpc*WIN]


        plan.in_maps.append({
            "eaT": eaT,
            "invdeg": invd,
            "segmatT": segT,
            "idx": idx,
            "h0": h0,
            "h0T": h0T,
            "T_sb": T_sb,
            "k1_Wb": k1_Wb,
            "root": root_np,
            "fc2_W": fc2_np,
            "fc2_b": np.full((WIN, 1), plan.fc2_b, dtype=np.float32),
            "ident": ident,
        })
    return plan


def build_program(plan: Plan, debug=False, single_core=False):
    """Build the SPMD Bass program (one program, run on all cores).

    single_core=True replaces the AllGather with direct local h_full writes
    (and drops addr_space="Shared") so the program can run under TimelineSim
    for cost modeling."""
    W = WIDTH
    WPC = plan.wpc
    WIN = plan.win
    NT_W = plan.nt_w
    TOFF = plan.tile_off
    NTILES = plan.ntiles
    EPC = plan.epc
    NPAD = plan.nodes_pad
    VF_W = plan.vf_w
    DEP = plan.depth
    NC_ = plan.n_cores
    Relu = mybir.ActivationFunctionType.Relu

    nc = bacc.Bacc("TRN2", target_bir_lowering=False, debug=debug,
                   num_devices=NC_)

    # ---- I/O ----
    eaT_d = nc.dram_tensor("eaT", [4, EPC], F16, kind="ExternalInput")
    invd_d = nc.dram_tensor("invdeg", [128, NTILES], F32, kind="ExternalInput")
    segT_d = nc.dram_tensor("segmatT", [128, NTILES * WIN], F16, kind="ExternalInput")
    idx_d = nc.dram_tensor("idx", [128, EPC // 16], I16, kind="ExternalInput")
    h0_d = nc.dram_tensor("h0", [NPAD, 2 * W], F16, kind="ExternalInput")
    h0T_d = nc.dram_tensor("h0T", [W, WPC * WIN], F16, kind="ExternalInput")
    Tsb_d = nc.dram_tensor("T_sb", [128, NCH * W], F16, kind="ExternalInput")
    k1_d = nc.dram_tensor("k1_Wb", [4, 65], F16, kind="ExternalInput")
    root_d = nc.dram_tensor("root", [W, W], F16, kind="ExternalInput")
    fc2_d = nc.dram_tensor("fc2_W", [W, 1], F16, kind="ExternalInput")
    fc2b_d = nc.dram_tensor("fc2_b", [WIN, 1], F32, kind="ExternalInput")
    id_d = nc.dram_tensor("ident", [64, 64], F16, kind="ExternalInput")
    y_d = nc.dram_tensor("y", [WPC * WIN, 1], F32, kind="ExternalOutput")

    # internal DRAM for the h exchange
    h_slice = [nc.dram_tensor(f"h_slice{i}", [WPC * WIN, 2 * W], F16)
               for i in range(DEP - 1)]
    if single_core:
        h_fullA = [nc.dram_tensor(f"h_fullA{i}", [NPAD, 2 * W], F16)
                   for i in range(DEP - 1)]
        h_full = [nc.dram_tensor(f"h_fullB{i}", [NPAD, 2 * W], F16)
                  for i in range(DEP - 1)]
    else:
        h_full = [nc.dram_tensor(f"h_full{i}", [NPAD, 2 * W], F16,
                                 addr_space="Shared")
                  for i in range(DEP - 1)]
        h_fullA = h_full

    MAXNT = max(NT_W)
    with tile.TileContext(nc) as tc:
        with (
            tc.tile_pool(name="const", bufs=1) as cpool,
            tc.tile_pool(name="hsrc", bufs=2) as hsrc_pool,
            tc.tile_pool(name="z", bufs=2 * MAXNT + 1) as zpool,
            tc.tile_pool(name="zsum_sb", bufs=2) as zsum_sb_pool,
            tc.tile_pool(name="hT", bufs=2) as hT_pool,
            tc.tile_pool(name="small", bufs=4) as spool,
            tc.tile_pool(name="zsum_ps", bufs=2, space="PSUM") as zsum_ps_pool,
            tc.tile_pool(name="agg_ps", bufs=2, space="PSUM") as agg_ps_pool,
            tc.tile_pool(name="tr_ps", bufs=1, space="PSUM") as tr_ps_pool,
        ):
            nc.gpsimd.load_library(library_config.mlp)

            # preload the ACT function table (1.3us) under the const DMAs;
            # Copy needs no bias const-AP (whose DMA would land late)
            warm = cpool.tile([1, 1], F32)
            nc.vector.memset(warm[:], 0.0)
            nc.scalar.activation(warm[:], warm[:],
                                 mybir.ActivationFunctionType.Copy)

            # ---- constants; gather-critical tensors first so window 0's
            # gather + e2 chain + first zsum start ASAP ----
            n0 = NT_W[0] * 128 // 16
            idx0 = cpool.tile([128, n0], I16)
            nc.sync.dma_start(idx0[:], idx_d[:, :n0])
            idx = cpool.tile([128, EPC // 16], I16)
            nc.sync.dma_start(idx[:], idx_d[:])
            eaT = cpool.tile([4, EPC], F16)
            nc.sync.dma_start(eaT[:], eaT_d[:])
            invd = cpool.tile([128, NTILES], F32)
            nc.sync.dma_start(invd[:], invd_d[:])
            k1 = cpool.tile([4, 65], F16)
            nc.sync.dma_start(k1[:], k1_d[:])
            segT = cpool.tile([128, NTILES * WIN], F16)
            nc.sync.dma_start(segT[:], segT_d[:])
            Tsb = cpool.tile([128, NCH * W], F16)
            nc.sync.dma_start(Tsb[:], Tsb_d[:])
            h0T = cpool.tile([W, WPC * WIN], F16)
            nc.sync.dma_start(h0T[:], h0T_d[:])
            rootW = cpool.tile([W, W], F16)
            nc.sync.dma_start(rootW[:], root_d[:])
            fc2 = cpool.tile([W, 1], F16)
            nc.sync.dma_start(fc2[:], fc2_d[:])
            fc2b = cpool.tile([WIN, 1], F32)
            nc.sync.dma_start(fc2b[:], fc2b_d[:])
            ident = cpool.tile([64, 64], F16)
            nc.sync.dma_start(ident[:], id_d[:])

            # ---- e2dup: [128, NTILES*64*2] fp16, every value twice so the
            # z-build APs end in a packed (stride 1, count 2) dim on ALL
            # operands -> DVE 2x mode. relu(x*invdeg) = invdeg*relu(x) folds
            # the scatter-mean denominator into the ACT scale. ----
            e2dup = cpool.tile([128, NTILES * 65 * 2], F16)

            def build_e2dup(t):
                e2_ps = agg_ps_pool.tile([128, 65], F32, tag="a")
                nc.tensor.matmul(e2_ps[:], eaT[:, t * 128:(t + 1) * 128],
                                 k1[:], start=True, stop=True)
                dup = e2dup[:, t * 130:(t + 1) * 130] \
                    .rearrange("p (c b) -> p c b", b=2)
                for b in range(2):
                    nc.scalar.activation(dup[:, :, b], e2_ps[:], Relu,
                                         scale=invd[:, t: t + 1])

            def build_e2dup_win(w):
                for et in range(NT_W[w]):
                    build_e2dup(TOFF[w] + et)

            # only window 0's e2dup up front: emitting all of it here would
            # queue 24us of ACT work ahead of iteration 0's PSUM drains (ACT
            # executes in order) and stall the whole pipeline; z(w) also
            # waits on every e2dup write emitted before it (tile-granular
            # dependency tracking), so later windows' builds are staggered
            # through iteration 0.
            build_e2dup_win(0)

            hT_cur = h0T
            y_sb = spool.tile([WIN, WPC], F32, tag="y")

            for it in range(DEP):
                gsrcA = h0_d if it == 0 else h_fullA[it - 1]
                gsrcB = h0_d if it == 0 else h_full[it - 1]
                # Two gathers per window into its own tiles: the MAIN gather
                # (tiles 0..nt-2, whose edges by construction have sources in
                # windows 0..wpc-2) runs as soon as those h windows land and
                # overlaps the last window's compute; only the small FIX
                # gather (last tile) waits for the final h window.
                h_srcs = [None] * WPC
                h_fix = [None] * WPC

                def issue_gather(w):
                    nt = NT_W[w]
                    o = TOFF[w] * 128
                    ix = idx0 if (w == 0 and it == 0) else idx
                    if w == 0 and plan.fix0_free:
                        # window 0 has no fix edges: gather whole window from
                        # A, in two calls so its first z-builds start as soon
                        # as the first tiles land at the iteration boundary
                        hs_w = hsrc_pool.tile([128, nt, 2 * W], F16, tag="h0f")
                        n = nt * 128
                        nc.gpsimd.dma_gather(
                            hs_w[:], gsrcA[:],
                            ix[:, o // 16:(o + n) // 16], n, n, 2 * W)
                        h_srcs[0] = hs_w
                        h_fix[0] = None
                        return
                    nm = (nt - 1) * 128
                    hs_w = hsrc_pool.tile([128, nt - 1, 2 * W], F16,
                                          tag=f"h{w}")
                    nc.gpsimd.dma_gather(
                        hs_w[:], gsrcA[:],
                        ix[:, o // 16:(o + nm) // 16], nm, nm, 2 * W)
                    h_srcs[w] = hs_w
                    hf_w = hsrc_pool.tile([128, 1, 2 * W], F16, tag=f"hf{w}")
                    nc.gpsimd.dma_gather(
                        hf_w[:], gsrcB[:],
                        ix[:, (o + nm) // 16:(o + nm + 128) // 16], 128, 128,
                        2 * W)
                    h_fix[w] = hf_w

                for _w0 in range(4):
                    issue_gather(_w0)
                hT_next = hT_pool.tile([W, WPC * WIN], F16)

                def write_h(w):
                    VF = VF_W[w]
                    # transpose hT_next[w] and write it to DRAM. Deferred by
                    # one window (emitted after the NEXT window's zsum
                    # passes) so the in-order PE never stalls waiting for
                    # relu(w) on ACT; the last two windows are emitted inline
                    # since their writes gate the next iteration's gathers.
                    h_ps = tr_ps_pool.tile([WIN, 64], F16, tag="tr")
                    nc.tensor.transpose(h_ps[:VF, :],
                                        hT_next[:, w * WIN:w * WIN + VF],
                                        ident[:])
                    h_sb = spool.tile([WIN, 64], F16, tag="hnew")
                    nc.scalar.copy(h_sb[:VF, :], h_ps[:VF, :])
                    if single_core:
                        if w < WPC - 1:
                            nc.sync.dma_start(
                                h_fullA[it][w * WIN:w * WIN + VF, :W],
                                h_sb[:VF, :])
                            if w == WPC - 2:
                                # B gets windows 0..wpc-2 via one bulk
                                # copy (fix gathers wait for the last
                                # window anyway, so this is off the
                                # critical path)
                                nc.sync.dma_start(
                                    h_full[it][: (WPC - 1) * WIN, :],
                                    h_fullA[it][: (WPC - 1) * WIN, :])
                        else:
                            nc.sync.dma_start(
                                h_full[it][w * WIN:w * WIN + VF, :W],
                                h_sb[:VF, :])
                    else:
                        nc.sync.dma_start(
                            h_slice[it][w * WIN:w * WIN + VF, :W],
                            h_sb[:VF, :])

                def emit_y(w):
                    VF = VF_W[w]
                    y_ps = agg_ps_pool.tile([WIN, 1], F32, tag="a")
                    nc.tensor.matmul(y_ps[:VF, :],
                                     hT_next[:, w * WIN:w * WIN + VF],
                                     fc2[:], start=True, stop=True)
                    nc.vector.tensor_add(y_sb[:VF, w: w + 1], y_ps[:VF, :],
                                         fc2b[:VF, :])

                pending_tr = []
                pending_y = []
                for w in range(WPC):
                    nt = NT_W[w]
                    t0 = TOFF[w]
                    VF = VF_W[w]
                    zs = []
                    for et in range(nt):
                        t = t0 + et
                        z = zpool.tile([128, 65 * 64], F16)
                        # all-fp16 operands with packed (1,2) last dims hit
                        # the DVE 2x perf mode (stride-0 last dims do not)
                        zv = z[:].rearrange("p (c a b) -> p c a b", c=65, b=2)
                        full = w == 0 and plan.fix0_free
                        h_t = h_srcs[w] if (full or et < nt - 1) else h_fix[w]
                        e_t = et if (full or et < nt - 1) else 0
                        hs = h_t[:, e_t, :W] \
                            .rearrange("p (a b) -> p a b", b=2) \
                            .unsqueeze(1).broadcast_to((128, 65, 32, 2))
                        e2 = e2dup[:, t * 130:(t + 1) * 130] \
                            .rearrange("p (c b) -> p c b", b=2) \
                            .unsqueeze(2).broadcast_to((128, 65, 32, 2))
                        if et == nt - 1 and w == 0 \
                                and not plan.fix0_free:
                            # window 0's fix tile sits on the iteration
                            # boundary critical path: build it in KH-chunk
                            # pieces so pass 0 can start after the first one
                            for c0 in range(0, 65, 8):
                                c1 = min(c0 + 8, 65)
                                nc.vector.tensor_mul(
                                    zv[:, c0:c1, :, :], hs[:, c0:c1, :, :],
                                    e2[:, c0:c1, :, :])
                            zs.append(z)
                            continue
                        # offload part of the first tile of each window to
                        # the (idle) GPSIMD engine; DVE builds the rest.
                        # Not in iteration 0's first windows: Pool is still
                        # busy with the initial gather burst there.
                        if et == 0 and not (it == 0 and w < 3):
                            nc.gpsimd.tensor_mul(
                                zv[:, :28, :, :], hs[:, :28, :, :],
                                e2[:, :28, :, :])
                            nc.vector.tensor_mul(
                                zv[:, 28:, :, :], hs[:, 28:, :, :],
                                e2[:, 28:, :, :])
                        else:
                            nc.vector.tensor_mul(zv, hs, e2)
                        zs.append(z)
                    if w + 4 < WPC:
                        issue_gather(w + 4)
                    if it == 0:
                        if w == 0 and WPC > 1:
                            build_e2dup_win(1)
                        if w + 2 < WPC:
                            build_e2dup_win(w + 2)
                    # zsum in KH-chunk PSUM passes (back-to-back on PE; the
                    # drains pipeline on ACT), then the T-contract block.
                    # moving dim = VF (window's real node count); the last ci
                    # chunk is 64 rows (z is 65*64 wide).
                    zsum_sb = zsum_sb_pool.tile([128, NCH * VF], F16)
                    for p0 in range(0, NCH, KH):
                        p1 = min(p0 + KH, NCH)
                        zsum_ps = zsum_ps_pool.tile([128, KH * VF], F32)
                        for k in range(p0, p1):
                            cw = min(128, 65 * 64 - k * 128)
                            for et in range(nt):
                                nc.tensor.matmul(
                                    zsum_ps[:cw, (k - p0) * VF:
                                            (k - p0 + 1) * VF],
                                    zs[et][:, k * 128:k * 128 + cw],
                                    segT[:, (t0 + et) * WIN:
                                         (t0 + et) * WIN + VF],
                                    start=(et == 0), stop=(et == nt - 1))
                        # keep the DVE free for z-builds (critical engine) —
                        # drain PSUM on ACT
                        nc.scalar.copy(zsum_sb[:, p0 * VF:p1 * VF],
                                       zsum_ps[:, :(p1 - p0) * VF])
                    if pending_tr:
                        write_h(pending_tr.pop())
                    while len(pending_y) > 1:
                        emit_y(pending_y.pop(0))
                    agg_ps = agg_ps_pool.tile([64, VF], F32, tag="a")
                    for k in range(NCH):
                        cw = min(128, 65 * 64 - k * 128)
                        nc.tensor.matmul(agg_ps[:],
                                         Tsb[:cw, k * W:(k + 1) * W],
                                         zsum_sb[:cw, k * VF:(k + 1) * VF],
                                         start=(k == 0), stop=False)
                    nc.tensor.matmul(agg_ps[:], rootW[:],
                                     hT_cur[:, w * WIN:w * WIN + VF],
                                     start=False, stop=True)
                    nc.scalar.activation(hT_next[:, w * WIN:w * WIN + VF],
                                         agg_ps[:], Relu)
                    if it == DEP - 1:
                        pending_y.append(w)
                        if w == WPC - 1:
                            while pending_y:
                                emit_y(pending_y.pop(0))
                    else:
                        if w >= WPC - 2:
                            # w8's write gates the next iteration's main
                            # gathers, w9's its fix gathers: keep both inline
                            if pending_tr:
                                write_h(pending_tr.pop())
                            write_h(w)
                        else:
                            pending_tr.append(w)
                hT_cur = hT_next
                if it < DEP - 1 and not single_core:
                    nc.gpsimd.collective_compute(
                        "AllGather",
                        mybir.AluOpType.bypass,
                        ins=[h_slice[it][:].opt()],
                        outs=[h_full[it][:].opt()],
                        replica_groups=[list(range(NC_))],
                    )

            # ---- output ----
            y_view = y_d[:].rearrange("(w v) o -> v (w o)", w=WPC)
            nc.sync.dma_start(y_view, y_sb[:])

    nc.compile()
    return nc


def kernel(**inputs) -> np.ndarray:
    from concourse.bass_utils import run_bass_kernel_spmd

    plan = make_plan(**{k: np.asarray(v) for k, v in inputs.items()})
    nc = build_program(plan)
    core_ids = list(range(plan.n_cores))
    res = run_bass_kernel_spmd(nc, plan.in_maps, core_ids,
                               trace=bool(int(os.environ.get("KERNEL_TRACE", "0"))))
    y = np.concatenate([res.results[r]["y"] for r in range(plan.n_cores)], axis=0)
    out = y[plan.devnode]
    kernel.last_results = res
    kernel.last_plan = plan
    return out


# revision 46
# speedup vs baseline: 2.1955x; 1.0002x over previous
"""Trainium2 Bass kernel for nn_Net_MP_68805376082308 (NNConv-style GNN).

Reference computation (see problem statement):
    h = x@fc1 + b
    e2 = relu(edge_attr@k1 + b1)                     # [E, 64]
    ew = (e2 @ k2 + b2).reshape(E, 64, 64)           # never materialized here!
    for 4 iters:
        msg  = einsum('ei,eio->eo', h[src], ew)
        agg  = segment_sum(msg, dst) / max(deg,1)
        h    = relu(agg + h@root)
    out = h @ fc2 + b

Device algorithm (per core, node-sharded, dst-grouped edge slots):
    e2s[e, c]    = relu((edge_attr@k1+b1)[e,c]) * invdeg[dst[e]]  (c in 0..63)
    z[e, c*64+i] = e2s[e,c] * h[src[e], i]        # DVE, fp16 pair-trick APs
    zsumT[ci, v] = sum_e z[e,ci] * SegMat[e,v]    # PE, z stationary (scatter
                                                  #  commutes with k2 contract)
    aggT[o, v]   = T_cm.T @ zsumT + root.T @ hT   # PE (T_cm rows 4096..4159
                                                  #  carry k2_b, paired with
                                                  #  z's invdeg column)
    hT           = relu(aggT)                     # ACT
    h[src] gather via SWDGE dma_gather; h exchanged across 8 cores with an
    AllGather after each iteration.

Windows hold 128 dst-node slots and 5 edge tiles each. Edges whose SOURCE
node lies in any core's last window are segregated into each window's last
("fix") tile: the other tiles' gathers then depend only on h windows 0..8 and
overlap the last window's compute at the iteration boundary (split h_fullA/B
tensors express this to the dependency tracker in the single-core cost model;
the real 8-core path keeps one h_full fed by an AllGather).

kernel(**inputs) takes the FULL unsharded inputs and returns [10000, 1] fp32.
"""

import math
import os
import sys
from dataclasses import dataclass, field

import numpy as np

sys.path.insert(0, "/opt/trn_rl_repo")

import concourse.bacc as bacc
import concourse.bass as bass
import concourse.mybir as mybir
import concourse.tile as tile
from concourse import library_config

F32 = mybir.dt.float32
F16 = mybir.dt.float16
I16 = mybir.dt.int16

WIDTH = 64
DEPTH = 4
NCH = 33                # ci chunks of 128 (66*64/128)
KH = 8                  # chunks per PSUM pass


@dataclass
class Plan:
    """Host-side preprocessing result: all per-core device input arrays plus
    the compile-time structure constants."""

    n_cores: int
    wpc: int                 # windows per core
    nt_w: list = None        # tiles per window (same layout for every core)
    nodes_pad: int = 0
    depth: int = DEPTH
    win: int = 128           # nodes per scatter window
    devnode: np.ndarray = None     # [N] original node -> device row
    fix0_free: bool = False
    vf_w: list = None
    in_maps: list = field(default_factory=list)
    fc2_b: float = 0.0

    @property
    def ntiles(self):        # edge tiles per core
        return sum(self.nt_w)

    @property
    def epc(self):           # edge slots per core
        return self.ntiles * 128

    @property
    def tile_off(self):      # first tile index of each window
        off, out = 0, []
        for n in self.nt_w:
            out.append(off)
            off += n
        return out


def make_plan(x, edge_index, edge_attr, fc1_W, fc1_b, k1_W, k1_b, k2_W, k2_b,
              root, conv_b, fc2_W, fc2_b, n_cores=8, depth=DEPTH):
    W = WIDTH
    N = x.shape[0]
    E = edge_index.shape[1]
    src = np.asarray(edge_index[0], dtype=np.int64)
    dst = np.asarray(edge_index[1], dtype=np.int64)
    assert np.all(np.asarray(conv_b) == 0.0), "kernel assumes conv_b == 0"

    WIN = 128
    wpc = max(1, int(math.ceil(N / WIN / n_cores)))
    n_windows = n_cores * wpc
    nodes_pad = n_windows * WIN
    # per-position node caps: the zsum/T/root moving dim is the node count
    # VF_W[w] (< 128) of each window; the 4-tile window gets fewer nodes so
    # its smaller edge capacity still suffices. Total = N / n_cores.
    per_core = N // n_cores
    VF_W = [128] * wpc
    if wpc == 10:
        VF_W = [128, 128, 128, 128, 128, 128, 128, 128, 101,
                per_core - 8 * 128 - 101]
    else:
        base = per_core // wpc
        VF_W = [base] * wpc
        VF_W[-1] += per_core - base * wpc
    assert sum(VF_W) == per_core and all(0 < v <= 128 for v in VF_W)
    ncap = np.array([v for _ in range(n_cores) for v in VF_W], dtype=np.int64)

    counts = np.bincount(dst, minlength=N).astype(np.float64)
    denom = np.where(counts > 0, counts, 1.0)
    invdeg_node = (1.0 / denom).astype(np.float32)

    # Per-window edge-tile capacities: 49 tiles (6272 slots) is the minimum
    # that fits 6250 edges; the 4-tile window sits second-to-last (measured
    # best position).
    base_nt = [5, 5, 5, 5, 5, 5, 5, 5, 4, 5]
    if wpc != 10:  # generic fallback: balanced with one tiny last window
        per = int(math.ceil(E / n_cores / max(1, wpc - 1) / 128)) + 1
        base_nt = [per] * (wpc - 1) + [1]
    cap = np.array([nt * 128 for _ in range(n_cores) for nt in base_nt],
                   dtype=np.int64)

    order = np.argsort(-counts, kind="stable")
    win_edges = np.zeros(n_windows, dtype=np.int64)
    win_fill = np.zeros(n_windows, dtype=np.int64)
    node_window = np.zeros(N, dtype=np.int64)
    node_slot = np.zeros(N, dtype=np.int64)
    NEG = -(1 << 60)
    # greedy: place desc-degree nodes into the window with the most remaining
    # edge capacity that still has node slots; grow a window's capacity by a
    # tile if nothing fits.
    rem = cap.copy()
    for n in order:
        d = int(counts[n])
        w = int(np.argmax(rem))
        if rem[w] < d:
            cap[w] += 128 * int(math.ceil((d - rem[w]) / 128))
            rem[w] = cap[w] - win_edges[w]
        node_window[n] = w
        node_slot[n] = win_fill[w]
        win_fill[w] += 1
        win_edges[w] += d
        rem[w] = cap[w] - win_edges[w] if win_fill[w] < ncap[w] else NEG
    nt_all = (cap // 128).reshape(n_cores, wpc)
    # every core runs one compiled program -> shared nt layout: per-position max
    nt_w = [int(nt_all[:, i].max()) for i in range(wpc)]

    # Repair pass: make window 0 of every core free of "fix" in-edges (edges
    # sourced from any core's last window). Its whole gather then depends
    # only on h windows 0..wpc-2 and the first window of the next iteration
    # starts right at the boundary. Swap dirty w0 members with clean nodes
    # of similar degree from the same core's windows 0..wpc-2.
    lastwin = (node_window % wpc) == (wpc - 1)
    has_fix_in = np.zeros(N, dtype=bool)
    np.logical_or.at(has_fix_in, dst, lastwin[src])
    import bisect

    def try_repair(w0):
        """Swap w0's dirty nodes (fix in-edges) out; True on full success.
        Mutates node_window/node_slot/win_edges only for committed swaps."""
        r = w0 // wpc
        members = np.where(node_window == w0)[0]
        dirty = [n for n in members if has_fix_in[n]]
        cand = [n for n in np.where((node_window // wpc == r)
                                    & (node_window != w0)
                                    & ~lastwin[np.arange(N)]
                                    & ~has_fix_in)[0]]
        cand.sort(key=lambda n: counts[n])
        cdeg = [counts[n] for n in cand]
        swaps = []
        for n_out in dirty:
            d_out = counts[n_out]
            placed = False
            i0 = bisect.bisect_left(cdeg, d_out)
            for i in sorted(range(len(cand)),
                            key=lambda i: abs(i - i0)):
                n_in = cand[i]
                if n_in < 0:
                    continue
                wb = node_window[n_in]
                d_in = counts[n_in]
                if (win_edges[w0] + d_in - d_out <= cap[w0]
                        and win_edges[wb] + d_out - d_in <= cap[wb]):
                    node_window[n_out], node_window[n_in] = wb, w0
                    node_slot[n_out], node_slot[n_in] = \
                        node_slot[n_in], node_slot[n_out]
                    win_edges[w0] += d_in - d_out
                    win_edges[wb] += d_out - d_in
                    cand[i] = -1
                    swaps.append((n_out, n_in, wb))
                    placed = True
                    break
            if not placed:
                for n_out2, n_in2, wb2 in reversed(swaps):
                    node_window[n_out2], node_window[n_in2] = w0, wb2
                    node_slot[n_out2], node_slot[n_in2] = \
                        node_slot[n_in2], node_slot[n_out2]
                    win_edges[w0] += counts[n_out2] - counts[n_in2]
                    win_edges[wb2] += counts[n_in2] - counts[n_out2]
                return False
        return True

    fix0_free = True
    for r in range(n_cores):
        if try_repair(r * wpc):
            continue
        # relabel: swap position 0 with another nt=5 position whose window
        # repairs cleanly (window identity is just a label per core)
        done = False
        for alt in range(1, wpc - 1):
            if nt_w[alt] != nt_w[0]:
                continue
            wa, w0 = r * wpc + alt, r * wpc
            sel0 = node_window == w0
            sela = node_window == wa
            node_window[sel0], node_window[sela] = wa, w0
            win_edges[w0], win_edges[wa] = win_edges[wa], win_edges[w0]
            if try_repair(w0):
                done = True
                break
            sel0 = node_window == w0
            sela = node_window == wa
            node_window[sel0], node_window[sela] = wa, w0
            win_edges[w0], win_edges[wa] = win_edges[wa], win_edges[w0]
        if not done:
            fix0_free = False
    plan_fix0_free = fix0_free

    assert np.all(win_fill <= ncap)
    plan = Plan(n_cores=n_cores, wpc=wpc, nt_w=nt_w, nodes_pad=nodes_pad,
                depth=depth, win=WIN,
                fc2_b=float(np.asarray(fc2_b).reshape(())))
    plan.fix0_free = plan_fix0_free
    plan.vf_w = VF_W
    ntiles = plan.ntiles
    epc = plan.epc
    woff = [128 * t for t in plan.tile_off]   # slot offset of window in core

    plan.devnode = node_window * WIN + node_slot

    # edge -> slot within its dst window. Edges whose SOURCE lies in any
    # core's last window ("fix" edges) go to the tail of the window's slot
    # range (the last tile): the other tiles' gather then depends only on
    # h windows 0..wpc-2 and overlaps the last window's compute.
    devnode = node_window * WIN + node_slot
    edge_win = node_window[dst]
    is_fix = (devnode[src] % (wpc * WIN)) >= (wpc - 1) * WIN
    ord_e = np.argsort(edge_win, kind="stable")
    fill = np.zeros(n_windows, dtype=np.int64)
    fillb = np.zeros(n_windows, dtype=np.int64)
    eslot = np.zeros(E, dtype=np.int64)
    for e in ord_e:
        w = edge_win[e]
        core, wl = divmod(w, wpc)
        capw = nt_w[wl] * 128
        if is_fix[e]:
            fillb[w] += 1
            eslot[e] = core * epc + woff[wl] + capw - fillb[w]
        else:
            eslot[e] = core * epc + woff[wl] + fill[w]
            fill[w] += 1
    assert all(fill[w] + fillb[w] <= nt_w[w % wpc] * 128
               for w in range(n_windows))
    assert fillb.max() <= 128, "fix edges must fit the last tile"
    if plan_fix0_free:
        assert all(fillb[r * wpc] == 0 for r in range(n_cores))

    tot_slots = n_cores * epc
    slot_src = np.zeros(tot_slots, dtype=np.int64)
    slot_used = np.zeros(tot_slots, dtype=bool)
    slot_vloc = np.zeros(tot_slots, dtype=np.int64)
    slot_invdeg = np.zeros(tot_slots, dtype=np.float32)
    slot_ea = np.zeros((tot_slots, 3), dtype=np.float32)
    slot_src[eslot] = devnode[src]
    del devnode
    slot_used[eslot] = True
    slot_vloc[eslot] = node_slot[dst]
    slot_invdeg[eslot] = invdeg_node[dst]
    slot_ea[eslot] = np.asarray(edge_attr, dtype=np.float32)

    # weight repacks: T_cm [66*64, 64]: rows 0..4095 = k2_W, rows
    # 4096..4159 = k2_b (paired with z's invdeg column), rest zero.
    # chunk layout: T_sb[p, k*64+o] = T_cm[k*128+p, o]
    T_cm = np.zeros((66 * 64, W), dtype=np.float32)
    T_cm[: 64 * 64] = np.ascontiguousarray(
        np.asarray(k2_W, dtype=np.float32).reshape(64, 64, 64)
    ).reshape(64 * 64, W)
    T_cm[64 * 64 : 65 * 64] = np.asarray(k2_b, dtype=np.float32).reshape(64, 64)
    T_sb = np.ascontiguousarray(
        T_cm.reshape(NCH, 128, W).transpose(1, 0, 2)
    ).reshape(128, NCH * W).astype(np.float16)

    # k1 extended to 65 cols: 0-63 = [k1_W; k1_b], 64 = bias-row one (the
    # invdeg ACT-scale turns it into the invdeg column).
    k1_Wb = np.zeros((4, 65), dtype=np.float16)
    k1_Wb[:3, :64] = np.asarray(k1_W, dtype=np.float16)
    k1_Wb[3, :64] = np.asarray(k1_b, dtype=np.float16)
    k1_Wb[3, 64] = 1.0

    # h rows padded to 128 f16 (=256B) so SWDGE dma_gather's 256B-multiple
    # row-stride restriction is met; cols 64.. are never read by compute.
    h0 = np.zeros((nodes_pad, 2 * W), dtype=np.float16)
    h0[plan.devnode, :W] = (np.asarray(x, np.float32) @ np.asarray(fc1_W, np.float32)
                       + np.asarray(fc1_b, np.float32)).astype(np.float16)

    ident = np.eye(64, dtype=np.float16)
    root_np = np.asarray(root, dtype=np.float16)
    fc2_np = np.asarray(fc2_W, dtype=np.float16).reshape(W, 1)

    for r in range(n_cores):
        sl = slice(r * epc, (r + 1) * epc)
        c_ea = slot_ea[sl]
        c_used = slot_used[sl]
        c_invd = slot_invdeg[sl]
        c_vloc = slot_vloc[sl]
        c_src = slot_src[sl]

        eaT = np.zeros((4, epc), dtype=np.float16)
        eaT[:3] = c_ea.T.astype(np.float16)
        eaT[3] = 1.0
        # invdeg in [partition, tile] layout (slot s -> (s//128, s%128))
        invd = np.ascontiguousarray(
            c_invd.reshape(ntiles, 128).T)                       # [128, ntiles]
        tt = np.arange(epc) // 128
        pp = np.arange(epc) % 128
        segT = np.zeros((ntiles, 128, WIN), dtype=np.float16)
        segT[tt[c_used], pp[c_used], c_vloc[c_used]] = 1.0
        segT = np.ascontiguousarray(segT.transpose(1, 0, 2)).reshape(128, ntiles * WIN)

        idx = np.zeros((128, epc // 16), dtype=np.int16)
        base = c_src.astype(np.int16).reshape(epc // 16, 16).T   # [16, epc/16]
        for g in range(8):
            idx[16 * g : 16 * (g + 1)] = base

        h0T = np.ascontiguousarray(
            h0[r * wpc * WIN : (r + 1) * wpc * WIN, :W].T)       # [64, wpc*WIN]


        plan.in_maps.append({
            "eaT": eaT,
            "invdeg": invd,
            "segmatT": segT,
            "idx": idx,
            "h0": h0,
            "h0T": h0T,
            "T_sb": T_sb,
            "k1_Wb": k1_Wb,
            "root": root_np,
            "fc2_W": fc2_np,
            "fc2_b": np.full((WIN, 1), plan.fc2_b, dtype=np.float32),
            "ident": ident,
        })
    return plan


def build_program(plan: Plan, debug=False, single_core=False):
    """Build the SPMD Bass program (one program, run on all cores).

    single_core=True replaces the AllGather with direct local h_full writes
    (and drops addr_space="Shared") so the program can run under TimelineSim
    for cost modeling."""
    W = WIDTH
    WPC = plan.wpc
    WIN = plan.win
    NT_W = plan.nt_w
    TOFF = plan.tile_off
    NTILES = plan.ntiles
    EPC = plan.epc
    NPAD = plan.nodes_pad
    VF_W = plan.vf_w
    DEP = plan.depth
    NC_ = plan.n_cores
    Relu = mybir.ActivationFunctionType.Relu

    nc = bacc.Bacc("TRN2", target_bir_lowering=False, debug=debug,
                   num_devices=NC_)

    # ---- I/O ----
    eaT_d = nc.dram_tensor("eaT", [4, EPC], F16, kind="ExternalInput")
    invd_d = nc.dram_tensor("invdeg", [128, NTILES], F32, kind="ExternalInput")
    segT_d = nc.dram_tensor("segmatT", [128, NTILES * WIN], F16, kind="ExternalInput")
    idx_d = nc.dram_tensor("idx", [128, EPC // 16], I16, kind="ExternalInput")
    h0_d = nc.dram_tensor("h0", [NPAD, 2 * W], F16, kind="ExternalInput")
    h0T_d = nc.dram_tensor("h0T", [W, WPC * WIN], F16, kind="ExternalInput")
    Tsb_d = nc.dram_tensor("T_sb", [128, NCH * W], F16, kind="ExternalInput")
    k1_d = nc.dram_tensor("k1_Wb", [4, 65], F16, kind="ExternalInput")
    root_d = nc.dram_tensor("root", [W, W], F16, kind="ExternalInput")
    fc2_d = nc.dram_tensor("fc2_W", [W, 1], F16, kind="ExternalInput")
    fc2b_d = nc.dram_tensor("fc2_b", [WIN, 1], F32, kind="ExternalInput")
    id_d = nc.dram_tensor("ident", [64, 64], F16, kind="ExternalInput")
    y_d = nc.dram_tensor("y", [WPC * WIN, 1], F32, kind="ExternalOutput")

    # internal DRAM for the h exchange
    h_slice = [nc.dram_tensor(f"h_slice{i}", [WPC * WIN, 2 * W], F16)
               for i in range(DEP - 1)]
    if single_core:
        h_fullA = [nc.dram_tensor(f"h_fullA{i}", [NPAD, 2 * W], F16)
                   for i in range(DEP - 1)]
        h_full = [nc.dram_tensor(f"h_fullB{i}", [NPAD, 2 * W], F16)
                  for i in range(DEP - 1)]
    else:
        h_full = [nc.dram_tensor(f"h_full{i}", [NPAD, 2 * W], F16,
                                 addr_space="Shared")
                  for i in range(DEP - 1)]
        h_fullA = h_full

    MAXNT = max(NT_W)
    with tile.TileContext(nc) as tc:
        with (
            tc.tile_pool(name="const", bufs=1) as cpool,
            tc.tile_pool(name="hsrc", bufs=2) as hsrc_pool,
            tc.tile_pool(name="z", bufs=2 * MAXNT + 1) as zpool,
            tc.tile_pool(name="zsum_sb", bufs=2) as zsum_sb_pool,
            tc.tile_pool(name="hT", bufs=2) as hT_pool,
            tc.tile_pool(name="small", bufs=4) as spool,
            tc.tile_pool(name="zsum_ps", bufs=2, space="PSUM") as zsum_ps_pool,
            tc.tile_pool(name="agg_ps", bufs=2, space="PSUM") as agg_ps_pool,
            tc.tile_pool(name="tr_ps", bufs=1, space="PSUM") as tr_ps_pool,
        ):
            nc.gpsimd.load_library(library_config.mlp)

            # preload the ACT function table (1.3us) under the const DMAs;
            # Copy needs no bias const-AP (whose DMA would land late)
            warm = cpool.tile([1, 1], F32)
            nc.vector.memset(warm[:], 0.0)
            nc.scalar.activation(warm[:], warm[:],
                                 mybir.ActivationFunctionType.Copy)

            # ---- constants; gather-critical tensors first so window 0's
            # gather + e2 chain + first zsum start ASAP ----
            n0 = NT_W[0] * 128 // 16
            idx0 = cpool.tile([128, n0], I16)
            nc.sync.dma_start(idx0[:], idx_d[:, :n0])
            idx = cpool.tile([128, EPC // 16], I16)
            nc.sync.dma_start(idx[:], idx_d[:])
            eaT = cpool.tile([4, EPC], F16)
            nc.sync.dma_start(eaT[:], eaT_d[:])
            invd = cpool.tile([128, NTILES], F32)
            nc.sync.dma_start(invd[:], invd_d[:])
            k1 = cpool.tile([4, 65], F16)
            nc.sync.dma_start(k1[:], k1_d[:])
            segT = cpool.tile([128, NTILES * WIN], F16)
            nc.sync.dma_start(segT[:], segT_d[:])
            Tsb = cpool.tile([128, NCH * W], F16)
            nc.sync.dma_start(Tsb[:], Tsb_d[:])
            h0T = cpool.tile([W, WPC * WIN], F16)
            nc.sync.dma_start(h0T[:], h0T_d[:])
            rootW = cpool.tile([W, W], F16)
            nc.sync.dma_start(rootW[:], root_d[:])
            fc2 = cpool.tile([W, 1], F16)
            nc.sync.dma_start(fc2[:], fc2_d[:])
            fc2b = cpool.tile([WIN, 1], F32)
            nc.sync.dma_start(fc2b[:], fc2b_d[:])
            ident = cpool.tile([64, 64], F16)
            nc.sync.dma_start(ident[:], id_d[:])

            # ---- e2dup: [128, NTILES*64*2] fp16, every value twice so the
            # z-build APs end in a packed (stride 1, count 2) dim on ALL
            # operands -> DVE 2x mode. relu(x*invdeg) = invdeg*relu(x) folds
            # the scatter-mean denominator into the ACT scale. ----
            e2dup = cpool.tile([128, NTILES * 65 * 2], F16)

            def build_e2dup(t):
                e2_ps = agg_ps_pool.tile([128, 65], F32, tag="a")
                nc.tensor.matmul(e2_ps[:], eaT[:, t * 128:(t + 1) * 128],
                                 k1[:], start=True, stop=True)
                dup = e2dup[:, t * 130:(t + 1) * 130] \
                    .rearrange("p (c b) -> p c b", b=2)
                for b in range(2):
                    nc.scalar.activation(dup[:, :, b], e2_ps[:], Relu,
                                         scale=invd[:, t: t + 1])

            def build_e2dup_win(w):
                for et in range(NT_W[w]):
                    build_e2dup(TOFF[w] + et)

            # only window 0's e2dup up front: emitting all of it here would
            # queue 24us of ACT work ahead of iteration 0's PSUM drains (ACT
            # executes in order) and stall the whole pipeline; z(w) also
            # waits on every e2dup write emitted before it (tile-granular
            # dependency tracking), so later windows' builds are staggered
            # through iteration 0.
            build_e2dup_win(0)

            hT_cur = h0T
            y_sb = spool.tile([WIN, WPC], F32, tag="y")

            for it in range(DEP):
                gsrcA = h0_d if it == 0 else h_fullA[it - 1]
                gsrcB = h0_d if it == 0 else h_full[it - 1]
                # Two gathers per window into its own tiles: the MAIN gather
                # (tiles 0..nt-2, whose edges by construction have sources in
                # windows 0..wpc-2) runs as soon as those h windows land and
                # overlaps the last window's compute; only the small FIX
                # gather (last tile) waits for the final h window.
                h_srcs = [None] * WPC
                h_fix = [None] * WPC

                def issue_gather(w):
                    nt = NT_W[w]
                    o = TOFF[w] * 128
                    ix = idx0 if (w == 0 and it == 0) else idx
                    if w == 0 and plan.fix0_free:
                        # window 0 has no fix edges: gather whole window from
                        # A, in two calls so its first z-builds start as soon
                        # as the first tiles land at the iteration boundary
                        hs_w = hsrc_pool.tile([128, nt, 2 * W], F16, tag="h0f")
                        n = nt * 128
                        nc.gpsimd.dma_gather(
                            hs_w[:], gsrcA[:],
                            ix[:, o // 16:(o + n) // 16], n, n, 2 * W)
                        h_srcs[0] = hs_w
                        h_fix[0] = None
                        return
                    nm = (nt - 1) * 128
                    hs_w = hsrc_pool.tile([128, nt - 1, 2 * W], F16,
                                          tag=f"h{w}")
                    nc.gpsimd.dma_gather(
                        hs_w[:], gsrcA[:],
                        ix[:, o // 16:(o + nm) // 16], nm, nm, 2 * W)
                    h_srcs[w] = hs_w
                    hf_w = hsrc_pool.tile([128, 1, 2 * W], F16, tag=f"hf{w}")
                    nc.gpsimd.dma_gather(
                        hf_w[:], gsrcB[:],
                        ix[:, (o + nm) // 16:(o + nm + 128) // 16], 128, 128,
                        2 * W)
                    h_fix[w] = hf_w

                for _w0 in range(4):
                    issue_gather(_w0)
                hT_next = hT_pool.tile([W, WPC * WIN], F16)

                def write_h(w):
                    VF = VF_W[w]
                    # transpose hT_next[w] and write it to DRAM. Deferred by
                    # one window (emitted after the NEXT window's zsum
                    # passes) so the in-order PE never stalls waiting for
                    # relu(w) on ACT; the last two windows are emitted inline
                    # since their writes gate the next iteration's gathers.
                    h_ps = tr_ps_pool.tile([WIN, 64], F16, tag="tr")
                    nc.tensor.transpose(h_ps[:VF, :],
                                        hT_next[:, w * WIN:w * WIN + VF],
                                        ident[:])
                    h_sb = spool.tile([WIN, 64], F16, tag="hnew")
                    nc.scalar.copy(h_sb[:VF, :], h_ps[:VF, :])
                    if single_core:
                        if w < WPC - 1:
                            nc.sync.dma_start(
                                h_fullA[it][w * WIN:w * WIN + VF, :W],
                                h_sb[:VF, :])
                            if w == WPC - 2:
                                # B gets windows 0..wpc-2 via one bulk
                                # copy (fix gathers wait for the last
                                # window anyway, so this is off the
                                # critical path)
                                nc.sync.dma_start(
                                    h_full[it][: (WPC - 1) * WIN, :],
                                    h_fullA[it][: (WPC - 1) * WIN, :])
                        else:
                            nc.sync.dma_start(
                                h_full[it][w * WIN:w * WIN + VF, :W],
                                h_sb[:VF, :])
                    else:
                        nc.sync.dma_start(
                            h_slice[it][w * WIN:w * WIN + VF, :W],
                            h_sb[:VF, :])

                def emit_y(w):
                    VF = VF_W[w]
                    y_ps = agg_ps_pool.tile([WIN, 1], F32, tag="a")
                    nc.tensor.matmul(y_ps[:VF, :],
                                     hT_next[:, w * WIN:w * WIN + VF],
                                     fc2[:], start=True, stop=True)
                    nc.vector.tensor_add(y_sb[:VF, w: w + 1], y_ps[:VF, :],
                                         fc2b[:VF, :])

                pending_tr = []
                pending_y = []
                for w in range(WPC):
                    nt = NT_W[w]
                    t0 = TOFF[w]
                    VF = VF_W[w]
                    zs = []
                    for et in range(nt):
                        t = t0 + et
                        z = zpool.tile([128, 65 * 64], F16)
                        # all-fp16 operands with packed (1,2) last dims hit
                        # the DVE 2x perf mode (stride-0 last dims do not)
                        zv = z[:].rearrange("p (c a b) -> p c a b", c=65, b=2)
                        full = w == 0 and plan.fix0_free
                        h_t = h_srcs[w] if (full or et < nt - 1) else h_fix[w]
                        e_t = et if (full or et < nt - 1) else 0
                        hs = h_t[:, e_t, :W] \
                            .rearrange("p (a b) -> p a b", b=2) \
                            .unsqueeze(1).broadcast_to((128, 65, 32, 2))
                        e2 = e2dup[:, t * 130:(t + 1) * 130] \
                            .rearrange("p (c b) -> p c b", b=2) \
                            .unsqueeze(2).broadcast_to((128, 65, 32, 2))
                        if et == nt - 1 and w == 0 \
                                and not plan.fix0_free:
                            # window 0's fix tile sits on the iteration
                            # boundary critical path: build it in KH-chunk
                            # pieces so pass 0 can start after the first one
                            for c0 in range(0, 65, 8):
                                c1 = min(c0 + 8, 65)
                                nc.vector.tensor_mul(
                                    zv[:, c0:c1, :, :], hs[:, c0:c1, :, :],
                                    e2[:, c0:c1, :, :])
                            zs.append(z)
                            continue
                        # offload part of the first tile of each window to
                        # the (idle) GPSIMD engine; DVE builds the rest.
                        # Not in iteration 0's first windows: Pool is still
                        # busy with the initial gather burst there.
                        if et == 0 and not (it == 0 and w < 3):
                            nc.gpsimd.tensor_mul(
                                zv[:, :26, :, :], hs[:, :26, :, :],
                                e2[:, :26, :, :])
                            nc.vector.tensor_mul(
                                zv[:, 26:, :, :], hs[:, 26:, :, :],
                                e2[:, 26:, :, :])
                        else:
                            nc.vector.tensor_mul(zv, hs, e2)
                        zs.append(z)
                    if w + 4 < WPC:
                        issue_gather(w + 4)
                    if it == 0:
                        if w == 0 and WPC > 1:
                            build_e2dup_win(1)
                        if w + 2 < WPC:
                            build_e2dup_win(w + 2)
                    # zsum in KH-chunk PSUM passes (back-to-back on PE; the
                    # drains pipeline on ACT), then the T-contract block.
                    # moving dim = VF (window's real node count); the last ci
                    # chunk is 64 rows (z is 65*64 wide).
                    zsum_sb = zsum_sb_pool.tile([128, NCH * VF], F16)
                    for p0 in range(0, NCH, KH):
                        p1 = min(p0 + KH, NCH)
                        zsum_ps = zsum_ps_pool.tile([128, KH * VF], F32)
                        for k in range(p0, p1):
                            cw = min(128, 65 * 64 - k * 128)
                            for et in range(nt):
                                nc.tensor.matmul(
                                    zsum_ps[:cw, (k - p0) * VF:
                                            (k - p0 + 1) * VF],
                                    zs[et][:, k * 128:k * 128 + cw],
                                    segT[:, (t0 + et) * WIN:
                                         (t0 + et) * WIN + VF],
                                    start=(et == 0), stop=(et == nt - 1))
                        # keep the DVE free for z-builds (critical engine) —
                        # drain PSUM on ACT
                        nc.scalar.copy(zsum_sb[:, p0 * VF:p1 * VF],
                                       zsum_ps[:, :(p1 - p0) * VF])
                    if pending_tr:
                        write_h(pending_tr.pop())
                    while len(pending_y) > 1:
                        emit_y(pending_y.pop(0))
                    agg_ps = agg_ps_pool.tile([64, VF], F32, tag="a")
                    for k in range(NCH):
                        cw = min(128, 65 * 64 - k * 128)
                        nc.tensor.matmul(agg_ps[:],
                                         Tsb[:cw, k * W:(k + 1) * W],
                                         zsum_sb[:cw, k * VF:(k + 1) * VF],
                                         start=(k == 0), stop=False)
                    nc.tensor.matmul(agg_ps[:], rootW[:],
                                     hT_cur[:, w * WIN:w * WIN + VF],
                                     start=False, stop=True)
                    nc.scalar.activation(hT_next[:, w * WIN:w * WIN + VF],
                                         agg_ps[:], Relu)
                    if it == DEP - 1:
                        pending_y.append(w)
                        if w == WPC - 1:
                            while pending_y:
                                emit_y(pending_y.pop(0))
                    else:
                        if w >= WPC - 2:
                            # w8's write gates the next iteration's main
                            # gathers, w9's its fix gathers: keep both inline
                            if pending_tr:
                                write_h(pending_tr.pop())
                            write_h(w)
                        else:
                            pending_tr.append(w)
                hT_cur = hT_next
                if it < DEP - 1 and not single_core:
                    nc.gpsimd.collective_compute(
                        "AllGather",
                        mybir.AluOpType.bypass,
                        ins=[h_slice[it][:].opt()],
                        outs=[h_full[it][:].opt()],
                        replica_groups=[list(range(NC_))],
                    )

            # ---- output ----
            y_view = y_d[:].rearrange("(w v) o -> v (w o)", w=WPC)
            nc.sync.dma_start(y_view, y_sb[:])

    nc.compile()
    return nc


def kernel(**inputs) -> np.ndarray:
    from concourse.bass_utils import run_bass_kernel_spmd

    plan = make_plan(**{k: np.asarray(v) for k, v in inputs.items()})
    nc = build_program(plan)
    core_ids = list(range(plan.n_cores))
    res = run_bass_kernel_spmd(nc, plan.in_maps, core_ids,
                               trace=bool(int(os.environ.get("KERNEL_TRACE", "0"))))
    y = np.concatenate([res.results[r]["y"] for r in range(plan.n_cores)], axis=0)
    out = y[plan.devnode]
    kernel.last_results = res
    kernel.last_plan = plan
    return out
